# revision 2
# baseline (speedup 1.0000x reference)
"""RWKV-4 block (nn_Block_5669356833485) Trainium2 Bass kernel.

B=8, T=2048, C=1024, HID=4096. B-sharded across 8 NeuronCores (1 batch/core).
Feature-major layout [C-partitions, T-free].

fp8e4 DoubleRow matmuls (256-wide contraction, 0.5 cyc/row). Weights are
host-scaled by 128 before fp8 quantization (their natural ~0.02 magnitude
falls in e4m3's subnormal range) and unscaled in the matmul epilogues.
Time-mix lerps are folded into the matmuls by doubling the contraction
against z and shifted-z (z pair tiles [128, 2, 2064], data offset 16, pair
stride %16==0 per DoubleRow requirements). LN stats are pipelined into the
producing loops; WKV (bf16 scans, fp32 state) interleaves per channel block
with the projections. ek/v/sr/srf stay in SBUF; only xT and x2 round-trip
through DRAM for the residual adds.
Self-contained: hardcodes shapes; no sibling imports.
"""
import os
import sys
sys.path.insert(0, '/opt/trn_rl_repo')

KPHASES = int(os.environ.get("KPHASES", "99"))

import numpy as np
import ml_dtypes

import concourse.bass as bass
from concourse import bacc
import concourse.mybir as mybir
import concourse.tile as tile
from concourse.bass_utils import run_bass_kernel_spmd

F32 = mybir.dt.float32
F32R = mybir.dt.float32r
BF16 = mybir.dt.bfloat16
FP8 = mybir.dt.float8e4
AL = mybir.AluOpType
AF = mybir.ActivationFunctionType
DR = mybir.MatmulPerfMode.DoubleRow

B, T, C, HID = 8, 2048, 1024, 4096
NCB = C // 128          # 8 channel blocks
NPR = NCB // 2          # 4 channel pair-blocks
NHB = HID // 128        # 32 hidden blocks
NHP = NHB // 2          # 16 hidden pair-blocks
NT = T // 512           # 4 n-slices of 512
NTB = T // 128          # 16 token blocks
PAD = 16                # z pair tiles: [128, 2, PAD+T]; pair stride %16==0
TP = T + PAD
EPS = 1e-5
WS = 128.0              # weight pre-quantization scale
INV = 1.0 / WS

# cst columns (per 128-partition, indexed by block)
CW = 0        # wbar          [0:8)   by cb
CEU = 8       # exp(tf)       [8:16)  by cb
CBK = 16      # bk            [16:24) by m
CBV = 24      # bv            [24:32) by m
CBR = 32      # br            [32:40) by m
CFT = 40      # ftmk          [40:48) by cb
CFT1 = 48     # 1-ftmk        [48:56) by cb
CBFR = 56     # bfr           [56:64) by m
CEPS = 64     # eps           col 64
CBFK = 72     # bfk           [72:104) by hb
NCOLS = 104


def _emit(nc):
    # ---------------- DRAM I/O ----------------
    x_d = nc.declare_dram_parameter("x", [T, C], F32, isOutput=False)
    wk_d = nc.declare_dram_parameter("wk", [128, 2 * 4 * 2 * 1024], FP8, isOutput=False)
    wv_d = nc.declare_dram_parameter("wv", [128, 2 * 4 * 2 * 1024], FP8, isOutput=False)
    wr_d = nc.declare_dram_parameter("wr", [128, 2 * 4 * 2 * 1024], FP8, isOutput=False)
    wo_d = nc.declare_dram_parameter("wo", [2, 128, 4 * 2 * 1024], FP8, isOutput=False)
    fwr_d = nc.declare_dram_parameter("fwr", [2, 128, 4 * 2 * 1024], FP8, isOutput=False)
    fwk_d = nc.declare_dram_parameter("fwk", [2, 8, 128, 4 * 2 * 512], FP8, isOutput=False)
    fwv_d = nc.declare_dram_parameter("fwv", [2, 128, 16 * 2 * 1024], FP8, isOutput=False)
    cst_d = nc.declare_dram_parameter("cst", [128, NCOLS], F32, isOutput=False)
    ones1_d = nc.declare_dram_parameter("ones1", [128, 1], F32R, isOutput=False)
    onesb_d = nc.declare_dram_parameter("onesb", [1, 128], BF16, isOutput=False)
    ident_d = nc.declare_dram_parameter("ident", [128, 128], F32, isOutput=False)
    out_d = nc.declare_dram_parameter("out", [T, C], F32, isOutput=True)

    # DRAM scratch (per-cb granularity for fine deps)
    xT_sp = [nc.dram_tensor(f"xT_sp{i}", [128, T], F32) for i in range(NCB)]
    x2_sp = [nc.dram_tensor(f"x2_sp{i}", [128, T], F32) for i in range(NCB)]

    import contextlib

    with tile.TileContext(nc, pool_alloc_mode="queue") as tc:
        with tc.tile_pool(name="pc", bufs=1) as pc:
            cst = pc.tile([128, NCOLS], F32)
            nc.sync.dma_start(out=cst, in_=cst_d[:])
            ones1 = pc.tile([128, 1], F32R)
            nc.sync.dma_start(out=ones1, in_=ones1_d[:])
            onesb = pc.tile([1, 128], BF16)
            nc.sync.dma_start(out=onesb, in_=onesb_d[:])
            ident = pc.tile([128, 128], F32)
            nc.sync.dma_start(out=ident, in_=ident_d[:])
            ones_bf = pc.tile([128, T], BF16)
            nc.vector.memset(ones_bf, 1.0)

            def col(j):
                return cst[:, j:j + 1]

            # ---- incremental LN stats: two [1, T] psum tiles ----
            def ln_contrib(stat_ps, pool_tmp, src_f32r, cb, sl, tag):
                """Add channel-block cb's contribution for column slice sl."""
                mean_ps, msq_ps = stat_ps
                w = sl.stop - sl.start
                sq = pool_tmp.tile([128, w], F32R, tag=tag, bufs=3)
                nc.scalar.activation(sq, src_f32r.bitcast(F32)[:, sl], AF.Square)
                nc.tensor.matmul(mean_ps[:, sl], ones1, src_f32r[:, sl],
                                 start=(cb == 0), stop=(cb == NCB - 1))
                nc.tensor.matmul(msq_ps[:, sl], ones1, sq,
                                 start=(cb == 0), stop=(cb == NCB - 1))

            def ln_to_sbuf(stat_ps, pool_stat):
                mean_ps, msq_ps = stat_ps
                mean = pool_stat.tile([1, T], BF16, tag="mean_sb")
                msq = pool_stat.tile([1, T], BF16, tag="msq_sb")
                nc.scalar.mul(mean, mean_ps, 1.0 / C)
                nc.scalar.mul(msq, msq_ps, 1.0 / C)
                return mean, msq

            def ln_finish(mean, msq, pool_stat, uid):
                var = pool_stat.tile([1, T], BF16, tag="var_sb")
                nc.vector.tensor_mul(var, mean, mean)
                nc.vector.tensor_sub(var, msq, var)
                lnv = pool_stat.tile([1, T], BF16, tag="msq_sb", name=f"lnv{uid}")
                nc.scalar.activation(lnv, var, AF.Ln,
                                     bias=cst[0:1, CEPS:CEPS + 1], scale=1.0)
                rstd = pool_stat.tile([1, T], BF16, tag="var_sb", name=f"rstd{uid}")
                nc.scalar.activation(rstd, lnv, AF.Exp, bias=0.0, scale=-0.5)
                mrstd = pool_stat.tile([1, T], BF16, tag="mrstd_sb")
                nc.vector.tensor_mul(mrstd, mean, rstd)
                rstd_b = pool_stat.tile([128, T], BF16, tag="rstd_b")
                mrstd_b = pool_stat.tile([128, T], BF16, tag="mrstd_b")
                with tc.tile_pool(name=f"ps_bc{uid}", bufs=2, space="PSUM") as ps_bc:
                    for (src_s, dst) in ((rstd, rstd_b), (mrstd, mrstd_b)):
                        for n in range(NT):
                            sl = slice(n * 512, (n + 1) * 512)
                            bc = ps_bc.tile([128, 512], F32, tag="bc")
                            nc.tensor.matmul(bc, onesb, src_s[:, sl],
                                             start=True, stop=True)
                            if n % 2 == 0:
                                nc.scalar.copy(dst[:, sl], bc)
                            else:
                                nc.vector.tensor_copy(out=dst[:, sl], in_=bc)
                return rstd_b, mrstd_b

            # z pair tiles live through phase B (attention)
            es_z = contextlib.ExitStack()
            p_z = es_z.enter_context(tc.tile_pool(name="p_z", bufs=1, side="right"))
            z8 = [p_z.tile([128, 2, TP], FP8, tag=f"z{pr}", name=f"z{pr}")
                  for pr in range(NPR)]
            for pr in range(NPR):
                nc.vector.memset(z8[pr][:, :, 0:PAD], 0.0)

            # ================= PHASE A: load, transpose, LN1, z =================
            with tc.tile_pool(name="p_xT", bufs=1) as p_xT:
                xT = [p_xT.tile([128, T], F32R, tag=f"xT{cb}", name=f"xT{cb}")
                      for cb in range(NCB)]
                with tc.tile_pool(name="p_tmpA", bufs=1) as p_tmpA, \
                     tc.tile_pool(name="p_statA", bufs=1) as p_statA:
                    meanA = p_statA.tile([1, T], BF16, tag="mean_sb")
                    msqA = p_statA.tile([1, T], BF16, tag="msq_sb")
                    with tc.tile_pool(name="p_ld", bufs=3) as p_ld, \
                         tc.tile_pool(name="ps_stA", bufs=2,
                                      space="PSUM") as ps_stA, \
                         tc.tile_pool(name="ps_tr", bufs=4, space="PSUM") as ps_tr:
                        for tb in range(NTB):
                            xt = p_ld.tile([128, C], F32, tag="xtok")
                            nc.sync.dma_start(out=xt,
                                              in_=x_d[tb * 128:(tb + 1) * 128, :])
                            for cb in range(NCB):
                                pt = ps_tr.tile([128, 128], F32, tag="tr")
                                nc.tensor.transpose(
                                    pt, xt[:, cb * 128:(cb + 1) * 128], ident)
                                dst = xT[cb][:, tb * 128:(tb + 1) * 128]
                                if (tb + cb) % 2 == 0:
                                    nc.scalar.copy(dst, pt)
                                else:
                                    nc.vector.tensor_copy(out=dst, in_=pt)
                            if tb % 4 == 3:
                                n = tb // 4
                                sl = slice(n * 512, (n + 1) * 512)
                                mp = ps_stA.tile([1, 512], F32, tag="mA",
                                                 name=f"mA{n}")
                                qp = ps_stA.tile([1, 512], F32, tag="qA",
                                                 name=f"qA{n}")
                                for cb in range(NCB):
                                    sq = p_tmpA.tile([128, 512], F32R,
                                                     tag="sqA", bufs=3)
                                    nc.scalar.activation(
                                        sq, xT[cb].bitcast(F32)[:, sl],
                                        AF.Square)
                                    nc.tensor.matmul(
                                        mp, ones1, xT[cb][:, sl],
                                        start=(cb == 0), stop=(cb == NCB - 1))
                                    nc.tensor.matmul(
                                        qp, ones1, sq,
                                        start=(cb == 0), stop=(cb == NCB - 1))
                                nc.scalar.mul(meanA[:, sl], mp, 1.0 / C)
                                nc.scalar.mul(msqA[:, sl], qp, 1.0 / C)
                    for cb in range(NCB):
                        nc.sync.dma_start(out=xT_sp[cb][:], in_=xT[cb].bitcast(F32))
                    rstd_b, mrstd_b = ln_finish(meanA, msqA, p_statA, "A")
                    for cb in range(NCB):
                        pr, j = cb // 2, cb % 2
                        zt = p_tmpA.tile([128, T], F32, tag="zt", bufs=2)
                        nc.vector.tensor_mul(zt, xT[cb].bitcast(F32), rstd_b)
                        dst = z8[pr][:, j, PAD:PAD + T]
                        if cb % 2 == 0:
                            nc.vector.tensor_sub(dst, zt, mrstd_b)
                        else:
                            nc.gpsimd.tensor_sub(dst, zt, mrstd_b)

            # attention weights (opened after phase A frees xT)
            es_wo = contextlib.ExitStack()
            p_wo = es_wo.enter_context(tc.tile_pool(name="p_wo", bufs=1))
            wo = [p_wo.tile([128, 4, 2, 1024], FP8, tag=f"wo{i}",
                            name=f"wo{i}") for i in range(2)]
            nc.sync.dma_start(out=wo[0], in_=wo_d[0])
            nc.sync.dma_start(out=wo[1], in_=wo_d[1])
            es_w = contextlib.ExitStack()
            p_w = es_w.enter_context(tc.tile_pool(name="p_w", bufs=1, side="right"))
            wk = p_w.tile([128, 2, 4, 2, 1024], FP8, tag="wk")
            wv = p_w.tile([128, 2, 4, 2, 1024], FP8, tag="wv")
            wr = p_w.tile([128, 2, 4, 2, 1024], FP8, tag="wr")
            nc.sync.dma_start(out=wk, in_=wk_d[:])
            nc.sync.dma_start(out=wv, in_=wv_d[:])
            nc.sync.dma_start(out=wr, in_=wr_d[:])

            # ============ PHASE B: k/v/r projections + WKV per m ============
            es_sry = contextlib.ExitStack()
            p_sry = es_sry.enter_context(tc.tile_pool(name="p_sry", bufs=1))
            sryh = [p_sry.tile([128, 2, T], FP8, tag=f"sryh{pr}", name=f"sryh{pr}")
                    for pr in range(NPR)]
            sryl = [p_sry.tile([128, 2, T], FP8, tag=f"sryl{pr}", name=f"sryl{pr}")
                    for pr in range(NPR)]

            def zsl(k2, a, n):
                # a=0: current tokens; a=1: shifted by one
                lo = PAD - a + n * 512
                return z8[k2][:, :, lo:lo + 512]

            with tc.tile_pool(name="p_kvs", bufs=2) as p_kvs, \
                 tc.tile_pool(name="p_wt", bufs=2) as p_wt, \
                 tc.tile_pool(name="ps_mm", bufs=4, space="PSUM") as ps_mm:
                wkv_state = {}

                def wkv_front(m, ek, vv):
                    wrow = p_wt.tile([128, T], BF16, tag="wrow", name=f"wr{m}")
                    nc.vector.tensor_scalar(out=wrow, in0=ones_bf,
                                            scalar1=col(CW + m),
                                            scalar2=None, op0=AL.mult)
                    ekv = p_wt.tile([128, T], BF16, tag="ekv", name=f"ekv{m}")
                    nc.gpsimd.tensor_mul(ekv, ek, vv)
                    A = p_wt.tile([128, T + 1], BF16, tag="A", name=f"A{m}")
                    Bt = p_wt.tile([128, T + 1], BF16, tag="B", name=f"B{m}")
                    nc.vector.memset(A[:, 0:1], 0.0)
                    nc.vector.memset(Bt[:, 0:1], 0.0)
                    nc.vector.tensor_tensor_scan(
                        out=A[:, 1:T + 1], data0=wrow, data1=ekv,
                        initial=0.0, op0=AL.mult, op1=AL.add)
                    nc.vector.tensor_tensor_scan(
                        out=Bt[:, 1:T + 1], data0=wrow, data1=ek,
                        initial=0.0, op0=AL.mult, op1=AL.add)
                    nc.vector.scalar_tensor_tensor(
                        out=A[:, 0:T], in0=ekv, scalar=col(CEU + m),
                        in1=A[:, 0:T], op0=AL.mult, op1=AL.add)
                    nc.vector.scalar_tensor_tensor(
                        out=Bt[:, 0:T], in0=ek, scalar=col(CEU + m),
                        in1=Bt[:, 0:T], op0=AL.mult, op1=AL.add)
                    rec = p_wt.tile([128, T], BF16, tag="rec", name=f"rec{m}")
                    with nc.allow_low_precision(reason="wkv ratio bf16"):
                        nc.vector.reciprocal(rec, Bt[:, 0:T])
                    return A, rec

                def wkv_tail(m, A, rec, sr):
                    pr_m, j_m = m // 2, m % 2
                    y = p_wt.tile([128, T], BF16, tag="y", name=f"y{m}")
                    nc.gpsimd.tensor_mul(y, A[:, 0:T], rec)
                    sy = p_wt.tile([128, T], BF16, tag="sy", name=f"sy{m}")
                    nc.gpsimd.tensor_mul(sy, y, sr)
                    nc.scalar.copy(sryh[pr_m][:, j_m, :], sy)
                    nc.vector.tensor_sub(sryl[pr_m][:, j_m, :], sy,
                                         sryh[pr_m][:, j_m, :])

                for m in (range(NCB) if KPHASES >= 2 else ()):
                    ek = p_kvs.tile([128, T], BF16, tag="ek", name=f"ek{m}")
                    vv = p_kvs.tile([128, T], BF16, tag="vv", name=f"vv{m}")
                    sr = p_kvs.tile([128, T], BF16, tag="sr", name=f"sr{m}")
                    for (wt, dst, act, bcol) in (
                            (wk, ek, AF.Exp, CBK), (wv, vv, AF.Identity, CBV),
                            (wr, sr, AF.Sigmoid, CBR)):
                        for n in range(NT):
                            pmm = ps_mm.tile([128, 512], F32, tag="pmm")
                            for a in range(2):
                                for k2 in range(NPR):
                                    nc.tensor.matmul(
                                        pmm, wt[:, a, k2, :,
                                                m * 128:(m + 1) * 128],
                                        zsl(k2, a, n),
                                        start=(a == 0 and k2 == 0),
                                        stop=(a == 1 and k2 == NPR - 1),
                                        perf_mode=DR)
                            dsl = dst[:, n * 512:(n + 1) * 512]
                            nc.scalar.activation(dsl, pmm, act,
                                                 bias=col(bcol + m), scale=INV)
                    if KPHASES >= 3:
                        A, rec = wkv_front(m, ek, vv)
                        wkv_state[m] = (A, rec, sr)
                        if m >= 1:
                            wkv_tail(m - 1, *wkv_state.pop(m - 1))
                if KPHASES >= 3:
                    wkv_tail(NCB - 1, *wkv_state.pop(NCB - 1))

            es_w.close()
            es_z.close()

            # ===== PHASE C: out-proj + residual -> x2, fused LN2 stats =====
            es_x2 = contextlib.ExitStack()
            p_x2 = es_x2.enter_context(tc.tile_pool(name="p_x2", bufs=1))
            x2 = [p_x2.tile([128, T], F32R, tag=f"x2_{cb}", name=f"x2_{cb}")
                  for cb in range(NCB)]
            es_z2 = contextlib.ExitStack()
            p_z2 = es_z2.enter_context(tc.tile_pool(name="p_z2", bufs=1,
                                                    side="right"))
            z2t = [p_z2.tile([128, T + 1], BF16, tag=f"z2_{cb}", name=f"z2_{cb}")
                   for cb in range(NCB)]
            with tc.tile_pool(name="p_xr", bufs=2) as p_xr, \
                 tc.tile_pool(name="p_tmpD", bufs=1) as p_tmpD, \
                 tc.tile_pool(name="p_statD", bufs=1) as p_statD:
              with tc.tile_pool(name="ps_mo", bufs=2, space="PSUM") as ps_mo:
                for m in (range(NCB) if KPHASES >= 4 else ()):
                    xr = p_xr.tile([128, T], F32, tag="xr")
                    nc.sync.dma_start(out=xr, in_=xT_sp[m][:])
                    for n in range(NT):
                        sl = slice(n * 512, (n + 1) * 512)
                        pmm = ps_mo.tile([128, 512], F32, tag="pmo")
                        first = True
                        for (wi, ss) in ((0, sryh), (1, sryh), (0, sryl)):
                            for k2 in range(NPR):
                                nc.tensor.matmul(
                                    pmm, wo[wi][:, k2, :, m * 128:(m + 1) * 128],
                                    ss[k2][:, :, sl],
                                    start=first,
                                    stop=(wi == 0 and ss is sryl
                                          and k2 == NPR - 1),
                                    perf_mode=DR)
                                first = False
                        nc.vector.scalar_tensor_tensor(
                            out=x2[m][:, sl], in0=pmm, scalar=INV,
                            in1=xr[:, sl], op0=AL.mult, op1=AL.add)
                    nc.sync.dma_start(out=x2_sp[m][:], in_=x2[m].bitcast(F32))
              # ---- LN2 stats + finish -> z2 (plain bf16, col 0 zero) ----
              if True:
                if KPHASES >= 5:
                    with tc.tile_pool(name="ps_stD", bufs=1,
                                      space="PSUM") as ps_stD:
                        stat_ps2 = (ps_stD.tile([1, T], F32, tag="meanD", name="meanD"),
                                    ps_stD.tile([1, T], F32, tag="msqD", name="msqD"))
                        for n in range(NT):
                            sl2 = slice(n * 512, (n + 1) * 512)
                            for cb in range(NCB):
                                ln_contrib(stat_ps2, p_tmpD, x2[cb], cb, sl2,
                                           "sqD")
                        meanD, msqD = ln_to_sbuf(stat_ps2, p_statD)
                    rstd_b2, mrstd_b2 = ln_finish(meanD, msqD, p_statD, "D")
                    for cb in range(NCB):
                        nc.vector.memset(z2t[cb][:, 0:1], 0.0)
                        zt = p_tmpD.tile([128, T], F32, tag="zt2", bufs=1)
                        nc.vector.tensor_mul(zt, x2[cb].bitcast(F32), rstd_b2)
                        dst = z2t[cb][:, 1:T + 1]
                        if cb % 2 == 0:
                            nc.vector.tensor_sub(dst, zt, mrstd_b2)
                        else:
                            nc.gpsimd.tensor_sub(dst, zt, mrstd_b2)
            es_x2.close()
            es_sry.close()
            es_wo.close()

            # FFN weights: fwv hi/lo resident fp8; fwr till srf; fwk streamed
            es_fw = contextlib.ExitStack()
            p_fw = es_fw.enter_context(tc.tile_pool(name="p_fw", bufs=1))
            fwv = [p_fw.tile([128, 16, 2, 1024], FP8, tag=f"fwv{i}",
                             name=f"fwv{i}") for i in range(2)]
            if KPHASES >= 5:
                nc.sync.dma_start(out=fwv[0], in_=fwv_d[0])
                nc.sync.dma_start(out=fwv[1], in_=fwv_d[1])

            # ============ PHASE E: xf lerp (f_tmk == f_tmr), fWr -> srf ========
            es_xf = contextlib.ExitStack()
            p_xf = es_xf.enter_context(tc.tile_pool(name="p_xf", bufs=1))
            xfh = [p_xf.tile([128, 2, T], FP8, tag=f"xfh{pr}", name=f"xfh{pr}")
                   for pr in range(NPR)]
            xfl = [p_xf.tile([128, 2, T], FP8, tag=f"xfl{pr}", name=f"xfl{pr}")
                   for pr in range(NPR)]
            with tc.tile_pool(name="p_te", bufs=3) as p_te:
                for cb in (range(NCB) if KPHASES >= 6 else ()):
                    pr, j = cb // 2, cb % 2
                    t1 = p_te.tile([128, T], BF16, tag="t1")
                    nc.scalar.mul(t1, z2t[cb][:, 0:T], col(CFT1 + cb))
                    xfb = p_te.tile([128, T], BF16, tag="xfb")
                    nc.vector.scalar_tensor_tensor(
                        out=xfb, in0=z2t[cb][:, 1:T + 1],
                        scalar=col(CFT + cb), in1=t1, op0=AL.mult, op1=AL.add)
                    nc.scalar.copy(xfh[pr][:, j, :], xfb)
                    nc.gpsimd.tensor_sub(xfl[pr][:, j, :], xfb, xfh[pr][:, j, :])
            es_z2.close()

            es_srf = contextlib.ExitStack()
            p_srf = es_srf.enter_context(tc.tile_pool(name="p_srf", bufs=1))
            srf = [p_srf.tile([128, T], FP8, tag=f"srf{m}", name=f"srf{m}")
                   for m in range(NCB)]
            with tc.tile_pool(name="p_fwr", bufs=1) as p_fwr, \
                 tc.tile_pool(name="ps_fr", bufs=4, space="PSUM") as ps_fr:
                fwr = [p_fwr.tile([128, 4, 2, 1024], FP8, tag=f"fwr{i}",
                                  name=f"fwr{i}") for i in range(2)]
                if KPHASES >= 6:
                    nc.sync.dma_start(out=fwr[0], in_=fwr_d[0])
                    nc.sync.dma_start(out=fwr[1], in_=fwr_d[1])
                for m in (range(NCB) if KPHASES >= 6 else ()):
                    for n in range(NT):
                        pmm = ps_fr.tile([128, 512], F32, tag="pfr")
                        first = True
                        for (wi, xs) in ((0, xfh), (1, xfh), (0, xfl)):
                            for k2 in range(NPR):
                                nc.tensor.matmul(
                                    pmm, fwr[wi][:, k2, :, m * 128:(m + 1) * 128],
                                    xs[k2][:, :, n * 512:(n + 1) * 512],
                                    start=first,
                                    stop=(wi == 0 and xs is xfl and k2 == NPR - 1),
                                    perf_mode=DR)
                                first = False
                        nc.scalar.activation(srf[m][:, n * 512:(n + 1) * 512],
                                             pmm, AF.Sigmoid, bias=col(CBFR + m),
                                             scale=INV)

            # ============ PHASE F: FFN k/v matmuls + output ============
            # 3-pass residual fp8: W*x ~ Wh*xh + Wl*xh + Wh*xl
            with tc.tile_pool(name="p_fwkg", bufs=2) as p_fwkg, \
                 tc.tile_pool(name="p_kk", bufs=1) as p_kk, \
                 tc.tile_pool(name="p_rl", bufs=4) as p_rl, \
                 tc.tile_pool(name="p_x2c", bufs=3) as p_x2c, \
                 tc.tile_pool(name="p_fin", bufs=2) as p_fin, \
                 tc.tile_pool(name="p_ost", bufs=1) as p_ost, \
                 tc.tile_pool(name="ps_fk", bufs=3, space="PSUM") as ps_fk, \
                 tc.tile_pool(name="ps_fo", bufs=2, space="PSUM") as ps_fo, \
                 tc.tile_pool(name="ps_ot", bufs=2, space="PSUM") as ps_ot:
                for n in (range(NT) if KPHASES >= 7 else ()):
                    sl = slice(n * 512, (n + 1) * 512)
                    kkh = [p_kk.tile([128, 2, 512], FP8, tag=f"kkh{hp}",
                                     name=f"kkh{hp}_{n}") for hp in range(NHP)]
                    kkl = [p_kk.tile([128, 2, 512], FP8, tag=f"kkl{hp}",
                                     name=f"kkl{hp}_{n}") for hp in range(NHP)]
                    for g in range(8):
                        fg = [p_fwkg.tile([128, 4, 2, 512], FP8, tag=f"fwkg{i}",
                                          name=f"fwkg{i}_{n}_{g}")
                              for i in range(2)]
                        nc.sync.dma_start(out=fg[0], in_=fwk_d[0, g])
                        nc.sync.dma_start(out=fg[1], in_=fwk_d[1, g])
                        for i in range(4):
                            hb = g * 4 + i
                            hp, jh = hb // 2, hb % 2
                            pkk = ps_fk.tile([128, 512], F32, tag="pkk")
                            first = True
                            for (wi, xs) in ((0, xfh), (1, xfh), (0, xfl)):
                                for k2 in range(NPR):
                                    nc.tensor.matmul(
                                        pkk,
                                        fg[wi][:, k2, :, i * 128:(i + 1) * 128],
                                        xs[k2][:, :, sl],
                                        start=first,
                                        stop=(wi == 0 and xs is xfl
                                              and k2 == NPR - 1),
                                        perf_mode=DR)
                                    first = False
                            rl = p_rl.tile([128, 512], BF16, tag="rl")
                            if hb % 2 == 0:
                                nc.scalar.activation(rl, pkk, AF.Relu,
                                                     bias=col(CBFK + hb),
                                                     scale=INV)
                            else:
                                nc.vector.tensor_scalar(
                                    out=rl, in0=pkk, scalar1=INV,
                                    scalar2=0.0, op0=AL.mult, op1=AL.max)
                            t2 = p_rl.tile([128, 512], BF16, tag="t2")
                            nc.vector.tensor_mul(t2, rl, rl)
                            dh = kkh[hp][:, jh, :]
                            if hb % 2 == 0:
                                nc.scalar.copy(dh, t2)
                            else:
                                nc.vector.tensor_copy(out=dh, in_=t2)
                            nc.gpsimd.tensor_sub(kkl[hp][:, jh, :], t2, dh)
                    osts = [p_ost.tile([128, C], F32, tag=f"ost{j}",
                                       name=f"ost{n}_{j}") for j in range(4)]
                    for m in range(NCB):
                        po = ps_fo.tile([128, 512], F32, tag="po")
                        first = True
                        for (wi, ks) in ((0, kkh), (1, kkh), (0, kkl)):
                            for hp in range(NHP):
                                nc.tensor.matmul(
                                    po, fwv[wi][:, hp, :, m * 128:(m + 1) * 128],
                                    ks[hp],
                                    start=first,
                                    stop=(wi == 0 and ks is kkl
                                          and hp == NHP - 1),
                                    perf_mode=DR)
                                first = False
                        x2c = p_x2c.tile([128, 512], F32, tag="x2c")
                        nc.sync.dma_start(out=x2c, in_=x2_sp[m][:, sl])
                        rkv = p_fin.tile([128, 512], F32, tag="rkv")
                        nc.vector.scalar_tensor_tensor(
                            out=rkv, in0=po, scalar=INV, in1=srf[m][:, sl],
                            op0=AL.mult, op1=AL.mult)
                        fin = p_fin.tile([128, 512], F32, tag="fin")
                        if m % 2 == 0:
                            nc.gpsimd.tensor_add(fin, rkv, x2c)
                        else:
                            nc.vector.tensor_add(fin, rkv, x2c)
                        for j in range(4):
                            pt = ps_ot.tile([128, 128], F32, tag="ptr")
                            nc.tensor.transpose(pt, fin[:, j * 128:(j + 1) * 128],
                                                ident)
                            dst = osts[j][:, m * 128:(m + 1) * 128]
                            if (m + j) % 2 == 0:
                                nc.scalar.copy(dst, pt)
                            else:
                                nc.vector.tensor_copy(out=dst, in_=pt)
                    for j in range(4):
                        tb = n * 4 + j
                        nc.sync.dma_start(out=out_d[tb * 128:(tb + 1) * 128, :],
                                          in_=osts[j])
            es_srf.close()
            es_xf.close()
            es_fw.close()
    nc.finalize()
    return nc


_PROG = None


def _get_prog():
    global _PROG
    if _PROG is None:
        nc = bacc.Bacc()
        _PROG = _emit(nc)
    return _PROG


def _pair_w(WT, M_out):
    """WT: [K_in, M_out] fp8 (lhsT layout) -> [128, K_in//256, 2, M_out] flat."""
    K_in = WT.shape[0]
    npr = K_in // 256
    return np.ascontiguousarray(
        WT.reshape(npr, 2, 128, M_out).transpose(2, 0, 1, 3).reshape(128, -1))


def _q8_hl(WT):
    """WT f32 (pre-scaled by WS) -> (hi, lo) fp8 arrays."""
    f8 = ml_dtypes.float8_e4m3
    Ws = np.asarray(WT, np.float32) * np.float32(WS)
    assert np.abs(Ws).max() < 230.0
    hi = Ws.astype(f8)
    lo = (Ws - hi.astype(np.float32)).astype(f8)
    return hi, lo


def _fwk_hl(WT):
    """WT: [C, HID] -> fp8 [2(hl), 8(g), 128, 4(k2)*2(j)*512]; g = hid cols 512g."""
    hi, lo = _q8_hl(WT)
    out = []
    for W8 in (hi, lo):
        # pair layout per group: [128, k2, j, 512]
        Wp = W8.reshape(4, 2, 128, HID)  # [k2, j, c128, h]
        out.append(np.stack(
            [np.ascontiguousarray(
                Wp[:, :, :, g * 512:(g + 1) * 512].transpose(2, 0, 1, 3)
                .reshape(128, -1)) for g in range(8)]))
    return np.ascontiguousarray(np.stack(out))


def _fwv_hl(WT):
    """WT: [HID, C] -> fp8 [2(hl), 128, 16*2*1024] pair layout."""
    hi, lo = _q8_hl(WT)
    return np.ascontiguousarray(np.stack([_pair_w(W8, C) for W8 in (hi, lo)]))


def _q8s(W):
    """Scale by WS, quantize to fp8e4 (checks range)."""
    f8 = ml_dtypes.float8_e4m3
    Ws = np.asarray(W, np.float32) * np.float32(WS)
    assert np.abs(Ws).max() < 230.0, "weight scale overflow"
    return Ws.astype(f8)


def _prep_inputs(x, ln1_g, ln1_b, ln2_g, ln2_b, time_decay, time_first,
                 tmk, tmv, tmr, Wk, Wv, Wr, Wo, f_tmk, f_tmr, fWk, fWr, fWv):
    f32 = np.float32
    x = np.asarray(x, f32)
    g1 = np.asarray(ln1_g, f32); b1 = np.asarray(ln1_b, f32)
    g2 = np.asarray(ln2_g, f32); b2 = np.asarray(ln2_b, f32)
    td = np.asarray(time_decay, np.float64); tf = np.asarray(time_first, np.float64)
    tmk = np.asarray(tmk, f32).reshape(C); tmv = np.asarray(tmv, f32).reshape(C)
    tmr = np.asarray(tmr, f32).reshape(C)
    ftmk = np.asarray(f_tmk, f32).reshape(C); ftmr = np.asarray(f_tmr, f32).reshape(C)
    assert np.array_equal(ftmk, ftmr), "kernel assumes f_tmk == f_tmr"
    Wk = np.asarray(Wk, f32); Wv = np.asarray(Wv, f32); Wr = np.asarray(Wr, f32)
    Wo = np.asarray(Wo, f32); fWk = np.asarray(fWk, f32); fWr = np.asarray(fWr, f32)
    fWv = np.asarray(fWv, f32)

    Wk1 = Wk * g1[None, :]; Wv1 = Wv * g1[None, :]; Wr1 = Wr * g1[None, :]
    bk = Wk @ b1; bv = Wv @ b1; br = Wr @ b1
    fWk1 = fWk * g2[None, :]; fWr1 = fWr * g2[None, :]
    bfk = fWk @ b2; bfr = fWr @ b2
    assert np.allclose(bfk, 0.0), "kernel assumes zero ln2 beta for relu path"

    wbar = np.exp(-np.exp(td)).astype(f32)
    eu = np.exp(tf).astype(f32)

    def packc(v):
        return np.asarray(v, f32).reshape(-1, 128).T

    cst = np.zeros((128, NCOLS), f32)
    cst[:, CW:CW + 8] = packc(wbar)
    cst[:, CEU:CEU + 8] = packc(eu)
    cst[:, CBK:CBK + 8] = packc(bk)
    cst[:, CBV:CBV + 8] = packc(bv)
    cst[:, CBR:CBR + 8] = packc(br)
    cst[:, CFT:CFT + 8] = packc(ftmk)
    cst[:, CFT1:CFT1 + 8] = packc(1 - ftmk)
    cst[:, CBFR:CBFR + 8] = packc(bfr)
    cst[:, CEPS] = EPS
    cst[:, CBFK:CBFK + 32] = packc(bfk)

    def lerp_pair(W1, tm):
        # [128, 2(ab), 4(k2), 2(j), 1024] flat; a=0: W*tm, a=1: W*(1-tm)
        Wa = _pair_w(_q8s((W1 * tm[None, :]).T), C)
        Wb = _pair_w(_q8s((W1 * (1 - tm)[None, :]).T), C)
        return np.ascontiguousarray(
            np.stack([Wa.reshape(128, 4, 2, 1024),
                      Wb.reshape(128, 4, 2, 1024)], axis=1).reshape(128, -1))

    shared = {
        "wk": lerp_pair(Wk1, tmk),
        "wv": lerp_pair(Wv1, tmv),
        "wr": lerp_pair(Wr1, tmr),
        "wo": _fwv_hl(Wo.T),
        "fwr": _fwv_hl(fWr1.T),
        "fwk": _fwk_hl(fWk1.T),
        "fwv": _fwv_hl(fWv.T),
        "cst": cst,
        "ones1": np.ones((128, 1), f32),
        "onesb": np.ones((1, 128), ml_dtypes.bfloat16),
        "ident": np.eye(128, dtype=f32),
    }
    in_maps = [dict(shared, x=np.ascontiguousarray(x[b])) for b in range(B)]
    return in_maps


def _run(in_maps, trace=False, **kw):
    nc = _get_prog()
    res = run_bass_kernel_spmd(nc, in_maps, core_ids=list(range(B)), trace=trace,
                               **kw)
    out = np.stack([np.asarray(res.results[b]["out"]) for b in range(B)], axis=0)
    return out.astype(np.float32), res


def kernel(*a, **kw):
    out, _ = _run(_prep_inputs(*a, **kw))
    return out


if __name__ == "__main__":
    _get_prog()
    print("program built ok")


# revision 3
# speedup vs baseline: 1.0375x; 1.0375x over previous
"""RWKV-4 block (nn_Block_5669356833485) Trainium2 Bass kernel.

B=8, T=2048, C=1024, HID=4096. B-sharded across 8 NeuronCores (1 batch/core).
Feature-major layout [C-partitions, T-free].

fp8e4 DoubleRow matmuls (256-wide contraction, 0.5 cyc/row). Weights are
host-scaled by 128 before fp8 quantization (their natural ~0.02 magnitude
falls in e4m3's subnormal range) and unscaled in the matmul epilogues.
Time-mix lerps are folded into the matmuls by doubling the contraction
against z and shifted-z (z pair tiles [128, 2, 2064], data offset 16, pair
stride %16==0 per DoubleRow requirements). LN stats are pipelined into the
producing loops; WKV (bf16 scans, fp32 state) interleaves per channel block
with the projections. ek/v/sr/srf stay in SBUF; only xT and x2 round-trip
through DRAM for the residual adds.
Self-contained: hardcodes shapes; no sibling imports.
"""
import os
import sys
sys.path.insert(0, '/opt/trn_rl_repo')

KPHASES = int(os.environ.get("KPHASES", "99"))

import numpy as np
import ml_dtypes

import concourse.bass as bass
from concourse import bacc
import concourse.mybir as mybir
import concourse.tile as tile
from concourse.bass_utils import run_bass_kernel_spmd

F32 = mybir.dt.float32
F32R = mybir.dt.float32r
BF16 = mybir.dt.bfloat16
FP8 = mybir.dt.float8e4
AL = mybir.AluOpType
AF = mybir.ActivationFunctionType
DR = mybir.MatmulPerfMode.DoubleRow

B, T, C, HID = 8, 2048, 1024, 4096
NCB = C // 128          # 8 channel blocks
NPR = NCB // 2          # 4 channel pair-blocks
NHB = HID // 128        # 32 hidden blocks
NHP = NHB // 2          # 16 hidden pair-blocks
NT = T // 512           # 4 n-slices of 512
NTB = T // 128          # 16 token blocks
PAD = 16                # z pair tiles: [128, 2, PAD+T]; pair stride %16==0
TP = T + PAD
EPS = 1e-5
WS = 128.0              # weight pre-quantization scale
INV = 1.0 / WS

# cst columns (per 128-partition, indexed by block)
CW = 0        # wbar          [0:8)   by cb
CEU = 8       # exp(tf)       [8:16)  by cb
CBK = 16      # bk            [16:24) by m
CBV = 24      # bv            [24:32) by m
CBR = 32      # br            [32:40) by m
CFT = 40      # ftmk          [40:48) by cb
CFT1 = 48     # 1-ftmk        [48:56) by cb
CBFR = 56     # bfr           [56:64) by m
CEPS = 64     # eps           col 64
CBFK = 72     # bfk           [72:104) by hb
NCOLS = 104


def _emit(nc):
    # ---------------- DRAM I/O ----------------
    x_d = nc.declare_dram_parameter("x", [T, C], F32, isOutput=False)
    wk_d = nc.declare_dram_parameter("wk", [128, 2 * 4 * 2 * 1024], FP8, isOutput=False)
    wv_d = nc.declare_dram_parameter("wv", [128, 2 * 4 * 2 * 1024], FP8, isOutput=False)
    wr_d = nc.declare_dram_parameter("wr", [128, 2 * 4 * 2 * 1024], FP8, isOutput=False)
    wo_d = nc.declare_dram_parameter("wo", [2, 128, 4 * 2 * 1024], FP8, isOutput=False)
    fwr_d = nc.declare_dram_parameter("fwr", [2, 128, 4 * 2 * 1024], FP8, isOutput=False)
    fwk_d = nc.declare_dram_parameter("fwk", [2, 8, 128, 4 * 2 * 512], FP8, isOutput=False)
    fwv_d = nc.declare_dram_parameter("fwv", [2, 128, 16 * 2 * 1024], FP8, isOutput=False)
    cst_d = nc.declare_dram_parameter("cst", [128, NCOLS], F32, isOutput=False)
    ones1_d = nc.declare_dram_parameter("ones1", [128, 1], F32R, isOutput=False)
    onesb_d = nc.declare_dram_parameter("onesb", [1, 128], BF16, isOutput=False)
    ident_d = nc.declare_dram_parameter("ident", [128, 128], F32, isOutput=False)
    out_d = nc.declare_dram_parameter("out", [T, C], F32, isOutput=True)

    # DRAM scratch (per-cb granularity for fine deps)
    xT_sp = [nc.dram_tensor(f"xT_sp{i}", [128, T], F32) for i in range(NCB)]
    x2_sp = [nc.dram_tensor(f"x2_sp{i}", [128, T], F32) for i in range(NCB)]

    import contextlib

    with tile.TileContext(nc, pool_alloc_mode="queue") as tc:
        with tc.tile_pool(name="pc", bufs=1) as pc:
            cst = pc.tile([128, NCOLS], F32)
            nc.sync.dma_start(out=cst, in_=cst_d[:])
            ones1 = pc.tile([128, 1], F32R)
            nc.sync.dma_start(out=ones1, in_=ones1_d[:])
            onesb = pc.tile([1, 128], BF16)
            nc.sync.dma_start(out=onesb, in_=onesb_d[:])
            ident = pc.tile([128, 128], F32)
            nc.sync.dma_start(out=ident, in_=ident_d[:])
            ones_bf = pc.tile([128, T], BF16)
            nc.vector.memset(ones_bf, 1.0)

            def col(j):
                return cst[:, j:j + 1]

            # ---- incremental LN stats: two [1, T] psum tiles ----
            def ln_contrib(stat_ps, pool_tmp, src_f32r, cb, sl, tag):
                """Add channel-block cb's contribution for column slice sl."""
                mean_ps, msq_ps = stat_ps
                w = sl.stop - sl.start
                sq = pool_tmp.tile([128, w], F32R, tag=tag, bufs=3)
                nc.scalar.activation(sq, src_f32r.bitcast(F32)[:, sl], AF.Square)
                nc.tensor.matmul(mean_ps[:, sl], ones1, src_f32r[:, sl],
                                 start=(cb == 0), stop=(cb == NCB - 1))
                nc.tensor.matmul(msq_ps[:, sl], ones1, sq,
                                 start=(cb == 0), stop=(cb == NCB - 1))

            def ln_to_sbuf(stat_ps, pool_stat):
                mean_ps, msq_ps = stat_ps
                mean = pool_stat.tile([1, T], BF16, tag="mean_sb")
                msq = pool_stat.tile([1, T], BF16, tag="msq_sb")
                nc.scalar.mul(mean, mean_ps, 1.0 / C)
                nc.scalar.mul(msq, msq_ps, 1.0 / C)
                return mean, msq

            def ln_finish(mean, msq, pool_stat, uid):
                var = pool_stat.tile([1, T], BF16, tag="var_sb")
                nc.vector.tensor_mul(var, mean, mean)
                nc.vector.tensor_sub(var, msq, var)
                lnv = pool_stat.tile([1, T], BF16, tag="msq_sb", name=f"lnv{uid}")
                nc.scalar.activation(lnv, var, AF.Ln,
                                     bias=cst[0:1, CEPS:CEPS + 1], scale=1.0)
                rstd = pool_stat.tile([1, T], BF16, tag="var_sb", name=f"rstd{uid}")
                nc.scalar.activation(rstd, lnv, AF.Exp, bias=0.0, scale=-0.5)
                mrstd = pool_stat.tile([1, T], BF16, tag="mrstd_sb")
                nc.vector.tensor_mul(mrstd, mean, rstd)
                rstd_b = pool_stat.tile([128, T], BF16, tag="rstd_b")
                mrstd_b = pool_stat.tile([128, T], BF16, tag="mrstd_b")
                with tc.tile_pool(name=f"ps_bc{uid}", bufs=2, space="PSUM") as ps_bc:
                    for (src_s, dst) in ((rstd, rstd_b), (mrstd, mrstd_b)):
                        for n in range(NT):
                            sl = slice(n * 512, (n + 1) * 512)
                            bc = ps_bc.tile([128, 512], F32, tag="bc")
                            nc.tensor.matmul(bc, onesb, src_s[:, sl],
                                             start=True, stop=True)
                            if n % 2 == 0:
                                nc.scalar.copy(dst[:, sl], bc)
                            else:
                                nc.vector.tensor_copy(out=dst[:, sl], in_=bc)
                return rstd_b, mrstd_b

            # z pair tiles live through phase B (attention)
            es_z = contextlib.ExitStack()
            p_z = es_z.enter_context(tc.tile_pool(name="p_z", bufs=1, side="right"))
            z8 = [p_z.tile([128, 2, TP], FP8, tag=f"z{pr}", name=f"z{pr}")
                  for pr in range(NPR)]
            for pr in range(NPR):
                nc.vector.memset(z8[pr][:, :, 0:PAD], 0.0)

            # ================= PHASE A: load, transpose, LN1, z =================
            with tc.tile_pool(name="p_xT", bufs=1) as p_xT:
                xT = [p_xT.tile([128, T], F32R, tag=f"xT{cb}", name=f"xT{cb}")
                      for cb in range(NCB)]
                with tc.tile_pool(name="p_tmpA", bufs=1) as p_tmpA, \
                     tc.tile_pool(name="p_statA", bufs=1) as p_statA:
                    meanA = p_statA.tile([1, T], BF16, tag="mean_sb")
                    msqA = p_statA.tile([1, T], BF16, tag="msq_sb")
                    with tc.tile_pool(name="p_ld", bufs=3) as p_ld, \
                         tc.tile_pool(name="ps_stA", bufs=2,
                                      space="PSUM") as ps_stA, \
                         tc.tile_pool(name="ps_tr", bufs=4, space="PSUM") as ps_tr:
                        for tb in range(NTB):
                            xt = p_ld.tile([128, C], F32, tag="xtok")
                            nc.sync.dma_start(out=xt,
                                              in_=x_d[tb * 128:(tb + 1) * 128, :])
                            for cb in range(NCB):
                                pt = ps_tr.tile([128, 128], F32, tag="tr")
                                nc.tensor.transpose(
                                    pt, xt[:, cb * 128:(cb + 1) * 128], ident)
                                dst = xT[cb][:, tb * 128:(tb + 1) * 128]
                                if (tb + cb) % 2 == 0:
                                    nc.scalar.copy(dst, pt)
                                else:
                                    nc.vector.tensor_copy(out=dst, in_=pt)
                            if tb % 4 == 3:
                                n = tb // 4
                                sl = slice(n * 512, (n + 1) * 512)
                                mp = ps_stA.tile([1, 512], F32, tag="mA",
                                                 name=f"mA{n}")
                                qp = ps_stA.tile([1, 512], F32, tag="qA",
                                                 name=f"qA{n}")
                                for cb in range(NCB):
                                    sq = p_tmpA.tile([128, 512], F32R,
                                                     tag="sqA", bufs=3)
                                    nc.scalar.activation(
                                        sq, xT[cb].bitcast(F32)[:, sl],
                                        AF.Square)
                                    nc.tensor.matmul(
                                        mp, ones1, xT[cb][:, sl],
                                        start=(cb == 0), stop=(cb == NCB - 1))
                                    nc.tensor.matmul(
                                        qp, ones1, sq,
                                        start=(cb == 0), stop=(cb == NCB - 1))
                                nc.scalar.mul(meanA[:, sl], mp, 1.0 / C)
                                nc.scalar.mul(msqA[:, sl], qp, 1.0 / C)
                    for cb in range(NCB):
                        nc.sync.dma_start(out=xT_sp[cb][:], in_=xT[cb].bitcast(F32))
                    rstd_b, mrstd_b = ln_finish(meanA, msqA, p_statA, "A")
                    for cb in range(NCB):
                        pr, j = cb // 2, cb % 2
                        zt = p_tmpA.tile([128, T], F32, tag="zt", bufs=2)
                        nc.vector.tensor_mul(zt, xT[cb].bitcast(F32), rstd_b)
                        dst = z8[pr][:, j, PAD:PAD + T]
                        if cb % 2 == 0:
                            nc.vector.tensor_sub(dst, zt, mrstd_b)
                        else:
                            nc.gpsimd.tensor_sub(dst, zt, mrstd_b)

            # attention weights (opened after phase A frees xT)
            es_wo = contextlib.ExitStack()
            p_wo = es_wo.enter_context(tc.tile_pool(name="p_wo", bufs=1))
            wo = [p_wo.tile([128, 4, 2, 1024], FP8, tag=f"wo{i}",
                            name=f"wo{i}") for i in range(2)]
            nc.sync.dma_start(out=wo[0], in_=wo_d[0])
            nc.sync.dma_start(out=wo[1], in_=wo_d[1])
            es_w = contextlib.ExitStack()
            p_w = es_w.enter_context(tc.tile_pool(name="p_w", bufs=1, side="right"))
            wk = p_w.tile([128, 2, 4, 2, 1024], FP8, tag="wk")
            wv = p_w.tile([128, 2, 4, 2, 1024], FP8, tag="wv")
            wr = p_w.tile([128, 2, 4, 2, 1024], FP8, tag="wr")
            nc.sync.dma_start(out=wk, in_=wk_d[:])
            nc.sync.dma_start(out=wv, in_=wv_d[:])
            nc.sync.dma_start(out=wr, in_=wr_d[:])

            # ============ PHASE B: k/v/r projections + WKV per m ============
            es_sry = contextlib.ExitStack()
            p_sry = es_sry.enter_context(tc.tile_pool(name="p_sry", bufs=1))
            sryh = [p_sry.tile([128, 2, T], FP8, tag=f"sryh{pr}", name=f"sryh{pr}")
                    for pr in range(NPR)]
            sryl = [p_sry.tile([128, 2, T], FP8, tag=f"sryl{pr}", name=f"sryl{pr}")
                    for pr in range(NPR)]

            def zsl(k2, a, n):
                # a=0: current tokens; a=1: shifted by one
                lo = PAD - a + n * 512
                return z8[k2][:, :, lo:lo + 512]

            with tc.tile_pool(name="p_kvs", bufs=2) as p_kvs, \
                 tc.tile_pool(name="p_wt", bufs=2) as p_wt, \
                 tc.tile_pool(name="ps_mm", bufs=6, space="PSUM") as ps_mm:
                wkv_state = {}

                def wkv_front(m, ek, vv):
                    wrow = p_wt.tile([128, T], BF16, tag="wrow", name=f"wr{m}")
                    nc.vector.tensor_scalar(out=wrow, in0=ones_bf,
                                            scalar1=col(CW + m),
                                            scalar2=None, op0=AL.mult)
                    ekv = p_wt.tile([128, T], BF16, tag="ekv", name=f"ekv{m}")
                    nc.gpsimd.tensor_mul(ekv, ek, vv)
                    A = p_wt.tile([128, T + 1], BF16, tag="A", name=f"A{m}")
                    Bt = p_wt.tile([128, T + 1], BF16, tag="B", name=f"B{m}")
                    nc.vector.memset(A[:, 0:1], 0.0)
                    nc.vector.memset(Bt[:, 0:1], 0.0)
                    nc.vector.tensor_tensor_scan(
                        out=A[:, 1:T + 1], data0=wrow, data1=ekv,
                        initial=0.0, op0=AL.mult, op1=AL.add)
                    nc.vector.tensor_tensor_scan(
                        out=Bt[:, 1:T + 1], data0=wrow, data1=ek,
                        initial=0.0, op0=AL.mult, op1=AL.add)
                    nc.vector.scalar_tensor_tensor(
                        out=A[:, 0:T], in0=ekv, scalar=col(CEU + m),
                        in1=A[:, 0:T], op0=AL.mult, op1=AL.add)
                    nc.vector.scalar_tensor_tensor(
                        out=Bt[:, 0:T], in0=ek, scalar=col(CEU + m),
                        in1=Bt[:, 0:T], op0=AL.mult, op1=AL.add)
                    rec = p_wt.tile([128, T], BF16, tag="rec", name=f"rec{m}")
                    with nc.allow_low_precision(reason="wkv ratio bf16"):
                        nc.vector.reciprocal(rec, Bt[:, 0:T])
                    return A, rec

                def wkv_tail(m, A, rec, sr):
                    pr_m, j_m = m // 2, m % 2
                    y = p_wt.tile([128, T], BF16, tag="y", name=f"y{m}")
                    nc.gpsimd.tensor_mul(y, A[:, 0:T], rec)
                    sy = p_wt.tile([128, T], BF16, tag="sy", name=f"sy{m}")
                    nc.gpsimd.tensor_mul(sy, y, sr)
                    nc.scalar.copy(sryh[pr_m][:, j_m, :], sy)
                    nc.vector.tensor_sub(sryl[pr_m][:, j_m, :], sy,
                                         sryh[pr_m][:, j_m, :])

                for m in (range(NCB) if KPHASES >= 2 else ()):
                    ek = p_kvs.tile([128, T], BF16, tag="ek", name=f"ek{m}")
                    vv = p_kvs.tile([128, T], BF16, tag="vv", name=f"vv{m}")
                    sr = p_kvs.tile([128, T], BF16, tag="sr", name=f"sr{m}")
                    for (wt, dst, act, bcol) in (
                            (wk, ek, AF.Exp, CBK), (wv, vv, AF.Identity, CBV),
                            (wr, sr, AF.Sigmoid, CBR)):
                        for n in range(NT):
                            pmm = ps_mm.tile([128, 512], F32, tag="pmm")
                            for a in range(2):
                                for k2 in range(NPR):
                                    nc.tensor.matmul(
                                        pmm, wt[:, a, k2, :,
                                                m * 128:(m + 1) * 128],
                                        zsl(k2, a, n),
                                        start=(a == 0 and k2 == 0),
                                        stop=(a == 1 and k2 == NPR - 1),
                                        perf_mode=DR)
                            dsl = dst[:, n * 512:(n + 1) * 512]
                            nc.scalar.activation(dsl, pmm, act,
                                                 bias=col(bcol + m), scale=INV)
                    if KPHASES >= 3:
                        A, rec = wkv_front(m, ek, vv)
                        wkv_state[m] = (A, rec, sr)
                        if m >= 1:
                            wkv_tail(m - 1, *wkv_state.pop(m - 1))
                if KPHASES >= 3:
                    wkv_tail(NCB - 1, *wkv_state.pop(NCB - 1))

            es_w.close()
            es_z.close()

            # ===== PHASE C: out-proj + residual -> x2, fused LN2 stats =====
            es_x2 = contextlib.ExitStack()
            p_x2 = es_x2.enter_context(tc.tile_pool(name="p_x2", bufs=1))
            x2 = [p_x2.tile([128, T], F32R, tag=f"x2_{cb}", name=f"x2_{cb}")
                  for cb in range(NCB)]
            es_z2 = contextlib.ExitStack()
            p_z2 = es_z2.enter_context(tc.tile_pool(name="p_z2", bufs=1,
                                                    side="right"))
            z2t = [p_z2.tile([128, T + 1], BF16, tag=f"z2_{cb}", name=f"z2_{cb}")
                   for cb in range(NCB)]
            with tc.tile_pool(name="p_xr", bufs=2) as p_xr, \
                 tc.tile_pool(name="p_tmpD", bufs=1) as p_tmpD, \
                 tc.tile_pool(name="p_statD", bufs=1) as p_statD:
              with tc.tile_pool(name="ps_mo", bufs=4, space="PSUM") as ps_mo:
                for m in (range(NCB) if KPHASES >= 4 else ()):
                    xr = p_xr.tile([128, T], F32, tag="xr")
                    nc.sync.dma_start(out=xr, in_=xT_sp[m][:])
                    for n in range(NT):
                        sl = slice(n * 512, (n + 1) * 512)
                        pmm = ps_mo.tile([128, 512], F32, tag="pmo")
                        first = True
                        for (wi, ss) in ((0, sryh), (1, sryh), (0, sryl)):
                            for k2 in range(NPR):
                                nc.tensor.matmul(
                                    pmm, wo[wi][:, k2, :, m * 128:(m + 1) * 128],
                                    ss[k2][:, :, sl],
                                    start=first,
                                    stop=(wi == 0 and ss is sryl
                                          and k2 == NPR - 1),
                                    perf_mode=DR)
                                first = False
                        nc.vector.scalar_tensor_tensor(
                            out=x2[m][:, sl], in0=pmm, scalar=INV,
                            in1=xr[:, sl], op0=AL.mult, op1=AL.add)
                    nc.sync.dma_start(out=x2_sp[m][:], in_=x2[m].bitcast(F32))
              # ---- LN2 stats + finish -> z2 (plain bf16, col 0 zero) ----
              if True:
                if KPHASES >= 5:
                    with tc.tile_pool(name="ps_stD", bufs=1,
                                      space="PSUM") as ps_stD:
                        stat_ps2 = (ps_stD.tile([1, T], F32, tag="meanD", name="meanD"),
                                    ps_stD.tile([1, T], F32, tag="msqD", name="msqD"))
                        for n in range(NT):
                            sl2 = slice(n * 512, (n + 1) * 512)
                            for cb in range(NCB):
                                ln_contrib(stat_ps2, p_tmpD, x2[cb], cb, sl2,
                                           "sqD")
                        meanD, msqD = ln_to_sbuf(stat_ps2, p_statD)
                    rstd_b2, mrstd_b2 = ln_finish(meanD, msqD, p_statD, "D")
                    for cb in range(NCB):
                        nc.vector.memset(z2t[cb][:, 0:1], 0.0)
                        zt = p_tmpD.tile([128, T], F32, tag="zt2", bufs=1)
                        nc.vector.tensor_mul(zt, x2[cb].bitcast(F32), rstd_b2)
                        dst = z2t[cb][:, 1:T + 1]
                        if cb % 2 == 0:
                            nc.vector.tensor_sub(dst, zt, mrstd_b2)
                        else:
                            nc.gpsimd.tensor_sub(dst, zt, mrstd_b2)
            es_x2.close()
            es_sry.close()
            es_wo.close()

            # FFN weights: fwv hi/lo resident fp8; fwr till srf; fwk streamed
            es_fw = contextlib.ExitStack()
            p_fw = es_fw.enter_context(tc.tile_pool(name="p_fw", bufs=1))
            fwv = [p_fw.tile([128, 16, 2, 1024], FP8, tag=f"fwv{i}",
                             name=f"fwv{i}") for i in range(2)]
            if KPHASES >= 5:
                nc.sync.dma_start(out=fwv[0], in_=fwv_d[0])
                nc.sync.dma_start(out=fwv[1], in_=fwv_d[1])

            # ============ PHASE E: xf lerp (f_tmk == f_tmr), fWr -> srf ========
            es_xf = contextlib.ExitStack()
            p_xf = es_xf.enter_context(tc.tile_pool(name="p_xf", bufs=1))
            xfh = [p_xf.tile([128, 2, T], FP8, tag=f"xfh{pr}", name=f"xfh{pr}")
                   for pr in range(NPR)]
            xfl = [p_xf.tile([128, 2, T], FP8, tag=f"xfl{pr}", name=f"xfl{pr}")
                   for pr in range(NPR)]
            with tc.tile_pool(name="p_te", bufs=3) as p_te:
                for cb in (range(NCB) if KPHASES >= 6 else ()):
                    pr, j = cb // 2, cb % 2
                    t1 = p_te.tile([128, T], BF16, tag="t1")
                    nc.scalar.mul(t1, z2t[cb][:, 0:T], col(CFT1 + cb))
                    xfb = p_te.tile([128, T], BF16, tag="xfb")
                    nc.vector.scalar_tensor_tensor(
                        out=xfb, in0=z2t[cb][:, 1:T + 1],
                        scalar=col(CFT + cb), in1=t1, op0=AL.mult, op1=AL.add)
                    nc.scalar.copy(xfh[pr][:, j, :], xfb)
                    nc.gpsimd.tensor_sub(xfl[pr][:, j, :], xfb, xfh[pr][:, j, :])
            es_z2.close()

            es_srf = contextlib.ExitStack()
            p_srf = es_srf.enter_context(tc.tile_pool(name="p_srf", bufs=1))
            srf = [p_srf.tile([128, T], FP8, tag=f"srf{m}", name=f"srf{m}")
                   for m in range(NCB)]
            with tc.tile_pool(name="p_fwr", bufs=1) as p_fwr, \
                 tc.tile_pool(name="ps_fr", bufs=4, space="PSUM") as ps_fr:
                fwr = [p_fwr.tile([128, 4, 2, 1024], FP8, tag=f"fwr{i}",
                                  name=f"fwr{i}") for i in range(2)]
                if KPHASES >= 6:
                    nc.sync.dma_start(out=fwr[0], in_=fwr_d[0])
                    nc.sync.dma_start(out=fwr[1], in_=fwr_d[1])
                for m in (range(NCB) if KPHASES >= 6 else ()):
                    for n in range(NT):
                        pmm = ps_fr.tile([128, 512], F32, tag="pfr")
                        first = True
                        for (wi, xs) in ((0, xfh), (1, xfh), (0, xfl)):
                            for k2 in range(NPR):
                                nc.tensor.matmul(
                                    pmm, fwr[wi][:, k2, :, m * 128:(m + 1) * 128],
                                    xs[k2][:, :, n * 512:(n + 1) * 512],
                                    start=first,
                                    stop=(wi == 0 and xs is xfl and k2 == NPR - 1),
                                    perf_mode=DR)
                                first = False
                        nc.scalar.activation(srf[m][:, n * 512:(n + 1) * 512],
                                             pmm, AF.Sigmoid, bias=col(CBFR + m),
                                             scale=INV)

            # ============ PHASE F: FFN k/v matmuls + output ============
            # 3-pass residual fp8: W*x ~ Wh*xh + Wl*xh + Wh*xl
            with tc.tile_pool(name="p_fwkg", bufs=2) as p_fwkg, \
                 tc.tile_pool(name="p_kk", bufs=1) as p_kk, \
                 tc.tile_pool(name="p_rl", bufs=4) as p_rl, \
                 tc.tile_pool(name="p_x2c", bufs=3) as p_x2c, \
                 tc.tile_pool(name="p_fin", bufs=2) as p_fin, \
                 tc.tile_pool(name="p_ost", bufs=1) as p_ost, \
                 tc.tile_pool(name="ps_fk", bufs=4, space="PSUM") as ps_fk, \
                 tc.tile_pool(name="ps_fo", bufs=2, space="PSUM") as ps_fo, \
                 tc.tile_pool(name="ps_ot", bufs=2, space="PSUM") as ps_ot:
                for n in (range(NT) if KPHASES >= 7 else ()):
                    sl = slice(n * 512, (n + 1) * 512)
                    kkh = [p_kk.tile([128, 2, 512], FP8, tag=f"kkh{hp}",
                                     name=f"kkh{hp}_{n}") for hp in range(NHP)]
                    kkl = [p_kk.tile([128, 2, 512], FP8, tag=f"kkl{hp}",
                                     name=f"kkl{hp}_{n}") for hp in range(NHP)]
                    for g in range(8):
                        fg = [p_fwkg.tile([128, 4, 2, 512], FP8, tag=f"fwkg{i}",
                                          name=f"fwkg{i}_{n}_{g}")
                              for i in range(2)]
                        nc.sync.dma_start(out=fg[0], in_=fwk_d[0, g])
                        nc.sync.dma_start(out=fg[1], in_=fwk_d[1, g])
                        for i in range(4):
                            hb = g * 4 + i
                            hp, jh = hb // 2, hb % 2
                            pkk = ps_fk.tile([128, 512], F32, tag="pkk")
                            first = True
                            for (wi, xs) in ((0, xfh), (1, xfh), (0, xfl)):
                                for k2 in range(NPR):
                                    nc.tensor.matmul(
                                        pkk,
                                        fg[wi][:, k2, :, i * 128:(i + 1) * 128],
                                        xs[k2][:, :, sl],
                                        start=first,
                                        stop=(wi == 0 and xs is xfl
                                              and k2 == NPR - 1),
                                        perf_mode=DR)
                                    first = False
                            rl = p_rl.tile([128, 512], BF16, tag="rl")
                            if hb % 2 == 0:
                                nc.scalar.activation(rl, pkk, AF.Relu,
                                                     bias=col(CBFK + hb),
                                                     scale=INV)
                            else:
                                nc.vector.tensor_scalar(
                                    out=rl, in0=pkk, scalar1=INV,
                                    scalar2=0.0, op0=AL.mult, op1=AL.max)
                            t2 = p_rl.tile([128, 512], BF16, tag="t2")
                            nc.vector.tensor_mul(t2, rl, rl)
                            dh = kkh[hp][:, jh, :]
                            if hb % 2 == 0:
                                nc.scalar.copy(dh, t2)
                            else:
                                nc.vector.tensor_copy(out=dh, in_=t2)
                            nc.gpsimd.tensor_sub(kkl[hp][:, jh, :], t2, dh)
                    osts = [p_ost.tile([128, C], F32, tag=f"ost{j}",
                                       name=f"ost{n}_{j}") for j in range(4)]
                    for m in range(NCB):
                        po = ps_fo.tile([128, 512], F32, tag="po")
                        first = True
                        for (wi, ks) in ((0, kkh), (1, kkh), (0, kkl)):
                            for hp in range(NHP):
                                nc.tensor.matmul(
                                    po, fwv[wi][:, hp, :, m * 128:(m + 1) * 128],
                                    ks[hp],
                                    start=first,
                                    stop=(wi == 0 and ks is kkl
                                          and hp == NHP - 1),
                                    perf_mode=DR)
                                first = False
                        x2c = p_x2c.tile([128, 512], F32, tag="x2c")
                        nc.sync.dma_start(out=x2c, in_=x2_sp[m][:, sl])
                        rkv = p_fin.tile([128, 512], F32, tag="rkv")
                        nc.vector.scalar_tensor_tensor(
                            out=rkv, in0=po, scalar=INV, in1=srf[m][:, sl],
                            op0=AL.mult, op1=AL.mult)
                        fin = p_fin.tile([128, 512], F32, tag="fin")
                        if m % 2 == 0:
                            nc.gpsimd.tensor_add(fin, rkv, x2c)
                        else:
                            nc.vector.tensor_add(fin, rkv, x2c)
                        for j in range(4):
                            pt = ps_ot.tile([128, 128], F32, tag="ptr")
                            nc.tensor.transpose(pt, fin[:, j * 128:(j + 1) * 128],
                                                ident)
                            dst = osts[j][:, m * 128:(m + 1) * 128]
                            if (m + j) % 2 == 0:
                                nc.scalar.copy(dst, pt)
                            else:
                                nc.vector.tensor_copy(out=dst, in_=pt)
                    for j in range(4):
                        tb = n * 4 + j
                        nc.sync.dma_start(out=out_d[tb * 128:(tb + 1) * 128, :],
                                          in_=osts[j])
            es_srf.close()
            es_xf.close()
            es_fw.close()
    nc.finalize()
    return nc


_PROG = None


def _get_prog():
    global _PROG
    if _PROG is None:
        nc = bacc.Bacc()
        _PROG = _emit(nc)
    return _PROG


def _pair_w(WT, M_out):
    """WT: [K_in, M_out] fp8 (lhsT layout) -> [128, K_in//256, 2, M_out] flat."""
    K_in = WT.shape[0]
    npr = K_in // 256
    return np.ascontiguousarray(
        WT.reshape(npr, 2, 128, M_out).transpose(2, 0, 1, 3).reshape(128, -1))


def _q8_hl(WT):
    """WT f32 (pre-scaled by WS) -> (hi, lo) fp8 arrays."""
    f8 = ml_dtypes.float8_e4m3
    Ws = np.asarray(WT, np.float32) * np.float32(WS)
    assert np.abs(Ws).max() < 230.0
    hi = Ws.astype(f8)
    lo = (Ws - hi.astype(np.float32)).astype(f8)
    return hi, lo


def _fwk_hl(WT):
    """WT: [C, HID] -> fp8 [2(hl), 8(g), 128, 4(k2)*2(j)*512]; g = hid cols 512g."""
    hi, lo = _q8_hl(WT)
    out = []
    for W8 in (hi, lo):
        # pair layout per group: [128, k2, j, 512]
        Wp = W8.reshape(4, 2, 128, HID)  # [k2, j, c128, h]
        out.append(np.stack(
            [np.ascontiguousarray(
                Wp[:, :, :, g * 512:(g + 1) * 512].transpose(2, 0, 1, 3)
                .reshape(128, -1)) for g in range(8)]))
    return np.ascontiguousarray(np.stack(out))


def _fwv_hl(WT):
    """WT: [HID, C] -> fp8 [2(hl), 128, 16*2*1024] pair layout."""
    hi, lo = _q8_hl(WT)
    return np.ascontiguousarray(np.stack([_pair_w(W8, C) for W8 in (hi, lo)]))


def _q8s(W):
    """Scale by WS, quantize to fp8e4 (checks range)."""
    f8 = ml_dtypes.float8_e4m3
    Ws = np.asarray(W, np.float32) * np.float32(WS)
    assert np.abs(Ws).max() < 230.0, "weight scale overflow"
    return Ws.astype(f8)


def _prep_inputs(x, ln1_g, ln1_b, ln2_g, ln2_b, time_decay, time_first,
                 tmk, tmv, tmr, Wk, Wv, Wr, Wo, f_tmk, f_tmr, fWk, fWr, fWv):
    f32 = np.float32
    x = np.asarray(x, f32)
    g1 = np.asarray(ln1_g, f32); b1 = np.asarray(ln1_b, f32)
    g2 = np.asarray(ln2_g, f32); b2 = np.asarray(ln2_b, f32)
    td = np.asarray(time_decay, np.float64); tf = np.asarray(time_first, np.float64)
    tmk = np.asarray(tmk, f32).reshape(C); tmv = np.asarray(tmv, f32).reshape(C)
    tmr = np.asarray(tmr, f32).reshape(C)
    ftmk = np.asarray(f_tmk, f32).reshape(C); ftmr = np.asarray(f_tmr, f32).reshape(C)
    assert np.array_equal(ftmk, ftmr), "kernel assumes f_tmk == f_tmr"
    Wk = np.asarray(Wk, f32); Wv = np.asarray(Wv, f32); Wr = np.asarray(Wr, f32)
    Wo = np.asarray(Wo, f32); fWk = np.asarray(fWk, f32); fWr = np.asarray(fWr, f32)
    fWv = np.asarray(fWv, f32)

    Wk1 = Wk * g1[None, :]; Wv1 = Wv * g1[None, :]; Wr1 = Wr * g1[None, :]
    bk = Wk @ b1; bv = Wv @ b1; br = Wr @ b1
    fWk1 = fWk * g2[None, :]; fWr1 = fWr * g2[None, :]
    bfk = fWk @ b2; bfr = fWr @ b2
    assert np.allclose(bfk, 0.0), "kernel assumes zero ln2 beta for relu path"

    wbar = np.exp(-np.exp(td)).astype(f32)
    eu = np.exp(tf).astype(f32)

    def packc(v):
        return np.asarray(v, f32).reshape(-1, 128).T

    cst = np.zeros((128, NCOLS), f32)
    cst[:, CW:CW + 8] = packc(wbar)
    cst[:, CEU:CEU + 8] = packc(eu)
    cst[:, CBK:CBK + 8] = packc(bk)
    cst[:, CBV:CBV + 8] = packc(bv)
    cst[:, CBR:CBR + 8] = packc(br)
    cst[:, CFT:CFT + 8] = packc(ftmk)
    cst[:, CFT1:CFT1 + 8] = packc(1 - ftmk)
    cst[:, CBFR:CBFR + 8] = packc(bfr)
    cst[:, CEPS] = EPS
    cst[:, CBFK:CBFK + 32] = packc(bfk)

    def lerp_pair(W1, tm):
        # [128, 2(ab), 4(k2), 2(j), 1024] flat; a=0: W*tm, a=1: W*(1-tm)
        Wa = _pair_w(_q8s((W1 * tm[None, :]).T), C)
        Wb = _pair_w(_q8s((W1 * (1 - tm)[None, :]).T), C)
        return np.ascontiguousarray(
            np.stack([Wa.reshape(128, 4, 2, 1024),
                      Wb.reshape(128, 4, 2, 1024)], axis=1).reshape(128, -1))

    shared = {
        "wk": lerp_pair(Wk1, tmk),
        "wv": lerp_pair(Wv1, tmv),
        "wr": lerp_pair(Wr1, tmr),
        "wo": _fwv_hl(Wo.T),
        "fwr": _fwv_hl(fWr1.T),
        "fwk": _fwk_hl(fWk1.T),
        "fwv": _fwv_hl(fWv.T),
        "cst": cst,
        "ones1": np.ones((128, 1), f32),
        "onesb": np.ones((1, 128), ml_dtypes.bfloat16),
        "ident": np.eye(128, dtype=f32),
    }
    in_maps = [dict(shared, x=np.ascontiguousarray(x[b])) for b in range(B)]
    return in_maps


def _run(in_maps, trace=False, **kw):
    nc = _get_prog()
    res = run_bass_kernel_spmd(nc, in_maps, core_ids=list(range(B)), trace=trace,
                               **kw)
    out = np.stack([np.asarray(res.results[b]["out"]) for b in range(B)], axis=0)
    return out.astype(np.float32), res


def kernel(*a, **kw):
    out, _ = _run(_prep_inputs(*a, **kw))
    return out


if __name__ == "__main__":
    _get_prog()
    print("program built ok")


# revision 4
# speedup vs baseline: 1.0445x; 1.0068x over previous
"""RWKV-4 block (nn_Block_5669356833485) Trainium2 Bass kernel.

B=8, T=2048, C=1024, HID=4096. B-sharded across 8 NeuronCores (1 batch/core).
Feature-major layout [C-partitions, T-free].

fp8e4 DoubleRow matmuls (256-wide contraction, 0.5 cyc/row). Weights are
host-scaled by 128 before fp8 quantization (their natural ~0.02 magnitude
falls in e4m3's subnormal range) and unscaled in the matmul epilogues.
Time-mix lerps are folded into the matmuls by doubling the contraction
against z and shifted-z (z pair tiles [128, 2, 2064], data offset 16, pair
stride %16==0 per DoubleRow requirements). LN stats are pipelined into the
producing loops; WKV (bf16 scans, fp32 state) interleaves per channel block
with the projections. ek/v/sr/srf stay in SBUF; only xT and x2 round-trip
through DRAM for the residual adds.
Self-contained: hardcodes shapes; no sibling imports.
"""
import os
import sys
sys.path.insert(0, '/opt/trn_rl_repo')

KPHASES = int(os.environ.get("KPHASES", "99"))

import numpy as np
import ml_dtypes

import concourse.bass as bass
from concourse import bacc
import concourse.mybir as mybir
import concourse.tile as tile
from concourse.bass_utils import run_bass_kernel_spmd

F32 = mybir.dt.float32
F32R = mybir.dt.float32r
BF16 = mybir.dt.bfloat16
FP8 = mybir.dt.float8e4
AL = mybir.AluOpType
AF = mybir.ActivationFunctionType
DR = mybir.MatmulPerfMode.DoubleRow

B, T, C, HID = 8, 2048, 1024, 4096
NCB = C // 128          # 8 channel blocks
NPR = NCB // 2          # 4 channel pair-blocks
NHB = HID // 128        # 32 hidden blocks
NHP = NHB // 2          # 16 hidden pair-blocks
NT = T // 512           # 4 n-slices of 512
NTB = T // 128          # 16 token blocks
PAD = 16                # z pair tiles: [128, 2, PAD+T]; pair stride %16==0
TP = T + PAD
EPS = 1e-5
WS = 128.0              # weight pre-quantization scale
INV = 1.0 / WS

# cst columns (per 128-partition, indexed by block)
CW = 0        # wbar          [0:8)   by cb
CEU = 8       # exp(tf)       [8:16)  by cb
CBK = 16      # bk            [16:24) by m
CBV = 24      # bv            [24:32) by m
CBR = 32      # br            [32:40) by m
CFT = 40      # ftmk          [40:48) by cb
CFT1 = 48     # 1-ftmk        [48:56) by cb
CBFR = 56     # bfr           [56:64) by m
CEPS = 64     # eps           col 64
CBFK = 72     # bfk           [72:104) by hb
NCOLS = 104


def _emit(nc):
    # ---------------- DRAM I/O ----------------
    x_d = nc.declare_dram_parameter("x", [T, C], F32, isOutput=False)
    wk_d = nc.declare_dram_parameter("wk", [128, 2 * 4 * 2 * 1024], FP8, isOutput=False)
    wv_d = nc.declare_dram_parameter("wv", [128, 2 * 4 * 2 * 1024], FP8, isOutput=False)
    wr_d = nc.declare_dram_parameter("wr", [128, 2 * 4 * 2 * 1024], FP8, isOutput=False)
    wo_d = nc.declare_dram_parameter("wo", [2, 128, 4 * 2 * 1024], FP8, isOutput=False)
    fwr_d = nc.declare_dram_parameter("fwr", [2, 128, 4 * 2 * 1024], FP8, isOutput=False)
    fwk_d = nc.declare_dram_parameter("fwk", [2, 8, 128, 4 * 2 * 512], FP8, isOutput=False)
    fwv_d = nc.declare_dram_parameter("fwv", [2, 128, 16 * 2 * 1024], FP8, isOutput=False)
    cst_d = nc.declare_dram_parameter("cst", [128, NCOLS], F32, isOutput=False)
    ones1_d = nc.declare_dram_parameter("ones1", [128, 1], F32R, isOutput=False)
    onesb_d = nc.declare_dram_parameter("onesb", [1, 128], BF16, isOutput=False)
    ident_d = nc.declare_dram_parameter("ident", [128, 128], F32, isOutput=False)
    out_d = nc.declare_dram_parameter("out", [T, C], F32, isOutput=True)

    # DRAM scratch (per-cb granularity for fine deps)
    xT_sp = [nc.dram_tensor(f"xT_sp{i}", [128, T], F32) for i in range(NCB)]
    x2_sp = [nc.dram_tensor(f"x2_sp{i}", [128, T], F32) for i in range(NCB)]

    import contextlib

    with tile.TileContext(nc, pool_alloc_mode="queue") as tc:
        with tc.tile_pool(name="pc", bufs=1) as pc:
            cst = pc.tile([128, NCOLS], F32)
            nc.sync.dma_start(out=cst, in_=cst_d[:])
            ones1 = pc.tile([128, 1], F32R)
            nc.sync.dma_start(out=ones1, in_=ones1_d[:])
            onesb = pc.tile([1, 128], BF16)
            nc.sync.dma_start(out=onesb, in_=onesb_d[:])
            ident = pc.tile([128, 128], F32)
            nc.sync.dma_start(out=ident, in_=ident_d[:])
            ones_bf = pc.tile([128, T], BF16)
            nc.vector.memset(ones_bf, 1.0)

            def col(j):
                return cst[:, j:j + 1]

            # ---- incremental LN stats: two [1, T] psum tiles ----
            def ln_contrib(stat_ps, pool_tmp, src_f32r, cb, sl, tag):
                """Add channel-block cb's contribution for column slice sl."""
                mean_ps, msq_ps = stat_ps
                w = sl.stop - sl.start
                sq = pool_tmp.tile([128, w], F32R, tag=tag, bufs=3)
                nc.scalar.activation(sq, src_f32r.bitcast(F32)[:, sl], AF.Square)
                nc.tensor.matmul(mean_ps[:, sl], ones1, src_f32r[:, sl],
                                 start=(cb == 0), stop=(cb == NCB - 1))
                nc.tensor.matmul(msq_ps[:, sl], ones1, sq,
                                 start=(cb == 0), stop=(cb == NCB - 1))

            def ln_to_sbuf(stat_ps, pool_stat):
                mean_ps, msq_ps = stat_ps
                mean = pool_stat.tile([1, T], BF16, tag="mean_sb")
                msq = pool_stat.tile([1, T], BF16, tag="msq_sb")
                nc.scalar.mul(mean, mean_ps, 1.0 / C)
                nc.scalar.mul(msq, msq_ps, 1.0 / C)
                return mean, msq

            def ln_finish(mean, msq, pool_stat, uid):
                var = pool_stat.tile([1, T], BF16, tag="var_sb")
                nc.vector.tensor_mul(var, mean, mean)
                nc.vector.tensor_sub(var, msq, var)
                lnv = pool_stat.tile([1, T], BF16, tag="msq_sb", name=f"lnv{uid}")
                nc.scalar.activation(lnv, var, AF.Ln,
                                     bias=cst[0:1, CEPS:CEPS + 1], scale=1.0)
                rstd = pool_stat.tile([1, T], BF16, tag="var_sb", name=f"rstd{uid}")
                nc.scalar.activation(rstd, lnv, AF.Exp, bias=0.0, scale=-0.5)
                mrstd = pool_stat.tile([1, T], BF16, tag="mrstd_sb")
                nc.vector.tensor_mul(mrstd, mean, rstd)
                rstd_b = pool_stat.tile([128, T], BF16, tag="rstd_b")
                mrstd_b = pool_stat.tile([128, T], BF16, tag="mrstd_b")
                with tc.tile_pool(name=f"ps_bc{uid}", bufs=2, space="PSUM") as ps_bc:
                    for (src_s, dst) in ((rstd, rstd_b), (mrstd, mrstd_b)):
                        for n in range(NT):
                            sl = slice(n * 512, (n + 1) * 512)
                            bc = ps_bc.tile([128, 512], F32, tag="bc")
                            nc.tensor.matmul(bc, onesb, src_s[:, sl],
                                             start=True, stop=True)
                            if n % 2 == 0:
                                nc.scalar.copy(dst[:, sl], bc)
                            else:
                                nc.vector.tensor_copy(out=dst[:, sl], in_=bc)
                return rstd_b, mrstd_b

            # z pair tiles live through phase B (attention)
            es_z = contextlib.ExitStack()
            p_z = es_z.enter_context(tc.tile_pool(name="p_z", bufs=1, side="right"))
            z8 = [p_z.tile([128, 2, TP], FP8, tag=f"z{pr}", name=f"z{pr}")
                  for pr in range(NPR)]
            for pr in range(NPR):
                nc.vector.memset(z8[pr][:, :, 0:PAD], 0.0)

            # attention weights: prefetch during phase A
            es_wo = contextlib.ExitStack()
            p_wo = es_wo.enter_context(tc.tile_pool(name="p_wo", bufs=1))
            wo = [p_wo.tile([128, 4, 2, 1024], FP8, tag=f"wo{i}",
                            name=f"wo{i}") for i in range(2)]
            es_w = contextlib.ExitStack()
            p_w = es_w.enter_context(tc.tile_pool(name="p_w", bufs=1, side="right"))
            wk = p_w.tile([128, 2, 4, 2, 1024], FP8, tag="wk")
            wv = p_w.tile([128, 2, 4, 2, 1024], FP8, tag="wv")
            wr = p_w.tile([128, 2, 4, 2, 1024], FP8, tag="wr")

            # ================= PHASE A: load, transpose, LN1, z =================
            with tc.tile_pool(name="p_xT", bufs=1) as p_xT:
                xT = [p_xT.tile([128, T], F32R, tag=f"xT{cb}", name=f"xT{cb}")
                      for cb in range(NCB)]
                with tc.tile_pool(name="p_tmpA", bufs=1) as p_tmpA, \
                     tc.tile_pool(name="p_statA", bufs=1) as p_statA:
                    meanA = p_statA.tile([1, T], BF16, tag="mean_sb")
                    msqA = p_statA.tile([1, T], BF16, tag="msq_sb")
                    with tc.tile_pool(name="p_ld", bufs=3) as p_ld, \
                         tc.tile_pool(name="ps_stA", bufs=2,
                                      space="PSUM") as ps_stA, \
                         tc.tile_pool(name="ps_tr", bufs=4, space="PSUM") as ps_tr:
                        for tb in range(NTB):
                            xt = p_ld.tile([128, C], F32, tag="xtok")
                            nc.sync.dma_start(out=xt,
                                              in_=x_d[tb * 128:(tb + 1) * 128, :])
                            for cb in range(NCB):
                                pt = ps_tr.tile([128, 128], F32, tag="tr")
                                nc.tensor.transpose(
                                    pt, xt[:, cb * 128:(cb + 1) * 128], ident)
                                dst = xT[cb][:, tb * 128:(tb + 1) * 128]
                                if (tb + cb) % 2 == 0:
                                    nc.scalar.copy(dst, pt)
                                else:
                                    nc.vector.tensor_copy(out=dst, in_=pt)
                            if tb % 4 == 3:
                                n = tb // 4
                                sl = slice(n * 512, (n + 1) * 512)
                                mp = ps_stA.tile([1, 512], F32, tag="mA",
                                                 name=f"mA{n}")
                                qp = ps_stA.tile([1, 512], F32, tag="qA",
                                                 name=f"qA{n}")
                                for cb in range(NCB):
                                    sq = p_tmpA.tile([128, 512], F32R,
                                                     tag="sqA", bufs=3)
                                    nc.scalar.activation(
                                        sq, xT[cb].bitcast(F32)[:, sl],
                                        AF.Square)
                                    nc.tensor.matmul(
                                        mp, ones1, xT[cb][:, sl],
                                        start=(cb == 0), stop=(cb == NCB - 1))
                                    nc.tensor.matmul(
                                        qp, ones1, sq,
                                        start=(cb == 0), stop=(cb == NCB - 1))
                                nc.scalar.mul(meanA[:, sl], mp, 1.0 / C)
                                nc.scalar.mul(msqA[:, sl], qp, 1.0 / C)
                    # attention weight loads: queued after x-in, before spills
                    nc.sync.dma_start(out=wk, in_=wk_d[:])
                    nc.sync.dma_start(out=wv, in_=wv_d[:])
                    nc.sync.dma_start(out=wr, in_=wr_d[:])
                    nc.sync.dma_start(out=wo[0], in_=wo_d[0])
                    nc.sync.dma_start(out=wo[1], in_=wo_d[1])
                    for cb in range(NCB):
                        nc.sync.dma_start(out=xT_sp[cb][:], in_=xT[cb].bitcast(F32))
                    rstd_b, mrstd_b = ln_finish(meanA, msqA, p_statA, "A")
                    for cb in range(NCB):
                        pr, j = cb // 2, cb % 2
                        zt = p_tmpA.tile([128, T], F32, tag="zt", bufs=2)
                        nc.vector.tensor_mul(zt, xT[cb].bitcast(F32), rstd_b)
                        dst = z8[pr][:, j, PAD:PAD + T]
                        if cb % 2 == 0:
                            nc.vector.tensor_sub(dst, zt, mrstd_b)
                        else:
                            nc.gpsimd.tensor_sub(dst, zt, mrstd_b)

            # ============ PHASE B: k/v/r projections + WKV per m ============
            es_sry = contextlib.ExitStack()
            p_sry = es_sry.enter_context(tc.tile_pool(name="p_sry", bufs=1))
            sryh = [p_sry.tile([128, 2, T], FP8, tag=f"sryh{pr}", name=f"sryh{pr}")
                    for pr in range(NPR)]
            sryl = [p_sry.tile([128, 2, T], FP8, tag=f"sryl{pr}", name=f"sryl{pr}")
                    for pr in range(NPR)]

            def zsl(k2, a, n):
                # a=0: current tokens; a=1: shifted by one
                lo = PAD - a + n * 512
                return z8[k2][:, :, lo:lo + 512]

            with tc.tile_pool(name="p_kvs", bufs=2) as p_kvs, \
                 tc.tile_pool(name="p_wt", bufs=2) as p_wt, \
                 tc.tile_pool(name="ps_mm", bufs=8, space="PSUM") as ps_mm:
                wkv_state = {}

                def wkv_front(m, ek, vv):
                    # scanB first: depends only on ek (k epilogues), so DVE can
                    # start while Act still runs v/r epilogues. ekv on DVE keeps
                    # the ekv->scanA handoff on-engine (no cross-engine sem).
                    wrow = p_wt.tile([128, T], BF16, tag="wrow", name=f"wr{m}")
                    nc.vector.tensor_scalar(out=wrow, in0=ones_bf,
                                            scalar1=col(CW + m),
                                            scalar2=None, op0=AL.mult)
                    A = p_wt.tile([128, T + 1], BF16, tag="A", name=f"A{m}")
                    Bt = p_wt.tile([128, T + 1], BF16, tag="B", name=f"B{m}")
                    nc.vector.memset(Bt[:, 0:1], 0.0)
                    nc.vector.tensor_tensor_scan(
                        out=Bt[:, 1:T + 1], data0=wrow, data1=ek,
                        initial=0.0, op0=AL.mult, op1=AL.add)
                    ekv = p_wt.tile([128, T], BF16, tag="ekv", name=f"ekv{m}")
                    nc.vector.tensor_mul(ekv, ek, vv)
                    nc.vector.memset(A[:, 0:1], 0.0)
                    nc.vector.tensor_tensor_scan(
                        out=A[:, 1:T + 1], data0=wrow, data1=ekv,
                        initial=0.0, op0=AL.mult, op1=AL.add)
                    nc.vector.scalar_tensor_tensor(
                        out=Bt[:, 0:T], in0=ek, scalar=col(CEU + m),
                        in1=Bt[:, 0:T], op0=AL.mult, op1=AL.add)
                    rec = p_wt.tile([128, T], BF16, tag="rec", name=f"rec{m}")
                    with nc.allow_low_precision(reason="wkv ratio bf16"):
                        nc.vector.reciprocal(rec, Bt[:, 0:T])
                    nc.vector.scalar_tensor_tensor(
                        out=A[:, 0:T], in0=ekv, scalar=col(CEU + m),
                        in1=A[:, 0:T], op0=AL.mult, op1=AL.add)
                    return A, rec

                def wkv_tail(m, A, rec, sr):
                    pr_m, j_m = m // 2, m % 2
                    y = p_wt.tile([128, T], BF16, tag="y", name=f"y{m}")
                    nc.gpsimd.tensor_mul(y, A[:, 0:T], rec)
                    sy = p_wt.tile([128, T], BF16, tag="sy", name=f"sy{m}")
                    nc.gpsimd.tensor_mul(sy, y, sr)
                    nc.scalar.copy(sryh[pr_m][:, j_m, :], sy)
                    nc.vector.tensor_sub(sryl[pr_m][:, j_m, :], sy,
                                         sryh[pr_m][:, j_m, :])

                for m in (range(NCB) if KPHASES >= 2 else ()):
                    ek = p_kvs.tile([128, T], BF16, tag="ek", name=f"ek{m}")
                    vv = p_kvs.tile([128, T], BF16, tag="vv", name=f"vv{m}")
                    sr = p_kvs.tile([128, T], BF16, tag="sr", name=f"sr{m}")
                    for (wt, dst, act, bcol) in (
                            (wk, ek, AF.Exp, CBK), (wv, vv, AF.Identity, CBV),
                            (wr, sr, AF.Sigmoid, CBR)):
                        for n in range(NT):
                            pmm = ps_mm.tile([128, 512], F32, tag="pmm")
                            for a in range(2):
                                for k2 in range(NPR):
                                    nc.tensor.matmul(
                                        pmm, wt[:, a, k2, :,
                                                m * 128:(m + 1) * 128],
                                        zsl(k2, a, n),
                                        start=(a == 0 and k2 == 0),
                                        stop=(a == 1 and k2 == NPR - 1),
                                        perf_mode=DR)
                            dsl = dst[:, n * 512:(n + 1) * 512]
                            nc.scalar.activation(dsl, pmm, act,
                                                 bias=col(bcol + m), scale=INV)
                    if KPHASES >= 3:
                        A, rec = wkv_front(m, ek, vv)
                        wkv_state[m] = (A, rec, sr)
                        if m >= 1:
                            wkv_tail(m - 1, *wkv_state.pop(m - 1))
                if KPHASES >= 3:
                    wkv_tail(NCB - 1, *wkv_state.pop(NCB - 1))

            es_w.close()
            es_z.close()

            # ===== PHASE C: out-proj + residual -> x2, fused LN2 stats =====
            es_x2 = contextlib.ExitStack()
            p_x2 = es_x2.enter_context(tc.tile_pool(name="p_x2", bufs=1))
            x2 = [p_x2.tile([128, T], F32R, tag=f"x2_{cb}", name=f"x2_{cb}")
                  for cb in range(NCB)]
            es_z2 = contextlib.ExitStack()
            p_z2 = es_z2.enter_context(tc.tile_pool(name="p_z2", bufs=1,
                                                    side="right"))
            z2t = [p_z2.tile([128, T + 1], BF16, tag=f"z2_{cb}", name=f"z2_{cb}")
                   for cb in range(NCB)]
            with tc.tile_pool(name="p_xr", bufs=2) as p_xr, \
                 tc.tile_pool(name="p_tmpD", bufs=1) as p_tmpD, \
                 tc.tile_pool(name="p_statD", bufs=1) as p_statD:
              with tc.tile_pool(name="ps_mo", bufs=4, space="PSUM") as ps_mo:
                for m in (range(NCB) if KPHASES >= 4 else ()):
                    xr = p_xr.tile([128, T], F32, tag="xr")
                    nc.sync.dma_start(out=xr, in_=xT_sp[m][:])
                    for n in range(NT):
                        sl = slice(n * 512, (n + 1) * 512)
                        pmm = ps_mo.tile([128, 512], F32, tag="pmo")
                        first = True
                        for (wi, ss) in ((0, sryh), (1, sryh), (0, sryl)):
                            for k2 in range(NPR):
                                nc.tensor.matmul(
                                    pmm, wo[wi][:, k2, :, m * 128:(m + 1) * 128],
                                    ss[k2][:, :, sl],
                                    start=first,
                                    stop=(wi == 0 and ss is sryl
                                          and k2 == NPR - 1),
                                    perf_mode=DR)
                                first = False
                        nc.vector.scalar_tensor_tensor(
                            out=x2[m][:, sl], in0=pmm, scalar=INV,
                            in1=xr[:, sl], op0=AL.mult, op1=AL.add)
                    nc.sync.dma_start(out=x2_sp[m][:], in_=x2[m].bitcast(F32))
              # ---- LN2 stats + finish -> z2 (plain bf16, col 0 zero) ----
              if True:
                if KPHASES >= 5:
                    with tc.tile_pool(name="ps_stD", bufs=1,
                                      space="PSUM") as ps_stD:
                        stat_ps2 = (ps_stD.tile([1, T], F32, tag="meanD", name="meanD"),
                                    ps_stD.tile([1, T], F32, tag="msqD", name="msqD"))
                        for n in range(NT):
                            sl2 = slice(n * 512, (n + 1) * 512)
                            for cb in range(NCB):
                                ln_contrib(stat_ps2, p_tmpD, x2[cb], cb, sl2,
                                           "sqD")
                        meanD, msqD = ln_to_sbuf(stat_ps2, p_statD)
                    rstd_b2, mrstd_b2 = ln_finish(meanD, msqD, p_statD, "D")
                    for cb in range(NCB):
                        nc.vector.memset(z2t[cb][:, 0:1], 0.0)
                        zt = p_tmpD.tile([128, T], F32, tag="zt2", bufs=1)
                        nc.vector.tensor_mul(zt, x2[cb].bitcast(F32), rstd_b2)
                        dst = z2t[cb][:, 1:T + 1]
                        if cb % 2 == 0:
                            nc.vector.tensor_sub(dst, zt, mrstd_b2)
                        else:
                            nc.gpsimd.tensor_sub(dst, zt, mrstd_b2)
            es_x2.close()
            es_sry.close()
            es_wo.close()

            # FFN weights: fwv hi/lo resident fp8; fwr till srf; fwk streamed
            es_fw = contextlib.ExitStack()
            p_fw = es_fw.enter_context(tc.tile_pool(name="p_fw", bufs=1))
            fwv = [p_fw.tile([128, 16, 2, 1024], FP8, tag=f"fwv{i}",
                             name=f"fwv{i}") for i in range(2)]
            if KPHASES >= 5:
                nc.sync.dma_start(out=fwv[0], in_=fwv_d[0])
                nc.sync.dma_start(out=fwv[1], in_=fwv_d[1])

            # ============ PHASE E: xf lerp (f_tmk == f_tmr), fWr -> srf ========
            es_xf = contextlib.ExitStack()
            p_xf = es_xf.enter_context(tc.tile_pool(name="p_xf", bufs=1))
            xfh = [p_xf.tile([128, 2, T], FP8, tag=f"xfh{pr}", name=f"xfh{pr}")
                   for pr in range(NPR)]
            xfl = [p_xf.tile([128, 2, T], FP8, tag=f"xfl{pr}", name=f"xfl{pr}")
                   for pr in range(NPR)]
            with tc.tile_pool(name="p_te", bufs=3) as p_te:
                for cb in (range(NCB) if KPHASES >= 6 else ()):
                    pr, j = cb // 2, cb % 2
                    t1 = p_te.tile([128, T], BF16, tag="t1")
                    nc.scalar.mul(t1, z2t[cb][:, 0:T], col(CFT1 + cb))
                    xfb = p_te.tile([128, T], BF16, tag="xfb")
                    nc.vector.scalar_tensor_tensor(
                        out=xfb, in0=z2t[cb][:, 1:T + 1],
                        scalar=col(CFT + cb), in1=t1, op0=AL.mult, op1=AL.add)
                    nc.scalar.copy(xfh[pr][:, j, :], xfb)
                    nc.gpsimd.tensor_sub(xfl[pr][:, j, :], xfb, xfh[pr][:, j, :])
            es_z2.close()

            es_srf = contextlib.ExitStack()
            p_srf = es_srf.enter_context(tc.tile_pool(name="p_srf", bufs=1))
            srf = [p_srf.tile([128, T], FP8, tag=f"srf{m}", name=f"srf{m}")
                   for m in range(NCB)]
            with tc.tile_pool(name="p_fwr", bufs=1) as p_fwr, \
                 tc.tile_pool(name="ps_fr", bufs=4, space="PSUM") as ps_fr:
                fwr = [p_fwr.tile([128, 4, 2, 1024], FP8, tag=f"fwr{i}",
                                  name=f"fwr{i}") for i in range(2)]
                if KPHASES >= 6:
                    nc.sync.dma_start(out=fwr[0], in_=fwr_d[0])
                    nc.sync.dma_start(out=fwr[1], in_=fwr_d[1])
                for m in (range(NCB) if KPHASES >= 6 else ()):
                    for n in range(NT):
                        pmm = ps_fr.tile([128, 512], F32, tag="pfr")
                        first = True
                        for (wi, xs) in ((0, xfh), (1, xfh), (0, xfl)):
                            for k2 in range(NPR):
                                nc.tensor.matmul(
                                    pmm, fwr[wi][:, k2, :, m * 128:(m + 1) * 128],
                                    xs[k2][:, :, n * 512:(n + 1) * 512],
                                    start=first,
                                    stop=(wi == 0 and xs is xfl and k2 == NPR - 1),
                                    perf_mode=DR)
                                first = False
                        nc.scalar.activation(srf[m][:, n * 512:(n + 1) * 512],
                                             pmm, AF.Sigmoid, bias=col(CBFR + m),
                                             scale=INV)

            # ============ PHASE F: FFN k/v matmuls + output ============
            # 3-pass residual fp8: W*x ~ Wh*xh + Wl*xh + Wh*xl
            with tc.tile_pool(name="p_fwkg", bufs=2) as p_fwkg, \
                 tc.tile_pool(name="p_kk", bufs=1) as p_kk, \
                 tc.tile_pool(name="p_rl", bufs=4) as p_rl, \
                 tc.tile_pool(name="p_x2c", bufs=3) as p_x2c, \
                 tc.tile_pool(name="p_fin", bufs=2) as p_fin, \
                 tc.tile_pool(name="p_ost", bufs=1) as p_ost, \
                 tc.tile_pool(name="ps_fk", bufs=3, space="PSUM") as ps_fk, \
                 tc.tile_pool(name="ps_fo", bufs=2, space="PSUM") as ps_fo, \
                 tc.tile_pool(name="ps_ot", bufs=3, space="PSUM") as ps_ot:
                for n in (range(NT) if KPHASES >= 7 else ()):
                    sl = slice(n * 512, (n + 1) * 512)
                    kkh = [p_kk.tile([128, 2, 512], FP8, tag=f"kkh{hp}",
                                     name=f"kkh{hp}_{n}") for hp in range(NHP)]
                    kkl = [p_kk.tile([128, 2, 512], FP8, tag=f"kkl{hp}",
                                     name=f"kkl{hp}_{n}") for hp in range(NHP)]
                    for g in range(8):
                        fg = [p_fwkg.tile([128, 4, 2, 512], FP8, tag=f"fwkg{i}",
                                          name=f"fwkg{i}_{n}_{g}")
                              for i in range(2)]
                        nc.sync.dma_start(out=fg[0], in_=fwk_d[0, g])
                        nc.sync.dma_start(out=fg[1], in_=fwk_d[1, g])
                        for i in range(4):
                            hb = g * 4 + i
                            hp, jh = hb // 2, hb % 2
                            pkk = ps_fk.tile([128, 512], F32, tag="pkk")
                            first = True
                            for (wi, xs) in ((0, xfh), (1, xfh), (0, xfl)):
                                for k2 in range(NPR):
                                    nc.tensor.matmul(
                                        pkk,
                                        fg[wi][:, k2, :, i * 128:(i + 1) * 128],
                                        xs[k2][:, :, sl],
                                        start=first,
                                        stop=(wi == 0 and xs is xfl
                                              and k2 == NPR - 1),
                                        perf_mode=DR)
                                    first = False
                            rl = p_rl.tile([128, 512], BF16, tag="rl")
                            if hb % 2 == 0:
                                nc.scalar.activation(rl, pkk, AF.Relu,
                                                     bias=col(CBFK + hb),
                                                     scale=INV)
                            else:
                                nc.vector.tensor_scalar(
                                    out=rl, in0=pkk, scalar1=INV,
                                    scalar2=0.0, op0=AL.mult, op1=AL.max)
                            t2 = p_rl.tile([128, 512], BF16, tag="t2")
                            nc.vector.tensor_mul(t2, rl, rl)
                            dh = kkh[hp][:, jh, :]
                            if hb % 2 == 0:
                                nc.scalar.copy(dh, t2)
                            else:
                                nc.vector.tensor_copy(out=dh, in_=t2)
                            nc.gpsimd.tensor_sub(kkl[hp][:, jh, :], t2, dh)
                    osts = [p_ost.tile([128, C], F32, tag=f"ost{j}",
                                       name=f"ost{n}_{j}") for j in range(4)]
                    for m in range(NCB):
                        po = ps_fo.tile([128, 512], F32, tag="po")
                        first = True
                        for (wi, ks) in ((0, kkh), (1, kkh), (0, kkl)):
                            for hp in range(NHP):
                                nc.tensor.matmul(
                                    po, fwv[wi][:, hp, :, m * 128:(m + 1) * 128],
                                    ks[hp],
                                    start=first,
                                    stop=(wi == 0 and ks is kkl
                                          and hp == NHP - 1),
                                    perf_mode=DR)
                                first = False
                        x2c = p_x2c.tile([128, 512], F32, tag="x2c")
                        nc.sync.dma_start(out=x2c, in_=x2_sp[m][:, sl])
                        rkv = p_fin.tile([128, 512], F32, tag="rkv")
                        nc.vector.scalar_tensor_tensor(
                            out=rkv, in0=po, scalar=INV, in1=srf[m][:, sl],
                            op0=AL.mult, op1=AL.mult)
                        fin = p_fin.tile([128, 512], F32, tag="fin")
                        if m % 2 == 0:
                            nc.gpsimd.tensor_add(fin, rkv, x2c)
                        else:
                            nc.vector.tensor_add(fin, rkv, x2c)
                        for j in range(4):
                            pt = ps_ot.tile([128, 128], F32, tag="ptr")
                            nc.tensor.transpose(pt, fin[:, j * 128:(j + 1) * 128],
                                                ident)
                            dst = osts[j][:, m * 128:(m + 1) * 128]
                            if (m + j) % 2 == 0:
                                nc.scalar.copy(dst, pt)
                            else:
                                nc.vector.tensor_copy(out=dst, in_=pt)
                    for j in range(4):
                        tb = n * 4 + j
                        nc.sync.dma_start(out=out_d[tb * 128:(tb + 1) * 128, :],
                                          in_=osts[j])
            es_srf.close()
            es_xf.close()
            es_fw.close()
    nc.finalize()
    return nc


_PROG = None


def _get_prog():
    global _PROG
    if _PROG is None:
        nc = bacc.Bacc()
        _PROG = _emit(nc)
    return _PROG


def _pair_w(WT, M_out):
    """WT: [K_in, M_out] fp8 (lhsT layout) -> [128, K_in//256, 2, M_out] flat."""
    K_in = WT.shape[0]
    npr = K_in // 256
    return np.ascontiguousarray(
        WT.reshape(npr, 2, 128, M_out).transpose(2, 0, 1, 3).reshape(128, -1))


def _q8_hl(WT):
    """WT f32 (pre-scaled by WS) -> (hi, lo) fp8 arrays."""
    f8 = ml_dtypes.float8_e4m3
    Ws = np.asarray(WT, np.float32) * np.float32(WS)
    assert np.abs(Ws).max() < 230.0
    hi = Ws.astype(f8)
    lo = (Ws - hi.astype(np.float32)).astype(f8)
    return hi, lo


def _fwk_hl(WT):
    """WT: [C, HID] -> fp8 [2(hl), 8(g), 128, 4(k2)*2(j)*512]; g = hid cols 512g."""
    hi, lo = _q8_hl(WT)
    out = []
    for W8 in (hi, lo):
        # pair layout per group: [128, k2, j, 512]
        Wp = W8.reshape(4, 2, 128, HID)  # [k2, j, c128, h]
        out.append(np.stack(
            [np.ascontiguousarray(
                Wp[:, :, :, g * 512:(g + 1) * 512].transpose(2, 0, 1, 3)
                .reshape(128, -1)) for g in range(8)]))
    return np.ascontiguousarray(np.stack(out))


def _fwv_hl(WT):
    """WT: [HID, C] -> fp8 [2(hl), 128, 16*2*1024] pair layout."""
    hi, lo = _q8_hl(WT)
    return np.ascontiguousarray(np.stack([_pair_w(W8, C) for W8 in (hi, lo)]))


def _q8s(W):
    """Scale by WS, quantize to fp8e4 (checks range)."""
    f8 = ml_dtypes.float8_e4m3
    Ws = np.asarray(W, np.float32) * np.float32(WS)
    assert np.abs(Ws).max() < 230.0, "weight scale overflow"
    return Ws.astype(f8)


def _prep_inputs(x, ln1_g, ln1_b, ln2_g, ln2_b, time_decay, time_first,
                 tmk, tmv, tmr, Wk, Wv, Wr, Wo, f_tmk, f_tmr, fWk, fWr, fWv):
    f32 = np.float32
    x = np.asarray(x, f32)
    g1 = np.asarray(ln1_g, f32); b1 = np.asarray(ln1_b, f32)
    g2 = np.asarray(ln2_g, f32); b2 = np.asarray(ln2_b, f32)
    td = np.asarray(time_decay, np.float64); tf = np.asarray(time_first, np.float64)
    tmk = np.asarray(tmk, f32).reshape(C); tmv = np.asarray(tmv, f32).reshape(C)
    tmr = np.asarray(tmr, f32).reshape(C)
    ftmk = np.asarray(f_tmk, f32).reshape(C); ftmr = np.asarray(f_tmr, f32).reshape(C)
    assert np.array_equal(ftmk, ftmr), "kernel assumes f_tmk == f_tmr"
    Wk = np.asarray(Wk, f32); Wv = np.asarray(Wv, f32); Wr = np.asarray(Wr, f32)
    Wo = np.asarray(Wo, f32); fWk = np.asarray(fWk, f32); fWr = np.asarray(fWr, f32)
    fWv = np.asarray(fWv, f32)

    Wk1 = Wk * g1[None, :]; Wv1 = Wv * g1[None, :]; Wr1 = Wr * g1[None, :]
    bk = Wk @ b1; bv = Wv @ b1; br = Wr @ b1
    fWk1 = fWk * g2[None, :]; fWr1 = fWr * g2[None, :]
    bfk = fWk @ b2; bfr = fWr @ b2
    assert np.allclose(bfk, 0.0), "kernel assumes zero ln2 beta for relu path"

    wbar = np.exp(-np.exp(td)).astype(f32)
    eu = np.exp(tf).astype(f32)

    def packc(v):
        return np.asarray(v, f32).reshape(-1, 128).T

    cst = np.zeros((128, NCOLS), f32)
    cst[:, CW:CW + 8] = packc(wbar)
    cst[:, CEU:CEU + 8] = packc(eu)
    cst[:, CBK:CBK + 8] = packc(bk)
    cst[:, CBV:CBV + 8] = packc(bv)
    cst[:, CBR:CBR + 8] = packc(br)
    cst[:, CFT:CFT + 8] = packc(ftmk)
    cst[:, CFT1:CFT1 + 8] = packc(1 - ftmk)
    cst[:, CBFR:CBFR + 8] = packc(bfr)
    cst[:, CEPS] = EPS
    cst[:, CBFK:CBFK + 32] = packc(bfk)

    def lerp_pair(W1, tm):
        # [128, 2(ab), 4(k2), 2(j), 1024] flat; a=0: W*tm, a=1: W*(1-tm)
        Wa = _pair_w(_q8s((W1 * tm[None, :]).T), C)
        Wb = _pair_w(_q8s((W1 * (1 - tm)[None, :]).T), C)
        return np.ascontiguousarray(
            np.stack([Wa.reshape(128, 4, 2, 1024),
                      Wb.reshape(128, 4, 2, 1024)], axis=1).reshape(128, -1))

    shared = {
        "wk": lerp_pair(Wk1, tmk),
        "wv": lerp_pair(Wv1, tmv),
        "wr": lerp_pair(Wr1, tmr),
        "wo": _fwv_hl(Wo.T),
        "fwr": _fwv_hl(fWr1.T),
        "fwk": _fwk_hl(fWk1.T),
        "fwv": _fwv_hl(fWv.T),
        "cst": cst,
        "ones1": np.ones((128, 1), f32),
        "onesb": np.ones((1, 128), ml_dtypes.bfloat16),
        "ident": np.eye(128, dtype=f32),
    }
    in_maps = [dict(shared, x=np.ascontiguousarray(x[b])) for b in range(B)]
    return in_maps


def _run(in_maps, trace=False, **kw):
    nc = _get_prog()
    res = run_bass_kernel_spmd(nc, in_maps, core_ids=list(range(B)), trace=trace,
                               **kw)
    out = np.stack([np.asarray(res.results[b]["out"]) for b in range(B)], axis=0)
    return out.astype(np.float32), res


def kernel(*a, **kw):
    out, _ = _run(_prep_inputs(*a, **kw))
    return out


if __name__ == "__main__":
    _get_prog()
    print("program built ok")


# revision 5
# speedup vs baseline: 1.0493x; 1.0046x over previous
"""RWKV-4 block (nn_Block_5669356833485) Trainium2 Bass kernel.

B=8, T=2048, C=1024, HID=4096. B-sharded across 8 NeuronCores (1 batch/core).
Feature-major layout [C-partitions, T-free].

fp8e4 DoubleRow matmuls (256-wide contraction, 0.5 cyc/row). Weights are
host-scaled by 128 before fp8 quantization (their natural ~0.02 magnitude
falls in e4m3's subnormal range) and unscaled in the matmul epilogues.
Time-mix lerps are folded into the matmuls by doubling the contraction
against z and shifted-z (z pair tiles [128, 2, 2064], data offset 16, pair
stride %16==0 per DoubleRow requirements). LN stats are pipelined into the
producing loops; WKV (bf16 scans, fp32 state) interleaves per channel block
with the projections. ek/v/sr/srf stay in SBUF; only xT and x2 round-trip
through DRAM for the residual adds.
Self-contained: hardcodes shapes; no sibling imports.
"""
import os
import sys
sys.path.insert(0, '/opt/trn_rl_repo')

KPHASES = int(os.environ.get("KPHASES", "99"))

import numpy as np
import ml_dtypes

import concourse.bass as bass
from concourse import bacc
import concourse.mybir as mybir
import concourse.tile as tile
from concourse.bass_utils import run_bass_kernel_spmd

F32 = mybir.dt.float32
F32R = mybir.dt.float32r
BF16 = mybir.dt.bfloat16
FP8 = mybir.dt.float8e4
AL = mybir.AluOpType
AF = mybir.ActivationFunctionType
DR = mybir.MatmulPerfMode.DoubleRow

B, T, C, HID = 8, 2048, 1024, 4096
NCB = C // 128          # 8 channel blocks
NPR = NCB // 2          # 4 channel pair-blocks
NHB = HID // 128        # 32 hidden blocks
NHP = NHB // 2          # 16 hidden pair-blocks
NT = T // 512           # 4 n-slices of 512
NTB = T // 128          # 16 token blocks
PAD = 16                # z pair tiles: [128, 2, PAD+T]; pair stride %16==0
TP = T + PAD
EPS = 1e-5
WS = 128.0              # weight pre-quantization scale
INV = 1.0 / WS

# cst columns (per 128-partition, indexed by block)
CW = 0        # wbar          [0:8)   by cb
CEU = 8       # exp(tf)       [8:16)  by cb
CBK = 16      # bk            [16:24) by m
CBV = 24      # bv            [24:32) by m
CBR = 32      # br            [32:40) by m
CFT = 40      # ftmk          [40:48) by cb
CFT1 = 48     # 1-ftmk        [48:56) by cb
CBFR = 56     # bfr           [56:64) by m
CEPS = 64     # eps           col 64
CBFK = 72     # bfk           [72:104) by hb
NCOLS = 104


def _emit(nc):
    # ---------------- DRAM I/O ----------------
    x_d = nc.declare_dram_parameter("x", [T, C], F32, isOutput=False)
    wk_d = nc.declare_dram_parameter("wk", [128, 2 * 4 * 2 * 1024], FP8, isOutput=False)
    wv_d = nc.declare_dram_parameter("wv", [128, 2 * 4 * 2 * 1024], FP8, isOutput=False)
    wr_d = nc.declare_dram_parameter("wr", [128, 2 * 4 * 2 * 1024], FP8, isOutput=False)
    wo_d = nc.declare_dram_parameter("wo", [2, 128, 4 * 2 * 1024], FP8, isOutput=False)
    fwr_d = nc.declare_dram_parameter("fwr", [2, 128, 4 * 2 * 1024], FP8, isOutput=False)
    fwk_d = nc.declare_dram_parameter("fwk", [2, 8, 128, 4 * 2 * 512], FP8, isOutput=False)
    fwv_d = nc.declare_dram_parameter("fwv", [2, 128, 16 * 2 * 1024], FP8, isOutput=False)
    cst_d = nc.declare_dram_parameter("cst", [128, NCOLS], F32, isOutput=False)
    ones1_d = nc.declare_dram_parameter("ones1", [128, 1], F32R, isOutput=False)
    onesb_d = nc.declare_dram_parameter("onesb", [1, 128], BF16, isOutput=False)
    ident_d = nc.declare_dram_parameter("ident", [128, 128], F32, isOutput=False)
    out_d = nc.declare_dram_parameter("out", [T, C], F32, isOutput=True)

    # DRAM scratch (per-cb granularity for fine deps)
    xT_sp = [nc.dram_tensor(f"xT_sp{i}", [128, T], F32) for i in range(NCB)]
    x2_sp = [nc.dram_tensor(f"x2_sp{i}", [128, T], F32) for i in range(NCB)]

    import contextlib

    with tile.TileContext(nc, pool_alloc_mode="queue") as tc:
        with tc.tile_pool(name="pc", bufs=1) as pc:
            cst = pc.tile([128, NCOLS], F32)
            nc.sync.dma_start(out=cst, in_=cst_d[:])
            ones1 = pc.tile([128, 1], F32R)
            nc.sync.dma_start(out=ones1, in_=ones1_d[:])
            onesb = pc.tile([1, 128], BF16)
            nc.sync.dma_start(out=onesb, in_=onesb_d[:])
            ident = pc.tile([128, 128], F32)
            nc.sync.dma_start(out=ident, in_=ident_d[:])
            ones_bf = pc.tile([128, T], BF16)
            nc.vector.memset(ones_bf, 1.0)

            def col(j):
                return cst[:, j:j + 1]

            # ---- incremental LN stats: two [1, T] psum tiles ----
            def ln_contrib(stat_ps, pool_tmp, src_f32r, cb, sl, tag):
                """Add channel-block cb's contribution for column slice sl."""
                mean_ps, msq_ps = stat_ps
                w = sl.stop - sl.start
                sq = pool_tmp.tile([128, w], F32R, tag=tag, bufs=3)
                nc.scalar.activation(sq, src_f32r.bitcast(F32)[:, sl], AF.Square)
                nc.tensor.matmul(mean_ps[:, sl], ones1, src_f32r[:, sl],
                                 start=(cb == 0), stop=(cb == NCB - 1))
                nc.tensor.matmul(msq_ps[:, sl], ones1, sq,
                                 start=(cb == 0), stop=(cb == NCB - 1))

            def ln_to_sbuf(stat_ps, pool_stat):
                mean_ps, msq_ps = stat_ps
                mean = pool_stat.tile([1, T], BF16, tag="mean_sb")
                msq = pool_stat.tile([1, T], BF16, tag="msq_sb")
                nc.scalar.mul(mean, mean_ps, 1.0 / C)
                nc.scalar.mul(msq, msq_ps, 1.0 / C)
                return mean, msq

            def ln_finish(mean, msq, pool_stat, uid):
                var = pool_stat.tile([1, T], BF16, tag="var_sb")
                nc.vector.tensor_mul(var, mean, mean)
                nc.vector.tensor_sub(var, msq, var)
                lnv = pool_stat.tile([1, T], BF16, tag="msq_sb", name=f"lnv{uid}")
                nc.scalar.activation(lnv, var, AF.Ln,
                                     bias=cst[0:1, CEPS:CEPS + 1], scale=1.0)
                rstd = pool_stat.tile([1, T], BF16, tag="var_sb", name=f"rstd{uid}")
                nc.scalar.activation(rstd, lnv, AF.Exp, bias=0.0, scale=-0.5)
                mrstd = pool_stat.tile([1, T], BF16, tag="mrstd_sb")
                nc.vector.tensor_mul(mrstd, mean, rstd)
                rstd_b = pool_stat.tile([128, T], BF16, tag="rstd_b")
                mrstd_b = pool_stat.tile([128, T], BF16, tag="mrstd_b")
                with tc.tile_pool(name=f"ps_bc{uid}", bufs=2, space="PSUM") as ps_bc:
                    for (src_s, dst) in ((rstd, rstd_b), (mrstd, mrstd_b)):
                        for n in range(NT):
                            sl = slice(n * 512, (n + 1) * 512)
                            bc = ps_bc.tile([128, 512], F32, tag="bc")
                            nc.tensor.matmul(bc, onesb, src_s[:, sl],
                                             start=True, stop=True)
                            if n % 2 == 0:
                                nc.scalar.copy(dst[:, sl], bc)
                            else:
                                nc.vector.tensor_copy(out=dst[:, sl], in_=bc)
                return rstd_b, mrstd_b

            # z pair tiles live through phase B (attention)
            es_z = contextlib.ExitStack()
            p_z = es_z.enter_context(tc.tile_pool(name="p_z", bufs=1, side="right"))
            z8 = [p_z.tile([128, 2, TP], FP8, tag=f"z{pr}", name=f"z{pr}")
                  for pr in range(NPR)]
            for pr in range(NPR):
                nc.vector.memset(z8[pr][:, :, 0:PAD], 0.0)

            # attention weights: prefetch during phase A
            es_wo = contextlib.ExitStack()
            p_wo = es_wo.enter_context(tc.tile_pool(name="p_wo", bufs=1))
            wo = [p_wo.tile([128, 4, 2, 1024], FP8, tag=f"wo{i}",
                            name=f"wo{i}") for i in range(2)]
            es_w = contextlib.ExitStack()
            p_w = es_w.enter_context(tc.tile_pool(name="p_w", bufs=1, side="right"))
            wk = p_w.tile([128, 2, 4, 2, 1024], FP8, tag="wk")
            wv = p_w.tile([128, 2, 4, 2, 1024], FP8, tag="wv")
            wr = p_w.tile([128, 2, 4, 2, 1024], FP8, tag="wr")

            # ================= PHASE A: load, transpose, LN1, z =================
            with tc.tile_pool(name="p_xT", bufs=1) as p_xT:
                xT = [p_xT.tile([128, T], F32R, tag=f"xT{cb}", name=f"xT{cb}")
                      for cb in range(NCB)]
                with tc.tile_pool(name="p_tmpA", bufs=1) as p_tmpA, \
                     tc.tile_pool(name="p_statA", bufs=1) as p_statA:
                    meanA = p_statA.tile([1, T], BF16, tag="mean_sb")
                    msqA = p_statA.tile([1, T], BF16, tag="msq_sb")
                    with tc.tile_pool(name="p_ld", bufs=3) as p_ld, \
                         tc.tile_pool(name="ps_stA", bufs=2,
                                      space="PSUM") as ps_stA, \
                         tc.tile_pool(name="ps_tr", bufs=4, space="PSUM") as ps_tr:
                        for tb in range(NTB):
                            xt = p_ld.tile([128, C], F32, tag="xtok")
                            nc.sync.dma_start(out=xt,
                                              in_=x_d[tb * 128:(tb + 1) * 128, :])
                            for cb in range(NCB):
                                pt = ps_tr.tile([128, 128], F32, tag="tr")
                                nc.tensor.transpose(
                                    pt, xt[:, cb * 128:(cb + 1) * 128], ident)
                                dst = xT[cb][:, tb * 128:(tb + 1) * 128]
                                if (tb + cb) % 2 == 0:
                                    nc.scalar.copy(dst, pt)
                                else:
                                    nc.vector.tensor_copy(out=dst, in_=pt)
                            if tb % 4 == 3:
                                n = tb // 4
                                sl = slice(n * 512, (n + 1) * 512)
                                mp = ps_stA.tile([1, 512], F32, tag="mA",
                                                 name=f"mA{n}")
                                qp = ps_stA.tile([1, 512], F32, tag="qA",
                                                 name=f"qA{n}")
                                for cb in range(NCB):
                                    sq = p_tmpA.tile([128, 512], F32R,
                                                     tag="sqA", bufs=3)
                                    nc.scalar.activation(
                                        sq, xT[cb].bitcast(F32)[:, sl],
                                        AF.Square)
                                    nc.tensor.matmul(
                                        mp, ones1, xT[cb][:, sl],
                                        start=(cb == 0), stop=(cb == NCB - 1))
                                    nc.tensor.matmul(
                                        qp, ones1, sq,
                                        start=(cb == 0), stop=(cb == NCB - 1))
                                nc.scalar.mul(meanA[:, sl], mp, 1.0 / C)
                                nc.scalar.mul(msqA[:, sl], qp, 1.0 / C)
                    # attention weight loads: queued after x-in, before spills
                    nc.sync.dma_start(out=wk, in_=wk_d[:])
                    nc.sync.dma_start(out=wv, in_=wv_d[:])
                    nc.sync.dma_start(out=wr, in_=wr_d[:])
                    nc.sync.dma_start(out=wo[0], in_=wo_d[0])
                    nc.sync.dma_start(out=wo[1], in_=wo_d[1])
                    for cb in range(NCB):
                        nc.sync.dma_start(out=xT_sp[cb][:], in_=xT[cb].bitcast(F32))
                    rstd_b, mrstd_b = ln_finish(meanA, msqA, p_statA, "A")
                    for cb in range(NCB):
                        pr, j = cb // 2, cb % 2
                        zt = p_tmpA.tile([128, T], F32, tag="zt", bufs=2)
                        nc.vector.tensor_mul(zt, xT[cb].bitcast(F32), rstd_b)
                        dst = z8[pr][:, j, PAD:PAD + T]
                        if cb % 2 == 0:
                            nc.vector.tensor_sub(dst, zt, mrstd_b)
                        else:
                            nc.gpsimd.tensor_sub(dst, zt, mrstd_b)

            # ============ PHASE B: k/v/r projections + WKV per m ============
            es_sry = contextlib.ExitStack()
            p_sry = es_sry.enter_context(tc.tile_pool(name="p_sry", bufs=1))
            sryh = [p_sry.tile([128, 2, T], FP8, tag=f"sryh{pr}", name=f"sryh{pr}")
                    for pr in range(NPR)]
            sryl = [p_sry.tile([128, 2, T], FP8, tag=f"sryl{pr}", name=f"sryl{pr}")
                    for pr in range(NPR)]

            def zsl(k2, a, n):
                # a=0: current tokens; a=1: shifted by one
                lo = PAD - a + n * 512
                return z8[k2][:, :, lo:lo + 512]

            with tc.tile_pool(name="p_kvs", bufs=2) as p_kvs, \
                 tc.tile_pool(name="p_wt", bufs=2) as p_wt, \
                 tc.tile_pool(name="ps_mm", bufs=8, space="PSUM") as ps_mm:
                wkv_state = {}

                def wkv_front(m, ek, vv):
                    # scanB first: depends only on ek (k epilogues), so DVE can
                    # start while Act still runs v/r epilogues. ekv on DVE keeps
                    # the ekv->scanA handoff on-engine (no cross-engine sem).
                    wrow = p_wt.tile([128, T], BF16, tag="wrow", name=f"wr{m}")
                    nc.vector.tensor_scalar(out=wrow, in0=ones_bf,
                                            scalar1=col(CW + m),
                                            scalar2=None, op0=AL.mult)
                    A = p_wt.tile([128, T + 1], BF16, tag="A", name=f"A{m}")
                    Bt = p_wt.tile([128, T + 1], BF16, tag="B", name=f"B{m}")
                    nc.vector.memset(Bt[:, 0:1], 0.0)
                    nc.vector.tensor_tensor_scan(
                        out=Bt[:, 1:T + 1], data0=wrow, data1=ek,
                        initial=0.0, op0=AL.mult, op1=AL.add)
                    ekv = p_wt.tile([128, T], BF16, tag="ekv", name=f"ekv{m}")
                    nc.vector.tensor_mul(ekv, ek, vv)
                    nc.vector.memset(A[:, 0:1], 0.0)
                    nc.vector.tensor_tensor_scan(
                        out=A[:, 1:T + 1], data0=wrow, data1=ekv,
                        initial=0.0, op0=AL.mult, op1=AL.add)
                    nc.vector.scalar_tensor_tensor(
                        out=Bt[:, 0:T], in0=ek, scalar=col(CEU + m),
                        in1=Bt[:, 0:T], op0=AL.mult, op1=AL.add)
                    rec = p_wt.tile([128, T], BF16, tag="rec", name=f"rec{m}")
                    with nc.allow_low_precision(reason="wkv ratio bf16"):
                        nc.vector.reciprocal(rec, Bt[:, 0:T])
                    nc.vector.scalar_tensor_tensor(
                        out=A[:, 0:T], in0=ekv, scalar=col(CEU + m),
                        in1=A[:, 0:T], op0=AL.mult, op1=AL.add)
                    return A, rec

                def wkv_tail(m, A, rec, sr):
                    pr_m, j_m = m // 2, m % 2
                    y = p_wt.tile([128, T], BF16, tag="y", name=f"y{m}")
                    nc.gpsimd.tensor_mul(y, A[:, 0:T], rec)
                    sy = p_wt.tile([128, T], BF16, tag="sy", name=f"sy{m}")
                    nc.gpsimd.tensor_mul(sy, y, sr)
                    nc.scalar.copy(sryh[pr_m][:, j_m, :], sy)
                    nc.vector.tensor_sub(sryl[pr_m][:, j_m, :], sy,
                                         sryh[pr_m][:, j_m, :])

                for m in (range(NCB) if KPHASES >= 2 else ()):
                    ek = p_kvs.tile([128, T], BF16, tag="ek", name=f"ek{m}")
                    vv = p_kvs.tile([128, T], BF16, tag="vv", name=f"vv{m}")
                    sr = p_kvs.tile([128, T], BF16, tag="sr", name=f"sr{m}")
                    for (wt, dst, act, bcol) in (
                            (wk, ek, AF.Exp, CBK), (wv, vv, AF.Identity, CBV),
                            (wr, sr, AF.Sigmoid, CBR)):
                        for n in range(NT):
                            pmm = ps_mm.tile([128, 512], F32, tag="pmm")
                            for a in range(2):
                                for k2 in range(NPR):
                                    nc.tensor.matmul(
                                        pmm, wt[:, a, k2, :,
                                                m * 128:(m + 1) * 128],
                                        zsl(k2, a, n),
                                        start=(a == 0 and k2 == 0),
                                        stop=(a == 1 and k2 == NPR - 1),
                                        perf_mode=DR)
                            dsl = dst[:, n * 512:(n + 1) * 512]
                            nc.scalar.activation(dsl, pmm, act,
                                                 bias=col(bcol + m), scale=INV)
                    if KPHASES >= 3:
                        A, rec = wkv_front(m, ek, vv)
                        wkv_state[m] = (A, rec, sr)
                        if m >= 1:
                            wkv_tail(m - 1, *wkv_state.pop(m - 1))
                if KPHASES >= 3:
                    wkv_tail(NCB - 1, *wkv_state.pop(NCB - 1))

            es_w.close()
            es_z.close()

            # ===== PHASE C: out-proj + residual -> x2, fused LN2 stats =====
            es_x2 = contextlib.ExitStack()
            p_x2 = es_x2.enter_context(tc.tile_pool(name="p_x2", bufs=1))
            x2 = [p_x2.tile([128, T], F32R, tag=f"x2_{cb}", name=f"x2_{cb}")
                  for cb in range(NCB)]
            es_z2 = contextlib.ExitStack()
            p_z2 = es_z2.enter_context(tc.tile_pool(name="p_z2", bufs=1,
                                                    side="right"))
            z2t = [p_z2.tile([128, T + 1], BF16, tag=f"z2_{cb}", name=f"z2_{cb}")
                   for cb in range(NCB)]
            with tc.tile_pool(name="p_xr", bufs=2) as p_xr, \
                 tc.tile_pool(name="p_tmpD", bufs=1) as p_tmpD, \
                 tc.tile_pool(name="p_statD", bufs=1) as p_statD:
              with tc.tile_pool(name="ps_mo", bufs=4, space="PSUM") as ps_mo, \
                   tc.tile_pool(name="ps_st2", bufs=1, space="PSUM") as ps_st2:
                st_half = [(ps_st2.tile([1, 512], F32, tag=f"m{i}", name=f"mD{i}"),
                            ps_st2.tile([1, 512], F32, tag=f"q{i}", name=f"qD{i}"))
                           for i in range(2)]
                for m in (range(NCB) if KPHASES >= 4 else ()):
                    xr = p_xr.tile([128, T], F32, tag="xr")
                    nc.sync.dma_start(out=xr, in_=xT_sp[m][:])
                    for n in range(NT):
                        sl = slice(n * 512, (n + 1) * 512)
                        pmm = ps_mo.tile([128, 512], F32, tag="pmo")
                        first = True
                        for (wi, ss) in ((0, sryh), (1, sryh), (0, sryl)):
                            for k2 in range(NPR):
                                nc.tensor.matmul(
                                    pmm, wo[wi][:, k2, :, m * 128:(m + 1) * 128],
                                    ss[k2][:, :, sl],
                                    start=first,
                                    stop=(wi == 0 and ss is sryl
                                          and k2 == NPR - 1),
                                    perf_mode=DR)
                                first = False
                        nc.vector.scalar_tensor_tensor(
                            out=x2[m][:, sl], in0=pmm, scalar=INV,
                            in1=xr[:, sl], op0=AL.mult, op1=AL.add)
                    nc.sync.dma_start(out=x2_sp[m][:], in_=x2[m].bitcast(F32))
                    if KPHASES >= 5:
                        for i in range(2):
                            sl2 = slice(i * 512, (i + 1) * 512)
                            sq = p_tmpD.tile([128, 512], F32R, tag="sqD", bufs=3)
                            nc.scalar.activation(sq, x2[m].bitcast(F32)[:, sl2],
                                                 AF.Square)
                            nc.tensor.matmul(st_half[i][0], ones1, x2[m][:, sl2],
                                             start=(m == 0), stop=(m == NCB - 1))
                            nc.tensor.matmul(st_half[i][1], ones1, sq,
                                             start=(m == 0), stop=(m == NCB - 1))
                if KPHASES >= 5:
                    meanD = p_statD.tile([1, T], BF16, tag="mean_sb")
                    msqD = p_statD.tile([1, T], BF16, tag="msq_sb")
                    for i in range(2):
                        sl2 = slice(i * 512, (i + 1) * 512)
                        nc.scalar.mul(meanD[:, sl2], st_half[i][0], 1.0 / C)
                        nc.scalar.mul(msqD[:, sl2], st_half[i][1], 1.0 / C)
              # ---- LN2 stats + finish -> z2 (plain bf16, col 0 zero) ----
              if True:
                if KPHASES >= 5:
                    with tc.tile_pool(name="ps_stD", bufs=1,
                                      space="PSUM") as ps_stD:
                        st2 = [(ps_stD.tile([1, 512], F32, tag=f"m2{i}",
                                            name=f"mD2{i}"),
                                ps_stD.tile([1, 512], F32, tag=f"q2{i}",
                                            name=f"qD2{i}")) for i in range(2)]
                        for i in range(2):
                            n = 2 + i
                            sl2 = slice(n * 512, (n + 1) * 512)
                            for cb in range(NCB):
                                sq = p_tmpD.tile([128, 512], F32R, tag="sqD",
                                                 bufs=3)
                                nc.scalar.activation(
                                    sq, x2[cb].bitcast(F32)[:, sl2], AF.Square)
                                nc.tensor.matmul(st2[i][0], ones1,
                                                 x2[cb][:, sl2],
                                                 start=(cb == 0),
                                                 stop=(cb == NCB - 1))
                                nc.tensor.matmul(st2[i][1], ones1, sq,
                                                 start=(cb == 0),
                                                 stop=(cb == NCB - 1))
                            nc.scalar.mul(meanD[:, sl2], st2[i][0], 1.0 / C)
                            nc.scalar.mul(msqD[:, sl2], st2[i][1], 1.0 / C)
                    rstd_b2, mrstd_b2 = ln_finish(meanD, msqD, p_statD, "D")
                    for cb in range(NCB):
                        nc.vector.memset(z2t[cb][:, 0:1], 0.0)
                        zt = p_tmpD.tile([128, T], F32, tag="zt2", bufs=1)
                        nc.vector.tensor_mul(zt, x2[cb].bitcast(F32), rstd_b2)
                        dst = z2t[cb][:, 1:T + 1]
                        if cb % 2 == 0:
                            nc.vector.tensor_sub(dst, zt, mrstd_b2)
                        else:
                            nc.gpsimd.tensor_sub(dst, zt, mrstd_b2)
            es_x2.close()
            es_sry.close()
            es_wo.close()

            # FFN weights: fwv hi/lo resident fp8; fwr till srf; fwk streamed
            es_fw = contextlib.ExitStack()
            p_fw = es_fw.enter_context(tc.tile_pool(name="p_fw", bufs=1))
            fwv = [p_fw.tile([128, 16, 2, 1024], FP8, tag=f"fwv{i}",
                             name=f"fwv{i}") for i in range(2)]
            if KPHASES >= 5:
                nc.sync.dma_start(out=fwv[0], in_=fwv_d[0])
                nc.sync.dma_start(out=fwv[1], in_=fwv_d[1])

            # ============ PHASE E: xf lerp (f_tmk == f_tmr), fWr -> srf ========
            es_xf = contextlib.ExitStack()
            p_xf = es_xf.enter_context(tc.tile_pool(name="p_xf", bufs=1))
            xfh = [p_xf.tile([128, 2, T], FP8, tag=f"xfh{pr}", name=f"xfh{pr}")
                   for pr in range(NPR)]
            xfl = [p_xf.tile([128, 2, T], FP8, tag=f"xfl{pr}", name=f"xfl{pr}")
                   for pr in range(NPR)]
            with tc.tile_pool(name="p_te", bufs=3) as p_te:
                for cb in (range(NCB) if KPHASES >= 6 else ()):
                    pr, j = cb // 2, cb % 2
                    t1 = p_te.tile([128, T], BF16, tag="t1")
                    nc.scalar.mul(t1, z2t[cb][:, 0:T], col(CFT1 + cb))
                    xfb = p_te.tile([128, T], BF16, tag="xfb")
                    nc.vector.scalar_tensor_tensor(
                        out=xfb, in0=z2t[cb][:, 1:T + 1],
                        scalar=col(CFT + cb), in1=t1, op0=AL.mult, op1=AL.add)
                    nc.scalar.copy(xfh[pr][:, j, :], xfb)
                    nc.gpsimd.tensor_sub(xfl[pr][:, j, :], xfb, xfh[pr][:, j, :])
            es_z2.close()

            es_srf = contextlib.ExitStack()
            p_srf = es_srf.enter_context(tc.tile_pool(name="p_srf", bufs=1))
            srf = [p_srf.tile([128, T], FP8, tag=f"srf{m}", name=f"srf{m}")
                   for m in range(NCB)]
            with tc.tile_pool(name="p_fwr", bufs=1) as p_fwr, \
                 tc.tile_pool(name="ps_fr", bufs=4, space="PSUM") as ps_fr:
                fwr = [p_fwr.tile([128, 4, 2, 1024], FP8, tag=f"fwr{i}",
                                  name=f"fwr{i}") for i in range(2)]
                if KPHASES >= 6:
                    nc.sync.dma_start(out=fwr[0], in_=fwr_d[0])
                    nc.sync.dma_start(out=fwr[1], in_=fwr_d[1])
                for m in (range(NCB) if KPHASES >= 6 else ()):
                    for n in range(NT):
                        pmm = ps_fr.tile([128, 512], F32, tag="pfr")
                        first = True
                        for (wi, xs) in ((0, xfh), (1, xfh), (0, xfl)):
                            for k2 in range(NPR):
                                nc.tensor.matmul(
                                    pmm, fwr[wi][:, k2, :, m * 128:(m + 1) * 128],
                                    xs[k2][:, :, n * 512:(n + 1) * 512],
                                    start=first,
                                    stop=(wi == 0 and xs is xfl and k2 == NPR - 1),
                                    perf_mode=DR)
                                first = False
                        nc.scalar.activation(srf[m][:, n * 512:(n + 1) * 512],
                                             pmm, AF.Sigmoid, bias=col(CBFR + m),
                                             scale=INV)

            # ============ PHASE F: FFN k/v matmuls + output ============
            # 3-pass residual fp8: W*x ~ Wh*xh + Wl*xh + Wh*xl
            with tc.tile_pool(name="p_fwkg", bufs=2) as p_fwkg, \
                 tc.tile_pool(name="p_kk", bufs=1) as p_kk, \
                 tc.tile_pool(name="p_rl", bufs=4) as p_rl, \
                 tc.tile_pool(name="p_x2c", bufs=3) as p_x2c, \
                 tc.tile_pool(name="p_fin", bufs=2) as p_fin, \
                 tc.tile_pool(name="p_ost", bufs=1) as p_ost, \
                 tc.tile_pool(name="ps_fk", bufs=3, space="PSUM") as ps_fk, \
                 tc.tile_pool(name="ps_fo", bufs=2, space="PSUM") as ps_fo, \
                 tc.tile_pool(name="ps_ot", bufs=3, space="PSUM") as ps_ot:
                for n in (range(NT) if KPHASES >= 7 else ()):
                    sl = slice(n * 512, (n + 1) * 512)
                    kkh = [p_kk.tile([128, 2, 512], FP8, tag=f"kkh{hp}",
                                     name=f"kkh{hp}_{n}") for hp in range(NHP)]
                    kkl = [p_kk.tile([128, 2, 512], FP8, tag=f"kkl{hp}",
                                     name=f"kkl{hp}_{n}") for hp in range(NHP)]
                    for g in range(8):
                        fg = [p_fwkg.tile([128, 4, 2, 512], FP8, tag=f"fwkg{i}",
                                          name=f"fwkg{i}_{n}_{g}")
                              for i in range(2)]
                        nc.sync.dma_start(out=fg[0], in_=fwk_d[0, g])
                        nc.sync.dma_start(out=fg[1], in_=fwk_d[1, g])
                        for i in range(4):
                            hb = g * 4 + i
                            hp, jh = hb // 2, hb % 2
                            pkk = ps_fk.tile([128, 512], F32, tag="pkk")
                            first = True
                            for (wi, xs) in ((0, xfh), (1, xfh), (0, xfl)):
                                for k2 in range(NPR):
                                    nc.tensor.matmul(
                                        pkk,
                                        fg[wi][:, k2, :, i * 128:(i + 1) * 128],
                                        xs[k2][:, :, sl],
                                        start=first,
                                        stop=(wi == 0 and xs is xfl
                                              and k2 == NPR - 1),
                                        perf_mode=DR)
                                    first = False
                            rl = p_rl.tile([128, 512], BF16, tag="rl")
                            if hb % 2 == 0:
                                nc.scalar.activation(rl, pkk, AF.Relu,
                                                     bias=col(CBFK + hb),
                                                     scale=INV)
                            else:
                                nc.vector.tensor_scalar(
                                    out=rl, in0=pkk, scalar1=INV,
                                    scalar2=0.0, op0=AL.mult, op1=AL.max)
                            t2 = p_rl.tile([128, 512], BF16, tag="t2")
                            nc.vector.tensor_mul(t2, rl, rl)
                            dh = kkh[hp][:, jh, :]
                            if hb % 2 == 0:
                                nc.scalar.copy(dh, t2)
                            else:
                                nc.vector.tensor_copy(out=dh, in_=t2)
                            nc.gpsimd.tensor_sub(kkl[hp][:, jh, :], t2, dh)
                    osts = [p_ost.tile([128, C], F32, tag=f"ost{j}",
                                       name=f"ost{n}_{j}") for j in range(4)]
                    for m in range(NCB):
                        po = ps_fo.tile([128, 512], F32, tag="po")
                        first = True
                        for (wi, ks) in ((0, kkh), (1, kkh), (0, kkl)):
                            for hp in range(NHP):
                                nc.tensor.matmul(
                                    po, fwv[wi][:, hp, :, m * 128:(m + 1) * 128],
                                    ks[hp],
                                    start=first,
                                    stop=(wi == 0 and ks is kkl
                                          and hp == NHP - 1),
                                    perf_mode=DR)
                                first = False
                        x2c = p_x2c.tile([128, 512], F32, tag="x2c")
                        nc.sync.dma_start(out=x2c, in_=x2_sp[m][:, sl])
                        rkv = p_fin.tile([128, 512], F32, tag="rkv")
                        nc.vector.scalar_tensor_tensor(
                            out=rkv, in0=po, scalar=INV, in1=srf[m][:, sl],
                            op0=AL.mult, op1=AL.mult)
                        fin = p_fin.tile([128, 512], F32, tag="fin")
                        if m % 2 == 0:
                            nc.gpsimd.tensor_add(fin, rkv, x2c)
                        else:
                            nc.vector.tensor_add(fin, rkv, x2c)
                        for j in range(4):
                            pt = ps_ot.tile([128, 128], F32, tag="ptr")
                            nc.tensor.transpose(pt, fin[:, j * 128:(j + 1) * 128],
                                                ident)
                            dst = osts[j][:, m * 128:(m + 1) * 128]
                            if (m + j) % 2 == 0:
                                nc.scalar.copy(dst, pt)
                            else:
                                nc.vector.tensor_copy(out=dst, in_=pt)
                    for j in range(4):
                        tb = n * 4 + j
                        nc.sync.dma_start(out=out_d[tb * 128:(tb + 1) * 128, :],
                                          in_=osts[j])
            es_srf.close()
            es_xf.close()
            es_fw.close()
    nc.finalize()
    return nc


_PROG = None


def _get_prog():
    global _PROG
    if _PROG is None:
        nc = bacc.Bacc()
        _PROG = _emit(nc)
    return _PROG


def _pair_w(WT, M_out):
    """WT: [K_in, M_out] fp8 (lhsT layout) -> [128, K_in//256, 2, M_out] flat."""
    K_in = WT.shape[0]
    npr = K_in // 256
    return np.ascontiguousarray(
        WT.reshape(npr, 2, 128, M_out).transpose(2, 0, 1, 3).reshape(128, -1))


def _q8_hl(WT):
    """WT f32 (pre-scaled by WS) -> (hi, lo) fp8 arrays."""
    f8 = ml_dtypes.float8_e4m3
    Ws = np.asarray(WT, np.float32) * np.float32(WS)
    assert np.abs(Ws).max() < 230.0
    hi = Ws.astype(f8)
    lo = (Ws - hi.astype(np.float32)).astype(f8)
    return hi, lo


def _fwk_hl(WT):
    """WT: [C, HID] -> fp8 [2(hl), 8(g), 128, 4(k2)*2(j)*512]; g = hid cols 512g."""
    hi, lo = _q8_hl(WT)
    out = []
    for W8 in (hi, lo):
        # pair layout per group: [128, k2, j, 512]
        Wp = W8.reshape(4, 2, 128, HID)  # [k2, j, c128, h]
        out.append(np.stack(
            [np.ascontiguousarray(
                Wp[:, :, :, g * 512:(g + 1) * 512].transpose(2, 0, 1, 3)
                .reshape(128, -1)) for g in range(8)]))
    return np.ascontiguousarray(np.stack(out))


def _fwv_hl(WT):
    """WT: [HID, C] -> fp8 [2(hl), 128, 16*2*1024] pair layout."""
    hi, lo = _q8_hl(WT)
    return np.ascontiguousarray(np.stack([_pair_w(W8, C) for W8 in (hi, lo)]))


def _q8s(W):
    """Scale by WS, quantize to fp8e4 (checks range)."""
    f8 = ml_dtypes.float8_e4m3
    Ws = np.asarray(W, np.float32) * np.float32(WS)
    assert np.abs(Ws).max() < 230.0, "weight scale overflow"
    return Ws.astype(f8)


def _prep_inputs(x, ln1_g, ln1_b, ln2_g, ln2_b, time_decay, time_first,
                 tmk, tmv, tmr, Wk, Wv, Wr, Wo, f_tmk, f_tmr, fWk, fWr, fWv):
    f32 = np.float32
    x = np.asarray(x, f32)
    g1 = np.asarray(ln1_g, f32); b1 = np.asarray(ln1_b, f32)
    g2 = np.asarray(ln2_g, f32); b2 = np.asarray(ln2_b, f32)
    td = np.asarray(time_decay, np.float64); tf = np.asarray(time_first, np.float64)
    tmk = np.asarray(tmk, f32).reshape(C); tmv = np.asarray(tmv, f32).reshape(C)
    tmr = np.asarray(tmr, f32).reshape(C)
    ftmk = np.asarray(f_tmk, f32).reshape(C); ftmr = np.asarray(f_tmr, f32).reshape(C)
    assert np.array_equal(ftmk, ftmr), "kernel assumes f_tmk == f_tmr"
    Wk = np.asarray(Wk, f32); Wv = np.asarray(Wv, f32); Wr = np.asarray(Wr, f32)
    Wo = np.asarray(Wo, f32); fWk = np.asarray(fWk, f32); fWr = np.asarray(fWr, f32)
    fWv = np.asarray(fWv, f32)

    Wk1 = Wk * g1[None, :]; Wv1 = Wv * g1[None, :]; Wr1 = Wr * g1[None, :]
    bk = Wk @ b1; bv = Wv @ b1; br = Wr @ b1
    fWk1 = fWk * g2[None, :]; fWr1 = fWr * g2[None, :]
    bfk = fWk @ b2; bfr = fWr @ b2
    assert np.allclose(bfk, 0.0), "kernel assumes zero ln2 beta for relu path"

    wbar = np.exp(-np.exp(td)).astype(f32)
    eu = np.exp(tf).astype(f32)

    def packc(v):
        return np.asarray(v, f32).reshape(-1, 128).T

    cst = np.zeros((128, NCOLS), f32)
    cst[:, CW:CW + 8] = packc(wbar)
    cst[:, CEU:CEU + 8] = packc(eu)
    cst[:, CBK:CBK + 8] = packc(bk)
    cst[:, CBV:CBV + 8] = packc(bv)
    cst[:, CBR:CBR + 8] = packc(br)
    cst[:, CFT:CFT + 8] = packc(ftmk)
    cst[:, CFT1:CFT1 + 8] = packc(1 - ftmk)
    cst[:, CBFR:CBFR + 8] = packc(bfr)
    cst[:, CEPS] = EPS
    cst[:, CBFK:CBFK + 32] = packc(bfk)

    def lerp_pair(W1, tm):
        # [128, 2(ab), 4(k2), 2(j), 1024] flat; a=0: W*tm, a=1: W*(1-tm)
        Wa = _pair_w(_q8s((W1 * tm[None, :]).T), C)
        Wb = _pair_w(_q8s((W1 * (1 - tm)[None, :]).T), C)
        return np.ascontiguousarray(
            np.stack([Wa.reshape(128, 4, 2, 1024),
                      Wb.reshape(128, 4, 2, 1024)], axis=1).reshape(128, -1))

    shared = {
        "wk": lerp_pair(Wk1, tmk),
        "wv": lerp_pair(Wv1, tmv),
        "wr": lerp_pair(Wr1, tmr),
        "wo": _fwv_hl(Wo.T),
        "fwr": _fwv_hl(fWr1.T),
        "fwk": _fwk_hl(fWk1.T),
        "fwv": _fwv_hl(fWv.T),
        "cst": cst,
        "ones1": np.ones((128, 1), f32),
        "onesb": np.ones((1, 128), ml_dtypes.bfloat16),
        "ident": np.eye(128, dtype=f32),
    }
    in_maps = [dict(shared, x=np.ascontiguousarray(x[b])) for b in range(B)]
    return in_maps


def _run(in_maps, trace=False, **kw):
    nc = _get_prog()
    res = run_bass_kernel_spmd(nc, in_maps, core_ids=list(range(B)), trace=trace,
                               **kw)
    out = np.stack([np.asarray(res.results[b]["out"]) for b in range(B)], axis=0)
    return out.astype(np.float32), res


def kernel(*a, **kw):
    out, _ = _run(_prep_inputs(*a, **kw))
    return out


if __name__ == "__main__":
    _get_prog()
    print("program built ok")


# revision 6
# speedup vs baseline: 1.0501x; 1.0007x over previous
"""RWKV-4 block (nn_Block_5669356833485) Trainium2 Bass kernel.

B=8, T=2048, C=1024, HID=4096. B-sharded across 8 NeuronCores (1 batch/core).
Feature-major layout [C-partitions, T-free].

fp8e4 DoubleRow matmuls (256-wide contraction, 0.5 cyc/row). Weights are
host-scaled by 128 before fp8 quantization (their natural ~0.02 magnitude
falls in e4m3's subnormal range) and unscaled in the matmul epilogues.
Time-mix lerps are folded into the matmuls by doubling the contraction
against z and shifted-z (z pair tiles [128, 2, 2064], data offset 16, pair
stride %16==0 per DoubleRow requirements). LN stats are pipelined into the
producing loops; WKV (bf16 scans, fp32 state) interleaves per channel block
with the projections. ek/v/sr/srf stay in SBUF; only xT and x2 round-trip
through DRAM for the residual adds.
Self-contained: hardcodes shapes; no sibling imports.
"""
import os
import sys
sys.path.insert(0, '/opt/trn_rl_repo')

KPHASES = int(os.environ.get("KPHASES", "99"))

import numpy as np
import ml_dtypes

import concourse.bass as bass
from concourse import bacc
import concourse.mybir as mybir
import concourse.tile as tile
from concourse.bass_utils import run_bass_kernel_spmd

F32 = mybir.dt.float32
F32R = mybir.dt.float32r
BF16 = mybir.dt.bfloat16
FP8 = mybir.dt.float8e4
AL = mybir.AluOpType
AF = mybir.ActivationFunctionType
DR = mybir.MatmulPerfMode.DoubleRow

B, T, C, HID = 8, 2048, 1024, 4096
NCB = C // 128          # 8 channel blocks
NPR = NCB // 2          # 4 channel pair-blocks
NHB = HID // 128        # 32 hidden blocks
NHP = NHB // 2          # 16 hidden pair-blocks
NT = T // 512           # 4 n-slices of 512
NTB = T // 128          # 16 token blocks
PAD = 16                # z pair tiles: [128, 2, PAD+T]; pair stride %16==0
TP = T + PAD
EPS = 1e-5
WS = 128.0              # weight pre-quantization scale
INV = 1.0 / WS

# cst columns (per 128-partition, indexed by block)
CW = 0        # wbar          [0:8)   by cb
CEU = 8       # exp(tf)       [8:16)  by cb
CBK = 16      # bk            [16:24) by m
CBV = 24      # bv            [24:32) by m
CBR = 32      # br            [32:40) by m
CFT = 40      # ftmk          [40:48) by cb
CFT1 = 48     # 1-ftmk        [48:56) by cb
CBFR = 56     # bfr           [56:64) by m
CEPS = 64     # eps           col 64
CBFK = 72     # bfk           [72:104) by hb
NCOLS = 104


def _emit(nc):
    # ---------------- DRAM I/O ----------------
    x_d = nc.declare_dram_parameter("x", [T, C], F32, isOutput=False)
    wk_d = nc.declare_dram_parameter("wk", [128, 2 * 4 * 2 * 1024], FP8, isOutput=False)
    wv_d = nc.declare_dram_parameter("wv", [128, 2 * 4 * 2 * 1024], FP8, isOutput=False)
    wr_d = nc.declare_dram_parameter("wr", [128, 2 * 4 * 2 * 1024], FP8, isOutput=False)
    wo_d = nc.declare_dram_parameter("wo", [2, 128, 4 * 2 * 1024], FP8, isOutput=False)
    fwr_d = nc.declare_dram_parameter("fwr", [2, 128, 4 * 2 * 1024], FP8, isOutput=False)
    fwk_d = nc.declare_dram_parameter("fwk", [2, 8, 128, 4 * 2 * 512], FP8, isOutput=False)
    fwv_d = nc.declare_dram_parameter("fwv", [2, 128, 16 * 2 * 1024], FP8, isOutput=False)
    cst_d = nc.declare_dram_parameter("cst", [128, NCOLS], F32, isOutput=False)
    ones1_d = nc.declare_dram_parameter("ones1", [128, 1], F32R, isOutput=False)
    onesb_d = nc.declare_dram_parameter("onesb", [1, 128], BF16, isOutput=False)
    ident_d = nc.declare_dram_parameter("ident", [128, 128], F32, isOutput=False)
    out_d = nc.declare_dram_parameter("out", [T, C], F32, isOutput=True)

    # DRAM scratch (per-cb granularity for fine deps)
    xT_sp = [nc.dram_tensor(f"xT_sp{i}", [128, T], F32) for i in range(NCB)]
    x2_sp = [nc.dram_tensor(f"x2_sp{i}", [128, T], F32) for i in range(NCB)]

    import contextlib

    with tile.TileContext(nc, pool_alloc_mode="queue") as tc:
        with tc.tile_pool(name="pc", bufs=1) as pc:
            cst = pc.tile([128, NCOLS], F32)
            nc.sync.dma_start(out=cst, in_=cst_d[:])
            ones1 = pc.tile([128, 1], F32R)
            nc.sync.dma_start(out=ones1, in_=ones1_d[:])
            onesb = pc.tile([1, 128], BF16)
            nc.sync.dma_start(out=onesb, in_=onesb_d[:])
            ident = pc.tile([128, 128], F32)
            nc.sync.dma_start(out=ident, in_=ident_d[:])
            ones_bf = pc.tile([128, T], BF16)
            nc.vector.memset(ones_bf, 1.0)

            def col(j):
                return cst[:, j:j + 1]

            # ---- incremental LN stats: two [1, T] psum tiles ----
            def ln_contrib(stat_ps, pool_tmp, src_f32r, cb, sl, tag):
                """Add channel-block cb's contribution for column slice sl."""
                mean_ps, msq_ps = stat_ps
                w = sl.stop - sl.start
                sq = pool_tmp.tile([128, w], F32R, tag=tag, bufs=3)
                nc.scalar.activation(sq, src_f32r.bitcast(F32)[:, sl], AF.Square)
                nc.tensor.matmul(mean_ps[:, sl], ones1, src_f32r[:, sl],
                                 start=(cb == 0), stop=(cb == NCB - 1))
                nc.tensor.matmul(msq_ps[:, sl], ones1, sq,
                                 start=(cb == 0), stop=(cb == NCB - 1))

            def ln_to_sbuf(stat_ps, pool_stat):
                mean_ps, msq_ps = stat_ps
                mean = pool_stat.tile([1, T], BF16, tag="mean_sb")
                msq = pool_stat.tile([1, T], BF16, tag="msq_sb")
                nc.scalar.mul(mean, mean_ps, 1.0 / C)
                nc.scalar.mul(msq, msq_ps, 1.0 / C)
                return mean, msq

            def ln_finish(mean, msq, pool_stat, uid):
                var = pool_stat.tile([1, T], BF16, tag="var_sb")
                nc.vector.tensor_mul(var, mean, mean)
                nc.vector.tensor_sub(var, msq, var)
                lnv = pool_stat.tile([1, T], BF16, tag="msq_sb", name=f"lnv{uid}")
                nc.scalar.activation(lnv, var, AF.Ln,
                                     bias=cst[0:1, CEPS:CEPS + 1], scale=1.0)
                rstd = pool_stat.tile([1, T], BF16, tag="var_sb", name=f"rstd{uid}")
                nc.scalar.activation(rstd, lnv, AF.Exp, bias=0.0, scale=-0.5)
                mrstd = pool_stat.tile([1, T], BF16, tag="mrstd_sb")
                nc.vector.tensor_mul(mrstd, mean, rstd)
                rstd_b = pool_stat.tile([128, T], BF16, tag="rstd_b")
                mrstd_b = pool_stat.tile([128, T], BF16, tag="mrstd_b")
                with tc.tile_pool(name=f"ps_bc{uid}", bufs=2, space="PSUM") as ps_bc:
                    for (src_s, dst) in ((rstd, rstd_b), (mrstd, mrstd_b)):
                        for n in range(NT):
                            sl = slice(n * 512, (n + 1) * 512)
                            bc = ps_bc.tile([128, 512], F32, tag="bc")
                            nc.tensor.matmul(bc, onesb, src_s[:, sl],
                                             start=True, stop=True)
                            if n % 2 == 0:
                                nc.scalar.copy(dst[:, sl], bc)
                            else:
                                nc.vector.tensor_copy(out=dst[:, sl], in_=bc)
                return rstd_b, mrstd_b

            # z pair tiles live through phase B (attention)
            es_z = contextlib.ExitStack()
            p_z = es_z.enter_context(tc.tile_pool(name="p_z", bufs=1, side="right"))
            z8 = [p_z.tile([128, 2, TP], FP8, tag=f"z{pr}", name=f"z{pr}")
                  for pr in range(NPR)]
            for pr in range(NPR):
                nc.vector.memset(z8[pr][:, :, 0:PAD], 0.0)

            # attention weights: prefetch during phase A
            es_wo = contextlib.ExitStack()
            p_wo = es_wo.enter_context(tc.tile_pool(name="p_wo", bufs=1))
            wo = [p_wo.tile([128, 4, 2, 1024], FP8, tag=f"wo{i}",
                            name=f"wo{i}") for i in range(2)]
            es_w = contextlib.ExitStack()
            p_w = es_w.enter_context(tc.tile_pool(name="p_w", bufs=1, side="right"))
            wk = p_w.tile([128, 2, 4, 2, 1024], FP8, tag="wk")
            wv = p_w.tile([128, 2, 4, 2, 1024], FP8, tag="wv")
            wr = p_w.tile([128, 2, 4, 2, 1024], FP8, tag="wr")

            # ================= PHASE A: load, transpose, LN1, z =================
            with tc.tile_pool(name="p_xT", bufs=1) as p_xT:
                xT = [p_xT.tile([128, T], F32R, tag=f"xT{cb}", name=f"xT{cb}")
                      for cb in range(NCB)]
                with tc.tile_pool(name="p_tmpA", bufs=1) as p_tmpA, \
                     tc.tile_pool(name="p_statA", bufs=1) as p_statA:
                    meanA = p_statA.tile([1, T], BF16, tag="mean_sb")
                    msqA = p_statA.tile([1, T], BF16, tag="msq_sb")
                    with tc.tile_pool(name="p_ld", bufs=3) as p_ld, \
                         tc.tile_pool(name="ps_stA", bufs=2,
                                      space="PSUM") as ps_stA, \
                         tc.tile_pool(name="ps_tr", bufs=4, space="PSUM") as ps_tr:
                        for tb in range(NTB):
                            xt = p_ld.tile([128, C], F32, tag="xtok")
                            nc.sync.dma_start(out=xt,
                                              in_=x_d[tb * 128:(tb + 1) * 128, :])
                            for cb in range(NCB):
                                pt = ps_tr.tile([128, 128], F32, tag="tr")
                                nc.tensor.transpose(
                                    pt, xt[:, cb * 128:(cb + 1) * 128], ident)
                                dst = xT[cb][:, tb * 128:(tb + 1) * 128]
                                if (tb + cb) % 2 == 0:
                                    nc.scalar.copy(dst, pt)
                                else:
                                    nc.vector.tensor_copy(out=dst, in_=pt)
                            if tb % 4 == 3:
                                n = tb // 4
                                sl = slice(n * 512, (n + 1) * 512)
                                mp = ps_stA.tile([1, 512], F32, tag="mA",
                                                 name=f"mA{n}")
                                qp = ps_stA.tile([1, 512], F32, tag="qA",
                                                 name=f"qA{n}")
                                for cb in range(NCB):
                                    sq = p_tmpA.tile([128, 512], F32R,
                                                     tag="sqA", bufs=3)
                                    nc.scalar.activation(
                                        sq, xT[cb].bitcast(F32)[:, sl],
                                        AF.Square)
                                    nc.tensor.matmul(
                                        mp, ones1, xT[cb][:, sl],
                                        start=(cb == 0), stop=(cb == NCB - 1))
                                    nc.tensor.matmul(
                                        qp, ones1, sq,
                                        start=(cb == 0), stop=(cb == NCB - 1))
                                nc.scalar.mul(meanA[:, sl], mp, 1.0 / C)
                                nc.scalar.mul(msqA[:, sl], qp, 1.0 / C)
                    # attention weight loads: small chunks so queue-jumps
                    # ahead of x-loads steal only ~0.7us DMA slots
                    for (wt_, wd_) in ((wk, wk_d), (wv, wv_d), (wr, wr_d)):
                        for a_ in range(2):
                            for k2_ in range(4):
                                nc.sync.dma_start(
                                    out=wt_[:, a_, k2_],
                                    in_=wd_[:, (a_ * 4 + k2_) * 2048:
                                            (a_ * 4 + k2_ + 1) * 2048])
                    for i_ in range(2):
                        for k2_ in range(4):
                            nc.sync.dma_start(
                                out=wo[i_][:, k2_],
                                in_=wo_d[i_, :, k2_ * 2048:(k2_ + 1) * 2048])
                    for cb in range(NCB):
                        nc.sync.dma_start(out=xT_sp[cb][:], in_=xT[cb].bitcast(F32))
                    rstd_b, mrstd_b = ln_finish(meanA, msqA, p_statA, "A")
                    for cb in range(NCB):
                        pr, j = cb // 2, cb % 2
                        zt = p_tmpA.tile([128, T], F32, tag="zt", bufs=2)
                        nc.vector.tensor_mul(zt, xT[cb].bitcast(F32), rstd_b)
                        dst = z8[pr][:, j, PAD:PAD + T]
                        if cb % 2 == 0:
                            nc.vector.tensor_sub(dst, zt, mrstd_b)
                        else:
                            nc.gpsimd.tensor_sub(dst, zt, mrstd_b)

            # ============ PHASE B: k/v/r projections + WKV per m ============
            es_sry = contextlib.ExitStack()
            p_sry = es_sry.enter_context(tc.tile_pool(name="p_sry", bufs=1))
            sryh = [p_sry.tile([128, 2, T], FP8, tag=f"sryh{pr}", name=f"sryh{pr}")
                    for pr in range(NPR)]
            sryl = [p_sry.tile([128, 2, T], FP8, tag=f"sryl{pr}", name=f"sryl{pr}")
                    for pr in range(NPR)]

            def zsl(k2, a, n):
                # a=0: current tokens; a=1: shifted by one
                lo = PAD - a + n * 512
                return z8[k2][:, :, lo:lo + 512]

            with tc.tile_pool(name="p_kvs", bufs=2) as p_kvs, \
                 tc.tile_pool(name="p_wt", bufs=2) as p_wt, \
                 tc.tile_pool(name="ps_mm", bufs=8, space="PSUM") as ps_mm:
                wkv_state = {}

                def wkv_front(m, ek, vv):
                    # scanB first: depends only on ek (k epilogues), so DVE can
                    # start while Act still runs v/r epilogues. ekv on DVE keeps
                    # the ekv->scanA handoff on-engine (no cross-engine sem).
                    wrow = p_wt.tile([128, T], BF16, tag="wrow", name=f"wr{m}")
                    nc.vector.tensor_scalar(out=wrow, in0=ones_bf,
                                            scalar1=col(CW + m),
                                            scalar2=None, op0=AL.mult)
                    A = p_wt.tile([128, T + 1], BF16, tag="A", name=f"A{m}")
                    Bt = p_wt.tile([128, T + 1], BF16, tag="B", name=f"B{m}")
                    nc.vector.memset(Bt[:, 0:1], 0.0)
                    nc.vector.tensor_tensor_scan(
                        out=Bt[:, 1:T + 1], data0=wrow, data1=ek,
                        initial=0.0, op0=AL.mult, op1=AL.add)
                    ekv = p_wt.tile([128, T], BF16, tag="ekv", name=f"ekv{m}")
                    nc.vector.tensor_mul(ekv, ek, vv)
                    nc.vector.memset(A[:, 0:1], 0.0)
                    nc.vector.tensor_tensor_scan(
                        out=A[:, 1:T + 1], data0=wrow, data1=ekv,
                        initial=0.0, op0=AL.mult, op1=AL.add)
                    nc.vector.scalar_tensor_tensor(
                        out=Bt[:, 0:T], in0=ek, scalar=col(CEU + m),
                        in1=Bt[:, 0:T], op0=AL.mult, op1=AL.add)
                    rec = p_wt.tile([128, T], BF16, tag="rec", name=f"rec{m}")
                    with nc.allow_low_precision(reason="wkv ratio bf16"):
                        nc.vector.reciprocal(rec, Bt[:, 0:T])
                    nc.vector.scalar_tensor_tensor(
                        out=A[:, 0:T], in0=ekv, scalar=col(CEU + m),
                        in1=A[:, 0:T], op0=AL.mult, op1=AL.add)
                    return A, rec

                def wkv_tail(m, A, rec, sr):
                    pr_m, j_m = m // 2, m % 2
                    y = p_wt.tile([128, T], BF16, tag="y", name=f"y{m}")
                    nc.gpsimd.tensor_mul(y, A[:, 0:T], rec)
                    sy = p_wt.tile([128, T], BF16, tag="sy", name=f"sy{m}")
                    nc.gpsimd.tensor_mul(sy, y, sr)
                    nc.scalar.copy(sryh[pr_m][:, j_m, :], sy)
                    nc.vector.tensor_sub(sryl[pr_m][:, j_m, :], sy,
                                         sryh[pr_m][:, j_m, :])

                def proj_one(wt, m, dst, act, bcol):
                    for n in range(NT):
                        pmm = ps_mm.tile([128, 512], F32, tag="pmm")
                        for a in range(2):
                            for k2 in range(NPR):
                                nc.tensor.matmul(
                                    pmm, wt[:, a, k2, :,
                                            m * 128:(m + 1) * 128],
                                    zsl(k2, a, n),
                                    start=(a == 0 and k2 == 0),
                                    stop=(a == 1 and k2 == NPR - 1),
                                    perf_mode=DR)
                        dsl = dst[:, n * 512:(n + 1) * 512]
                        nc.scalar.activation(dsl, pmm, act,
                                             bias=col(bcol + m), scale=INV)

                for m in (range(NCB) if KPHASES >= 2 else ()):
                    ek = p_kvs.tile([128, T], BF16, tag="ek", name=f"ek{m}")
                    vv = p_kvs.tile([128, T], BF16, tag="vv", name=f"vv{m}")
                    sr = p_kvs.tile([128, T], BF16, tag="sr", name=f"sr{m}")
                    proj_one(wk, m, ek, AF.Exp, CBK)
                    proj_one(wv, m, vv, AF.Identity, CBV)
                    if KPHASES >= 3:
                        # front chain starts as soon as k/v epilogues land;
                        # r-proj (only needed by tail, one block later) after
                        A, rec = wkv_front(m, ek, vv)
                    proj_one(wr, m, sr, AF.Sigmoid, CBR)
                    if KPHASES >= 3:
                        wkv_state[m] = (A, rec, sr)
                        if m >= 1:
                            wkv_tail(m - 1, *wkv_state.pop(m - 1))
                if KPHASES >= 3:
                    wkv_tail(NCB - 1, *wkv_state.pop(NCB - 1))

            es_w.close()
            es_z.close()

            # ===== PHASE C: out-proj + residual -> x2, fused LN2 stats =====
            es_x2 = contextlib.ExitStack()
            p_x2 = es_x2.enter_context(tc.tile_pool(name="p_x2", bufs=1))
            x2 = [p_x2.tile([128, T], F32R, tag=f"x2_{cb}", name=f"x2_{cb}")
                  for cb in range(NCB)]
            es_z2 = contextlib.ExitStack()
            p_z2 = es_z2.enter_context(tc.tile_pool(name="p_z2", bufs=1,
                                                    side="right"))
            z2t = [p_z2.tile([128, T + 1], BF16, tag=f"z2_{cb}", name=f"z2_{cb}")
                   for cb in range(NCB)]
            with tc.tile_pool(name="p_xr", bufs=2) as p_xr, \
                 tc.tile_pool(name="p_tmpD", bufs=1) as p_tmpD, \
                 tc.tile_pool(name="p_statD", bufs=1) as p_statD:
              with tc.tile_pool(name="ps_mo", bufs=4, space="PSUM") as ps_mo, \
                   tc.tile_pool(name="ps_st2", bufs=1, space="PSUM") as ps_st2:
                st_half = [(ps_st2.tile([1, 512], F32, tag=f"m{i}", name=f"mD{i}"),
                            ps_st2.tile([1, 512], F32, tag=f"q{i}", name=f"qD{i}"))
                           for i in range(2)]
                for m in (range(NCB) if KPHASES >= 4 else ()):
                    xr = p_xr.tile([128, T], F32, tag="xr")
                    nc.sync.dma_start(out=xr, in_=xT_sp[m][:])
                    for n in range(NT):
                        sl = slice(n * 512, (n + 1) * 512)
                        pmm = ps_mo.tile([128, 512], F32, tag="pmo")
                        first = True
                        for (wi, ss) in ((0, sryh), (1, sryh), (0, sryl)):
                            for k2 in range(NPR):
                                nc.tensor.matmul(
                                    pmm, wo[wi][:, k2, :, m * 128:(m + 1) * 128],
                                    ss[k2][:, :, sl],
                                    start=first,
                                    stop=(wi == 0 and ss is sryl
                                          and k2 == NPR - 1),
                                    perf_mode=DR)
                                first = False
                        nc.vector.scalar_tensor_tensor(
                            out=x2[m][:, sl], in0=pmm, scalar=INV,
                            in1=xr[:, sl], op0=AL.mult, op1=AL.add)
                    nc.sync.dma_start(out=x2_sp[m][:], in_=x2[m].bitcast(F32))
                    if KPHASES >= 5:
                        for i in range(2):
                            sl2 = slice(i * 512, (i + 1) * 512)
                            sq = p_tmpD.tile([128, 512], F32R, tag="sqD", bufs=3)
                            nc.scalar.activation(sq, x2[m].bitcast(F32)[:, sl2],
                                                 AF.Square)
                            nc.tensor.matmul(st_half[i][0], ones1, x2[m][:, sl2],
                                             start=(m == 0), stop=(m == NCB - 1))
                            nc.tensor.matmul(st_half[i][1], ones1, sq,
                                             start=(m == 0), stop=(m == NCB - 1))
                if KPHASES >= 5:
                    meanD = p_statD.tile([1, T], BF16, tag="mean_sb")
                    msqD = p_statD.tile([1, T], BF16, tag="msq_sb")
                    for i in range(2):
                        sl2 = slice(i * 512, (i + 1) * 512)
                        nc.scalar.mul(meanD[:, sl2], st_half[i][0], 1.0 / C)
                        nc.scalar.mul(msqD[:, sl2], st_half[i][1], 1.0 / C)
              # ---- LN2 stats + finish -> z2 (plain bf16, col 0 zero) ----
              if True:
                if KPHASES >= 5:
                    with tc.tile_pool(name="ps_stD", bufs=1,
                                      space="PSUM") as ps_stD:
                        st2 = [(ps_stD.tile([1, 512], F32, tag=f"m2{i}",
                                            name=f"mD2{i}"),
                                ps_stD.tile([1, 512], F32, tag=f"q2{i}",
                                            name=f"qD2{i}")) for i in range(2)]
                        for i in range(2):
                            n = 2 + i
                            sl2 = slice(n * 512, (n + 1) * 512)
                            for cb in range(NCB):
                                sq = p_tmpD.tile([128, 512], F32R, tag="sqD",
                                                 bufs=3)
                                nc.scalar.activation(
                                    sq, x2[cb].bitcast(F32)[:, sl2], AF.Square)
                                nc.tensor.matmul(st2[i][0], ones1,
                                                 x2[cb][:, sl2],
                                                 start=(cb == 0),
                                                 stop=(cb == NCB - 1))
                                nc.tensor.matmul(st2[i][1], ones1, sq,
                                                 start=(cb == 0),
                                                 stop=(cb == NCB - 1))
                            nc.scalar.mul(meanD[:, sl2], st2[i][0], 1.0 / C)
                            nc.scalar.mul(msqD[:, sl2], st2[i][1], 1.0 / C)
                    rstd_b2, mrstd_b2 = ln_finish(meanD, msqD, p_statD, "D")
                    for cb in range(NCB):
                        nc.vector.memset(z2t[cb][:, 0:1], 0.0)
                        zt = p_tmpD.tile([128, T], F32, tag="zt2", bufs=1)
                        nc.vector.tensor_mul(zt, x2[cb].bitcast(F32), rstd_b2)
                        dst = z2t[cb][:, 1:T + 1]
                        if cb % 2 == 0:
                            nc.vector.tensor_sub(dst, zt, mrstd_b2)
                        else:
                            nc.gpsimd.tensor_sub(dst, zt, mrstd_b2)
            es_x2.close()
            es_sry.close()
            es_wo.close()

            # FFN weights: fwv hi/lo resident fp8; fwr till srf; fwk streamed
            es_fw = contextlib.ExitStack()
            p_fw = es_fw.enter_context(tc.tile_pool(name="p_fw", bufs=1))
            fwv = [p_fw.tile([128, 16, 2, 1024], FP8, tag=f"fwv{i}",
                             name=f"fwv{i}") for i in range(2)]
            if KPHASES >= 5:
                nc.sync.dma_start(out=fwv[0], in_=fwv_d[0])
                nc.sync.dma_start(out=fwv[1], in_=fwv_d[1])

            # ============ PHASE E: xf lerp (f_tmk == f_tmr), fWr -> srf ========
            es_xf = contextlib.ExitStack()
            p_xf = es_xf.enter_context(tc.tile_pool(name="p_xf", bufs=1))
            xfh = [p_xf.tile([128, 2, T], FP8, tag=f"xfh{pr}", name=f"xfh{pr}")
                   for pr in range(NPR)]
            xfl = [p_xf.tile([128, 2, T], FP8, tag=f"xfl{pr}", name=f"xfl{pr}")
                   for pr in range(NPR)]
            with tc.tile_pool(name="p_te", bufs=3) as p_te:
                for cb in (range(NCB) if KPHASES >= 6 else ()):
                    pr, j = cb // 2, cb % 2
                    t1 = p_te.tile([128, T], BF16, tag="t1")
                    nc.scalar.mul(t1, z2t[cb][:, 0:T], col(CFT1 + cb))
                    xfb = p_te.tile([128, T], BF16, tag="xfb")
                    nc.vector.scalar_tensor_tensor(
                        out=xfb, in0=z2t[cb][:, 1:T + 1],
                        scalar=col(CFT + cb), in1=t1, op0=AL.mult, op1=AL.add)
                    nc.scalar.copy(xfh[pr][:, j, :], xfb)
                    nc.gpsimd.tensor_sub(xfl[pr][:, j, :], xfb, xfh[pr][:, j, :])
            es_z2.close()

            es_srf = contextlib.ExitStack()
            p_srf = es_srf.enter_context(tc.tile_pool(name="p_srf", bufs=1))
            srf = [p_srf.tile([128, T], FP8, tag=f"srf{m}", name=f"srf{m}")
                   for m in range(NCB)]
            with tc.tile_pool(name="p_fwr", bufs=1) as p_fwr, \
                 tc.tile_pool(name="ps_fr", bufs=4, space="PSUM") as ps_fr:
                fwr = [p_fwr.tile([128, 4, 2, 1024], FP8, tag=f"fwr{i}",
                                  name=f"fwr{i}") for i in range(2)]
                if KPHASES >= 6:
                    nc.sync.dma_start(out=fwr[0], in_=fwr_d[0])
                    nc.sync.dma_start(out=fwr[1], in_=fwr_d[1])
                for m in (range(NCB) if KPHASES >= 6 else ()):
                    for n in range(NT):
                        pmm = ps_fr.tile([128, 512], F32, tag="pfr")
                        first = True
                        for (wi, xs) in ((0, xfh), (1, xfh), (0, xfl)):
                            for k2 in range(NPR):
                                nc.tensor.matmul(
                                    pmm, fwr[wi][:, k2, :, m * 128:(m + 1) * 128],
                                    xs[k2][:, :, n * 512:(n + 1) * 512],
                                    start=first,
                                    stop=(wi == 0 and xs is xfl and k2 == NPR - 1),
                                    perf_mode=DR)
                                first = False
                        nc.scalar.activation(srf[m][:, n * 512:(n + 1) * 512],
                                             pmm, AF.Sigmoid, bias=col(CBFR + m),
                                             scale=INV)

            # ============ PHASE F: FFN k/v matmuls + output ============
            # 3-pass residual fp8: W*x ~ Wh*xh + Wl*xh + Wh*xl
            with tc.tile_pool(name="p_fwkg", bufs=2) as p_fwkg, \
                 tc.tile_pool(name="p_kk", bufs=1) as p_kk, \
                 tc.tile_pool(name="p_rl", bufs=4) as p_rl, \
                 tc.tile_pool(name="p_x2c", bufs=3) as p_x2c, \
                 tc.tile_pool(name="p_fin", bufs=2) as p_fin, \
                 tc.tile_pool(name="p_ost", bufs=1) as p_ost, \
                 tc.tile_pool(name="ps_fk", bufs=3, space="PSUM") as ps_fk, \
                 tc.tile_pool(name="ps_fo", bufs=2, space="PSUM") as ps_fo, \
                 tc.tile_pool(name="ps_ot", bufs=3, space="PSUM") as ps_ot:
                for n in (range(NT) if KPHASES >= 7 else ()):
                    sl = slice(n * 512, (n + 1) * 512)
                    kkh = [p_kk.tile([128, 2, 512], FP8, tag=f"kkh{hp}",
                                     name=f"kkh{hp}_{n}") for hp in range(NHP)]
                    kkl = [p_kk.tile([128, 2, 512], FP8, tag=f"kkl{hp}",
                                     name=f"kkl{hp}_{n}") for hp in range(NHP)]
                    for g in range(8):
                        fg = [p_fwkg.tile([128, 4, 2, 512], FP8, tag=f"fwkg{i}",
                                          name=f"fwkg{i}_{n}_{g}")
                              for i in range(2)]
                        nc.sync.dma_start(out=fg[0], in_=fwk_d[0, g])
                        nc.sync.dma_start(out=fg[1], in_=fwk_d[1, g])
                        for i in range(4):
                            hb = g * 4 + i
                            hp, jh = hb // 2, hb % 2
                            pkk = ps_fk.tile([128, 512], F32, tag="pkk")
                            first = True
                            for (wi, xs) in ((0, xfh), (1, xfh), (0, xfl)):
                                for k2 in range(NPR):
                                    nc.tensor.matmul(
                                        pkk,
                                        fg[wi][:, k2, :, i * 128:(i + 1) * 128],
                                        xs[k2][:, :, sl],
                                        start=first,
                                        stop=(wi == 0 and xs is xfl
                                              and k2 == NPR - 1),
                                        perf_mode=DR)
                                    first = False
                            rl = p_rl.tile([128, 512], BF16, tag="rl")
                            if hb % 2 == 0:
                                nc.scalar.activation(rl, pkk, AF.Relu,
                                                     bias=col(CBFK + hb),
                                                     scale=INV)
                            else:
                                nc.vector.tensor_scalar(
                                    out=rl, in0=pkk, scalar1=INV,
                                    scalar2=0.0, op0=AL.mult, op1=AL.max)
                            t2 = p_rl.tile([128, 512], BF16, tag="t2")
                            nc.vector.tensor_mul(t2, rl, rl)
                            dh = kkh[hp][:, jh, :]
                            if hb % 2 == 0:
                                nc.scalar.copy(dh, t2)
                            else:
                                nc.vector.tensor_copy(out=dh, in_=t2)
                            nc.gpsimd.tensor_sub(kkl[hp][:, jh, :], t2, dh)
                    osts = [p_ost.tile([128, C], F32, tag=f"ost{j}",
                                       name=f"ost{n}_{j}") for j in range(4)]
                    for m in range(NCB):
                        po = ps_fo.tile([128, 512], F32, tag="po")
                        first = True
                        for (wi, ks) in ((0, kkh), (1, kkh), (0, kkl)):
                            for hp in range(NHP):
                                nc.tensor.matmul(
                                    po, fwv[wi][:, hp, :, m * 128:(m + 1) * 128],
                                    ks[hp],
                                    start=first,
                                    stop=(wi == 0 and ks is kkl
                                          and hp == NHP - 1),
                                    perf_mode=DR)
                                first = False
                        x2c = p_x2c.tile([128, 512], F32, tag="x2c")
                        nc.sync.dma_start(out=x2c, in_=x2_sp[m][:, sl])
                        rkv = p_fin.tile([128, 512], F32, tag="rkv")
                        nc.vector.scalar_tensor_tensor(
                            out=rkv, in0=po, scalar=INV, in1=srf[m][:, sl],
                            op0=AL.mult, op1=AL.mult)
                        fin = p_fin.tile([128, 512], F32, tag="fin")
                        if m % 2 == 0:
                            nc.gpsimd.tensor_add(fin, rkv, x2c)
                        else:
                            nc.vector.tensor_add(fin, rkv, x2c)
                        for j in range(4):
                            pt = ps_ot.tile([128, 128], F32, tag="ptr")
                            nc.tensor.transpose(pt, fin[:, j * 128:(j + 1) * 128],
                                                ident)
                            dst = osts[j][:, m * 128:(m + 1) * 128]
                            if (m + j) % 2 == 0:
                                nc.scalar.copy(dst, pt)
                            else:
                                nc.vector.tensor_copy(out=dst, in_=pt)
                    for j in range(4):
                        tb = n * 4 + j
                        nc.sync.dma_start(out=out_d[tb * 128:(tb + 1) * 128, :],
                                          in_=osts[j])
            es_srf.close()
            es_xf.close()
            es_fw.close()
    nc.finalize()
    return nc


_PROG = None


def _get_prog():
    global _PROG
    if _PROG is None:
        nc = bacc.Bacc()
        _PROG = _emit(nc)
    return _PROG


def _pair_w(WT, M_out):
    """WT: [K_in, M_out] fp8 (lhsT layout) -> [128, K_in//256, 2, M_out] flat."""
    K_in = WT.shape[0]
    npr = K_in // 256
    return np.ascontiguousarray(
        WT.reshape(npr, 2, 128, M_out).transpose(2, 0, 1, 3).reshape(128, -1))


def _q8_hl(WT):
    """WT f32 (pre-scaled by WS) -> (hi, lo) fp8 arrays."""
    f8 = ml_dtypes.float8_e4m3
    Ws = np.asarray(WT, np.float32) * np.float32(WS)
    assert np.abs(Ws).max() < 230.0
    hi = Ws.astype(f8)
    lo = (Ws - hi.astype(np.float32)).astype(f8)
    return hi, lo


def _fwk_hl(WT):
    """WT: [C, HID] -> fp8 [2(hl), 8(g), 128, 4(k2)*2(j)*512]; g = hid cols 512g."""
    hi, lo = _q8_hl(WT)
    out = []
    for W8 in (hi, lo):
        # pair layout per group: [128, k2, j, 512]
        Wp = W8.reshape(4, 2, 128, HID)  # [k2, j, c128, h]
        out.append(np.stack(
            [np.ascontiguousarray(
                Wp[:, :, :, g * 512:(g + 1) * 512].transpose(2, 0, 1, 3)
                .reshape(128, -1)) for g in range(8)]))
    return np.ascontiguousarray(np.stack(out))


def _fwv_hl(WT):
    """WT: [HID, C] -> fp8 [2(hl), 128, 16*2*1024] pair layout."""
    hi, lo = _q8_hl(WT)
    return np.ascontiguousarray(np.stack([_pair_w(W8, C) for W8 in (hi, lo)]))


def _q8s(W):
    """Scale by WS, quantize to fp8e4 (checks range)."""
    f8 = ml_dtypes.float8_e4m3
    Ws = np.asarray(W, np.float32) * np.float32(WS)
    assert np.abs(Ws).max() < 230.0, "weight scale overflow"
    return Ws.astype(f8)


def _prep_inputs(x, ln1_g, ln1_b, ln2_g, ln2_b, time_decay, time_first,
                 tmk, tmv, tmr, Wk, Wv, Wr, Wo, f_tmk, f_tmr, fWk, fWr, fWv):
    f32 = np.float32
    x = np.asarray(x, f32)
    g1 = np.asarray(ln1_g, f32); b1 = np.asarray(ln1_b, f32)
    g2 = np.asarray(ln2_g, f32); b2 = np.asarray(ln2_b, f32)
    td = np.asarray(time_decay, np.float64); tf = np.asarray(time_first, np.float64)
    tmk = np.asarray(tmk, f32).reshape(C); tmv = np.asarray(tmv, f32).reshape(C)
    tmr = np.asarray(tmr, f32).reshape(C)
    ftmk = np.asarray(f_tmk, f32).reshape(C); ftmr = np.asarray(f_tmr, f32).reshape(C)
    assert np.array_equal(ftmk, ftmr), "kernel assumes f_tmk == f_tmr"
    Wk = np.asarray(Wk, f32); Wv = np.asarray(Wv, f32); Wr = np.asarray(Wr, f32)
    Wo = np.asarray(Wo, f32); fWk = np.asarray(fWk, f32); fWr = np.asarray(fWr, f32)
    fWv = np.asarray(fWv, f32)

    Wk1 = Wk * g1[None, :]; Wv1 = Wv * g1[None, :]; Wr1 = Wr * g1[None, :]
    bk = Wk @ b1; bv = Wv @ b1; br = Wr @ b1
    fWk1 = fWk * g2[None, :]; fWr1 = fWr * g2[None, :]
    bfk = fWk @ b2; bfr = fWr @ b2
    assert np.allclose(bfk, 0.0), "kernel assumes zero ln2 beta for relu path"

    wbar = np.exp(-np.exp(td)).astype(f32)
    eu = np.exp(tf).astype(f32)

    def packc(v):
        return np.asarray(v, f32).reshape(-1, 128).T

    cst = np.zeros((128, NCOLS), f32)
    cst[:, CW:CW + 8] = packc(wbar)
    cst[:, CEU:CEU + 8] = packc(eu)
    cst[:, CBK:CBK + 8] = packc(bk)
    cst[:, CBV:CBV + 8] = packc(bv)
    cst[:, CBR:CBR + 8] = packc(br)
    cst[:, CFT:CFT + 8] = packc(ftmk)
    cst[:, CFT1:CFT1 + 8] = packc(1 - ftmk)
    cst[:, CBFR:CBFR + 8] = packc(bfr)
    cst[:, CEPS] = EPS
    cst[:, CBFK:CBFK + 32] = packc(bfk)

    def lerp_pair(W1, tm):
        # [128, 2(ab), 4(k2), 2(j), 1024] flat; a=0: W*tm, a=1: W*(1-tm)
        Wa = _pair_w(_q8s((W1 * tm[None, :]).T), C)
        Wb = _pair_w(_q8s((W1 * (1 - tm)[None, :]).T), C)
        return np.ascontiguousarray(
            np.stack([Wa.reshape(128, 4, 2, 1024),
                      Wb.reshape(128, 4, 2, 1024)], axis=1).reshape(128, -1))

    shared = {
        "wk": lerp_pair(Wk1, tmk),
        "wv": lerp_pair(Wv1, tmv),
        "wr": lerp_pair(Wr1, tmr),
        "wo": _fwv_hl(Wo.T),
        "fwr": _fwv_hl(fWr1.T),
        "fwk": _fwk_hl(fWk1.T),
        "fwv": _fwv_hl(fWv.T),
        "cst": cst,
        "ones1": np.ones((128, 1), f32),
        "onesb": np.ones((1, 128), ml_dtypes.bfloat16),
        "ident": np.eye(128, dtype=f32),
    }
    in_maps = [dict(shared, x=np.ascontiguousarray(x[b])) for b in range(B)]
    return in_maps


def _run(in_maps, trace=False, **kw):
    nc = _get_prog()
    res = run_bass_kernel_spmd(nc, in_maps, core_ids=list(range(B)), trace=trace,
                               **kw)
    out = np.stack([np.asarray(res.results[b]["out"]) for b in range(B)], axis=0)
    return out.astype(np.float32), res


def kernel(*a, **kw):
    out, _ = _run(_prep_inputs(*a, **kw))
    return out


if __name__ == "__main__":
    _get_prog()
    print("program built ok")


# revision 7
# speedup vs baseline: 1.0813x; 1.0298x over previous
"""RWKV-4 block (nn_Block_5669356833485) Trainium2 Bass kernel.

B=8, T=2048, C=1024, HID=4096. B-sharded across 8 NeuronCores (1 batch/core).
Feature-major layout [C-partitions, T-free].

fp8e4 DoubleRow matmuls (256-wide contraction, 0.5 cyc/row). Weights are
host-scaled by 128 before fp8 quantization (their natural ~0.02 magnitude
falls in e4m3's subnormal range) and unscaled in the matmul epilogues.
Time-mix lerps are folded into the matmuls by doubling the contraction
against z and shifted-z (z pair tiles [128, 2, 2064], data offset 16, pair
stride %16==0 per DoubleRow requirements). LN stats are pipelined into the
producing loops; WKV (bf16 scans, fp32 state) interleaves per channel block
with the projections. ek/v/sr/srf stay in SBUF; only xT and x2 round-trip
through DRAM for the residual adds.
Self-contained: hardcodes shapes; no sibling imports.
"""
import os
import sys
sys.path.insert(0, '/opt/trn_rl_repo')

KPHASES = int(os.environ.get("KPHASES", "99"))

import numpy as np
import ml_dtypes

import concourse.bass as bass
from concourse import bacc
import concourse.mybir as mybir
import concourse.tile as tile
from concourse.bass_utils import run_bass_kernel_spmd

F32 = mybir.dt.float32
F32R = mybir.dt.float32r
BF16 = mybir.dt.bfloat16
FP8 = mybir.dt.float8e4
AL = mybir.AluOpType
AF = mybir.ActivationFunctionType
DR = mybir.MatmulPerfMode.DoubleRow

B, T, C, HID = 8, 2048, 1024, 4096
NCB = C // 128          # 8 channel blocks
NPR = NCB // 2          # 4 channel pair-blocks
NHB = HID // 128        # 32 hidden blocks
NHP = NHB // 2          # 16 hidden pair-blocks
NT = T // 512           # 4 n-slices of 512
NTB = T // 128          # 16 token blocks
PAD = 16                # z pair tiles: [128, 2, PAD+T]; pair stride %16==0
TP = T + PAD
EPS = 1e-5
WS = 128.0              # weight pre-quantization scale
INV = 1.0 / WS

# cst columns (per 128-partition, indexed by block)
CW = 0        # wbar          [0:8)   by cb
CEU = 8       # exp(tf)       [8:16)  by cb
CBK = 16      # bk            [16:24) by m
CBV = 24      # bv            [24:32) by m
CBR = 32      # br            [32:40) by m
CFT = 40      # ftmk          [40:48) by cb
CFT1 = 48     # 1-ftmk        [48:56) by cb
CBFR = 56     # bfr           [56:64) by m
CEPS = 64     # eps           col 64
CBFK = 72     # bfk           [72:104) by hb
NCOLS = 104


def _emit(nc):
    # ---------------- DRAM I/O ----------------
    x_d = nc.declare_dram_parameter("x", [T, C], F32, isOutput=False)
    wk_d = nc.declare_dram_parameter("wk", [128, 2 * 4 * 2 * 1024], FP8, isOutput=False)
    wv_d = nc.declare_dram_parameter("wv", [128, 2 * 4 * 2 * 1024], FP8, isOutput=False)
    wr_d = nc.declare_dram_parameter("wr", [128, 2 * 4 * 2 * 1024], FP8, isOutput=False)
    wo_d = nc.declare_dram_parameter("wo", [2, 128, 4 * 2 * 1024], FP8, isOutput=False)
    fwr_d = nc.declare_dram_parameter("fwr", [2, 128, 4 * 2 * 1024], FP8, isOutput=False)
    fwk_d = nc.declare_dram_parameter("fwk", [2, 8, 128, 4 * 2 * 512], FP8, isOutput=False)
    fwv_d = nc.declare_dram_parameter("fwv", [2, 128, 16 * 2 * 1024], FP8, isOutput=False)
    cst_d = nc.declare_dram_parameter("cst", [128, NCOLS], F32, isOutput=False)
    ones1_d = nc.declare_dram_parameter("ones1", [128, 1], F32R, isOutput=False)
    onesb_d = nc.declare_dram_parameter("onesb", [1, 128], BF16, isOutput=False)
    ident_d = nc.declare_dram_parameter("ident", [128, 128], F32, isOutput=False)
    out_d = nc.declare_dram_parameter("out", [T, C], F32, isOutput=True)

    # DRAM scratch (per-cb granularity for fine deps)
    xT_sp = [nc.dram_tensor(f"xT_sp{i}", [128, T], F32) for i in range(NCB)]
    x2_sp = [nc.dram_tensor(f"x2_sp{i}", [128, T], F32) for i in range(NCB)]

    import contextlib

    with tile.TileContext(nc, pool_alloc_mode="queue") as tc:
        with tc.tile_pool(name="pc", bufs=1) as pc:
            cst = pc.tile([128, NCOLS], F32)
            nc.sync.dma_start(out=cst, in_=cst_d[:])
            ones1 = pc.tile([128, 1], F32R)
            nc.sync.dma_start(out=ones1, in_=ones1_d[:])
            onesb = pc.tile([1, 128], BF16)
            nc.sync.dma_start(out=onesb, in_=onesb_d[:])
            ident = pc.tile([128, 128], F32)
            nc.sync.dma_start(out=ident, in_=ident_d[:])
            ones_bf = pc.tile([128, T], BF16)
            nc.vector.memset(ones_bf, 1.0)

            def col(j):
                return cst[:, j:j + 1]

            # ---- incremental LN stats: two [1, T] psum tiles ----
            def ln_contrib(stat_ps, pool_tmp, src_f32r, cb, sl, tag):
                """Add channel-block cb's contribution for column slice sl."""
                mean_ps, msq_ps = stat_ps
                w = sl.stop - sl.start
                sq = pool_tmp.tile([128, w], F32R, tag=tag, bufs=3)
                nc.scalar.activation(sq, src_f32r.bitcast(F32)[:, sl], AF.Square)
                nc.tensor.matmul(mean_ps[:, sl], ones1, src_f32r[:, sl],
                                 start=(cb == 0), stop=(cb == NCB - 1))
                nc.tensor.matmul(msq_ps[:, sl], ones1, sq,
                                 start=(cb == 0), stop=(cb == NCB - 1))

            def ln_to_sbuf(stat_ps, pool_stat):
                mean_ps, msq_ps = stat_ps
                mean = pool_stat.tile([1, T], BF16, tag="mean_sb")
                msq = pool_stat.tile([1, T], BF16, tag="msq_sb")
                nc.scalar.mul(mean, mean_ps, 1.0 / C)
                nc.scalar.mul(msq, msq_ps, 1.0 / C)
                return mean, msq

            def ln_finish(mean, msq, pool_stat, uid):
                var = pool_stat.tile([1, T], BF16, tag="var_sb")
                nc.vector.tensor_mul(var, mean, mean)
                nc.vector.tensor_sub(var, msq, var)
                lnv = pool_stat.tile([1, T], BF16, tag="msq_sb", name=f"lnv{uid}")
                nc.scalar.activation(lnv, var, AF.Ln,
                                     bias=cst[0:1, CEPS:CEPS + 1], scale=1.0)
                rstd = pool_stat.tile([1, T], BF16, tag="var_sb", name=f"rstd{uid}")
                nc.scalar.activation(rstd, lnv, AF.Exp, bias=0.0, scale=-0.5)
                mrstd = pool_stat.tile([1, T], BF16, tag="mrstd_sb")
                nc.vector.tensor_mul(mrstd, mean, rstd)
                rstd_b = pool_stat.tile([128, T], BF16, tag="rstd_b")
                mrstd_b = pool_stat.tile([128, T], BF16, tag="mrstd_b")
                with tc.tile_pool(name=f"ps_bc{uid}", bufs=2, space="PSUM") as ps_bc:
                    for (src_s, dst) in ((rstd, rstd_b), (mrstd, mrstd_b)):
                        for n in range(NT):
                            sl = slice(n * 512, (n + 1) * 512)
                            bc = ps_bc.tile([128, 512], F32, tag="bc")
                            nc.tensor.matmul(bc, onesb, src_s[:, sl],
                                             start=True, stop=True)
                            if n % 2 == 0:
                                nc.scalar.copy(dst[:, sl], bc)
                            else:
                                nc.vector.tensor_copy(out=dst[:, sl], in_=bc)
                return rstd_b, mrstd_b

            # z pair tiles live through phase B (attention)
            es_z = contextlib.ExitStack()
            p_z = es_z.enter_context(tc.tile_pool(name="p_z", bufs=1, side="right"))
            z8 = [p_z.tile([128, 2, TP], FP8, tag=f"z{pr}", name=f"z{pr}")
                  for pr in range(NPR)]
            for pr in range(NPR):
                nc.vector.memset(z8[pr][:, :, 0:PAD], 0.0)

            # attention weights: prefetch during phase A
            es_wo = contextlib.ExitStack()
            p_wo = es_wo.enter_context(tc.tile_pool(name="p_wo", bufs=1))
            wo = [p_wo.tile([128, 4, 2, 1024], FP8, tag=f"wo{i}",
                            name=f"wo{i}") for i in range(2)]
            es_w = contextlib.ExitStack()
            p_w = es_w.enter_context(tc.tile_pool(name="p_w", bufs=1, side="right"))
            wk = p_w.tile([128, 2, 4, 2, 1024], FP8, tag="wk")
            wv = p_w.tile([128, 2, 4, 2, 1024], FP8, tag="wv")
            wr = p_w.tile([128, 2, 4, 2, 1024], FP8, tag="wr")

            # ================= PHASE A: load, transpose, LN1, z =================
            with tc.tile_pool(name="p_xT", bufs=1) as p_xT:
                xT = [p_xT.tile([128, T], F32R, tag=f"xT{cb}", name=f"xT{cb}")
                      for cb in range(NCB)]
                with tc.tile_pool(name="p_tmpA", bufs=1) as p_tmpA, \
                     tc.tile_pool(name="p_statA", bufs=1) as p_statA:
                    meanA = p_statA.tile([1, T], BF16, tag="mean_sb")
                    msqA = p_statA.tile([1, T], BF16, tag="msq_sb")
                    with tc.tile_pool(name="p_ld", bufs=3) as p_ld, \
                         tc.tile_pool(name="ps_stA", bufs=2,
                                      space="PSUM") as ps_stA, \
                         tc.tile_pool(name="ps_tr", bufs=4, space="PSUM") as ps_tr:
                        for tb in range(NTB):
                            xt = p_ld.tile([128, C], F32, tag="xtok")
                            nc.sync.dma_start(out=xt,
                                              in_=x_d[tb * 128:(tb + 1) * 128, :])
                            for cb in range(NCB):
                                pt = ps_tr.tile([128, 128], F32, tag="tr")
                                nc.tensor.transpose(
                                    pt, xt[:, cb * 128:(cb + 1) * 128], ident)
                                dst = xT[cb][:, tb * 128:(tb + 1) * 128]
                                if (tb + cb) % 2 == 0:
                                    nc.scalar.copy(dst, pt)
                                else:
                                    nc.vector.tensor_copy(out=dst, in_=pt)
                            if tb % 4 == 3:
                                n = tb // 4
                                sl = slice(n * 512, (n + 1) * 512)
                                mp = ps_stA.tile([1, 512], F32, tag="mA",
                                                 name=f"mA{n}")
                                qp = ps_stA.tile([1, 512], F32, tag="qA",
                                                 name=f"qA{n}")
                                for cb in range(NCB):
                                    sq = p_tmpA.tile([128, 512], F32R,
                                                     tag="sqA", bufs=3)
                                    nc.scalar.activation(
                                        sq, xT[cb].bitcast(F32)[:, sl],
                                        AF.Square)
                                    nc.tensor.matmul(
                                        mp, ones1, xT[cb][:, sl],
                                        start=(cb == 0), stop=(cb == NCB - 1))
                                    nc.tensor.matmul(
                                        qp, ones1, sq,
                                        start=(cb == 0), stop=(cb == NCB - 1))
                                nc.scalar.mul(meanA[:, sl], mp, 1.0 / C)
                                nc.scalar.mul(msqA[:, sl], qp, 1.0 / C)
                    # attention weight loads: small chunks so queue-jumps
                    # ahead of x-loads steal only ~0.7us DMA slots
                    for (wt_, wd_) in ((wk, wk_d), (wv, wv_d), (wr, wr_d)):
                        for a_ in range(2):
                            for k2_ in range(4):
                                nc.sync.dma_start(
                                    out=wt_[:, a_, k2_],
                                    in_=wd_[:, (a_ * 4 + k2_) * 2048:
                                            (a_ * 4 + k2_ + 1) * 2048])
                    for i_ in range(2):
                        for k2_ in range(4):
                            nc.sync.dma_start(
                                out=wo[i_][:, k2_],
                                in_=wo_d[i_, :, k2_ * 2048:(k2_ + 1) * 2048])
                    for cb in range(NCB):
                        nc.sync.dma_start(out=xT_sp[cb][:], in_=xT[cb].bitcast(F32))
                    rstd_b, mrstd_b = ln_finish(meanA, msqA, p_statA, "A")
                    for cb in range(NCB):
                        pr, j = cb // 2, cb % 2
                        zt = p_tmpA.tile([128, T], F32, tag="zt", bufs=2)
                        nc.vector.tensor_mul(zt, xT[cb].bitcast(F32), rstd_b)
                        dst = z8[pr][:, j, PAD:PAD + T]
                        if cb % 2 == 0:
                            nc.vector.tensor_sub(dst, zt, mrstd_b)
                        else:
                            nc.gpsimd.tensor_sub(dst, zt, mrstd_b)

            # ============ PHASE B: k/v/r projections + WKV per m ============
            es_sry = contextlib.ExitStack()
            p_sry = es_sry.enter_context(tc.tile_pool(name="p_sry", bufs=1))
            sryh = [p_sry.tile([128, 2, T], FP8, tag=f"sryh{pr}", name=f"sryh{pr}")
                    for pr in range(NPR)]
            sryl = [p_sry.tile([128, 2, T], FP8, tag=f"sryl{pr}", name=f"sryl{pr}")
                    for pr in range(NPR)]

            def zsl(k2, a, n):
                # a=0: current tokens; a=1: shifted by one
                lo = PAD - a + n * 512
                return z8[k2][:, :, lo:lo + 512]

            with tc.tile_pool(name="p_kvs", bufs=2) as p_kvs, \
                 tc.tile_pool(name="p_wt", bufs=2) as p_wt, \
                 tc.tile_pool(name="ps_mm", bufs=8, space="PSUM") as ps_mm:
                wkv_state = {}

                def wkv_front(m, ek, vv):
                    # scanB first: depends only on ek (k epilogues), so DVE can
                    # start while Act still runs v/r epilogues. ekv on DVE keeps
                    # the ekv->scanA handoff on-engine (no cross-engine sem).
                    wrow = p_wt.tile([128, T], BF16, tag="wrow", name=f"wr{m}")
                    nc.vector.tensor_scalar(out=wrow, in0=ones_bf,
                                            scalar1=col(CW + m),
                                            scalar2=None, op0=AL.mult)
                    A = p_wt.tile([128, T + 1], BF16, tag="A", name=f"A{m}")
                    Bt = p_wt.tile([128, T + 1], BF16, tag="B", name=f"B{m}")
                    nc.vector.memset(Bt[:, 0:1], 0.0)
                    nc.vector.tensor_tensor_scan(
                        out=Bt[:, 1:T + 1], data0=wrow, data1=ek,
                        initial=0.0, op0=AL.mult, op1=AL.add)
                    ekv = p_wt.tile([128, T], BF16, tag="ekv", name=f"ekv{m}")
                    nc.vector.tensor_mul(ekv, ek, vv)
                    nc.vector.memset(A[:, 0:1], 0.0)
                    nc.vector.tensor_tensor_scan(
                        out=A[:, 1:T + 1], data0=wrow, data1=ekv,
                        initial=0.0, op0=AL.mult, op1=AL.add)
                    nc.vector.scalar_tensor_tensor(
                        out=Bt[:, 0:T], in0=ek, scalar=col(CEU + m),
                        in1=Bt[:, 0:T], op0=AL.mult, op1=AL.add)
                    rec = p_wt.tile([128, T], BF16, tag="rec", name=f"rec{m}")
                    with nc.allow_low_precision(reason="wkv ratio bf16"):
                        nc.vector.reciprocal(rec, Bt[:, 0:T])
                    nc.vector.scalar_tensor_tensor(
                        out=A[:, 0:T], in0=ekv, scalar=col(CEU + m),
                        in1=A[:, 0:T], op0=AL.mult, op1=AL.add)
                    return A, rec

                def wkv_tail(m, A, rec, sr):
                    pr_m, j_m = m // 2, m % 2
                    y = p_wt.tile([128, T], BF16, tag="y", name=f"y{m}")
                    nc.gpsimd.tensor_mul(y, A[:, 0:T], rec)
                    sy = p_wt.tile([128, T], BF16, tag="sy", name=f"sy{m}")
                    nc.gpsimd.tensor_mul(sy, y, sr)
                    nc.scalar.copy(sryh[pr_m][:, j_m, :], sy)
                    nc.vector.tensor_sub(sryl[pr_m][:, j_m, :], sy,
                                         sryh[pr_m][:, j_m, :])

                def proj_one(wt, m, dst, act, bcol):
                    for n in range(NT):
                        pmm = ps_mm.tile([128, 512], F32, tag="pmm")
                        for a in range(2):
                            for k2 in range(NPR):
                                nc.tensor.matmul(
                                    pmm, wt[:, a, k2, :,
                                            m * 128:(m + 1) * 128],
                                    zsl(k2, a, n),
                                    start=(a == 0 and k2 == 0),
                                    stop=(a == 1 and k2 == NPR - 1),
                                    perf_mode=DR)
                        dsl = dst[:, n * 512:(n + 1) * 512]
                        nc.scalar.activation(dsl, pmm, act,
                                             bias=col(bcol + m), scale=INV)

                for m in (range(NCB) if KPHASES >= 2 else ()):
                    ek = p_kvs.tile([128, T], BF16, tag="ek", name=f"ek{m}")
                    vv = p_kvs.tile([128, T], BF16, tag="vv", name=f"vv{m}")
                    sr = p_kvs.tile([128, T], BF16, tag="sr", name=f"sr{m}")
                    proj_one(wk, m, ek, AF.Exp, CBK)
                    proj_one(wv, m, vv, AF.Identity, CBV)
                    if KPHASES >= 3:
                        # front chain starts as soon as k/v epilogues land;
                        # r-proj (only needed by tail, one block later) after
                        A, rec = wkv_front(m, ek, vv)
                    proj_one(wr, m, sr, AF.Sigmoid, CBR)
                    if KPHASES >= 3:
                        wkv_state[m] = (A, rec, sr)
                        if m >= 1:
                            wkv_tail(m - 1, *wkv_state.pop(m - 1))
                if KPHASES >= 3:
                    wkv_tail(NCB - 1, *wkv_state.pop(NCB - 1))

            es_w.close()
            es_z.close()

            # ===== PHASE C: out-proj + residual -> x2, fused LN2 stats =====
            es_x2 = contextlib.ExitStack()
            p_x2 = es_x2.enter_context(tc.tile_pool(name="p_x2", bufs=1))
            x2 = [p_x2.tile([128, T], F32R, tag=f"x2_{cb}", name=f"x2_{cb}")
                  for cb in range(NCB)]
            es_z2 = contextlib.ExitStack()
            p_z2 = es_z2.enter_context(tc.tile_pool(name="p_z2", bufs=1,
                                                    side="right"))
            z2t = [p_z2.tile([128, T + 1], BF16, tag=f"z2_{cb}", name=f"z2_{cb}")
                   for cb in range(NCB)]
            with tc.tile_pool(name="p_xr", bufs=2) as p_xr, \
                 tc.tile_pool(name="p_tmpD", bufs=1) as p_tmpD, \
                 tc.tile_pool(name="p_statD", bufs=1) as p_statD:
              with tc.tile_pool(name="ps_mo", bufs=4, space="PSUM") as ps_mo, \
                   tc.tile_pool(name="ps_st2", bufs=1, space="PSUM") as ps_st2:
                st_half = [(ps_st2.tile([1, 512], F32, tag=f"m{i}", name=f"mD{i}"),
                            ps_st2.tile([1, 512], F32, tag=f"q{i}", name=f"qD{i}"))
                           for i in range(2)]
                for m in (range(NCB) if KPHASES >= 4 else ()):
                    xr = p_xr.tile([128, T], F32, tag="xr")
                    nc.sync.dma_start(out=xr, in_=xT_sp[m][:])
                    for n in range(NT):
                        sl = slice(n * 512, (n + 1) * 512)
                        pmm = ps_mo.tile([128, 512], F32, tag="pmo")
                        first = True
                        for (wi, ss) in ((0, sryh), (1, sryh), (0, sryl)):
                            for k2 in range(NPR):
                                nc.tensor.matmul(
                                    pmm, wo[wi][:, k2, :, m * 128:(m + 1) * 128],
                                    ss[k2][:, :, sl],
                                    start=first,
                                    stop=(wi == 0 and ss is sryl
                                          and k2 == NPR - 1),
                                    perf_mode=DR)
                                first = False
                        nc.vector.scalar_tensor_tensor(
                            out=x2[m][:, sl], in0=pmm, scalar=INV,
                            in1=xr[:, sl], op0=AL.mult, op1=AL.add)
                    if KPHASES >= 5:
                        for i in range(2):
                            sl2 = slice(i * 512, (i + 1) * 512)
                            sq = p_tmpD.tile([128, 512], F32R, tag="sqD", bufs=3)
                            nc.scalar.activation(sq, x2[m].bitcast(F32)[:, sl2],
                                                 AF.Square)
                            nc.tensor.matmul(st_half[i][0], ones1, x2[m][:, sl2],
                                             start=(m == 0), stop=(m == NCB - 1))
                            nc.tensor.matmul(st_half[i][1], ones1, sq,
                                             start=(m == 0), stop=(m == NCB - 1))
                for m in (range(NCB) if KPHASES >= 4 else ()):
                    nc.sync.dma_start(out=x2_sp[m][:], in_=x2[m].bitcast(F32))
                if KPHASES >= 5:
                    meanD = p_statD.tile([1, T], BF16, tag="mean_sb")
                    msqD = p_statD.tile([1, T], BF16, tag="msq_sb")
                    for i in range(2):
                        sl2 = slice(i * 512, (i + 1) * 512)
                        nc.scalar.mul(meanD[:, sl2], st_half[i][0], 1.0 / C)
                        nc.scalar.mul(msqD[:, sl2], st_half[i][1], 1.0 / C)
              # ---- LN2 stats + finish -> z2 (plain bf16, col 0 zero) ----
              if True:
                if KPHASES >= 5:
                    with tc.tile_pool(name="ps_stD", bufs=1,
                                      space="PSUM") as ps_stD:
                        st2 = [(ps_stD.tile([1, 512], F32, tag=f"m2{i}",
                                            name=f"mD2{i}"),
                                ps_stD.tile([1, 512], F32, tag=f"q2{i}",
                                            name=f"qD2{i}")) for i in range(2)]
                        for i in range(2):
                            n = 2 + i
                            sl2 = slice(n * 512, (n + 1) * 512)
                            for cb in range(NCB):
                                sq = p_tmpD.tile([128, 512], F32R, tag="sqD",
                                                 bufs=3)
                                nc.scalar.activation(
                                    sq, x2[cb].bitcast(F32)[:, sl2], AF.Square)
                                nc.tensor.matmul(st2[i][0], ones1,
                                                 x2[cb][:, sl2],
                                                 start=(cb == 0),
                                                 stop=(cb == NCB - 1))
                                nc.tensor.matmul(st2[i][1], ones1, sq,
                                                 start=(cb == 0),
                                                 stop=(cb == NCB - 1))
                            nc.scalar.mul(meanD[:, sl2], st2[i][0], 1.0 / C)
                            nc.scalar.mul(msqD[:, sl2], st2[i][1], 1.0 / C)
                    rstd_b2, mrstd_b2 = ln_finish(meanD, msqD, p_statD, "D")
                    for cb in range(NCB):
                        nc.vector.memset(z2t[cb][:, 0:1], 0.0)
                        zt = p_tmpD.tile([128, T], F32, tag="zt2", bufs=1)
                        nc.vector.tensor_mul(zt, x2[cb].bitcast(F32), rstd_b2)
                        dst = z2t[cb][:, 1:T + 1]
                        if cb % 2 == 0:
                            nc.vector.tensor_sub(dst, zt, mrstd_b2)
                        else:
                            nc.gpsimd.tensor_sub(dst, zt, mrstd_b2)
            es_x2.close()
            es_sry.close()
            es_wo.close()

            # FFN weights: fwv hi/lo resident fp8; fwr till srf; fwk streamed
            es_fw = contextlib.ExitStack()
            p_fw = es_fw.enter_context(tc.tile_pool(name="p_fw", bufs=1))
            fwv = [p_fw.tile([128, 16, 2, 1024], FP8, tag=f"fwv{i}",
                             name=f"fwv{i}") for i in range(2)]
            if KPHASES >= 5:
                nc.sync.dma_start(out=fwv[0], in_=fwv_d[0])
                nc.sync.dma_start(out=fwv[1], in_=fwv_d[1])

            # ============ PHASE E: xf lerp (f_tmk == f_tmr), fWr -> srf ========
            es_xf = contextlib.ExitStack()
            p_xf = es_xf.enter_context(tc.tile_pool(name="p_xf", bufs=1))
            xfh = [p_xf.tile([128, 2, T], FP8, tag=f"xfh{pr}", name=f"xfh{pr}")
                   for pr in range(NPR)]
            xfl = [p_xf.tile([128, 2, T], FP8, tag=f"xfl{pr}", name=f"xfl{pr}")
                   for pr in range(NPR)]
            with tc.tile_pool(name="p_te", bufs=3) as p_te:
                for cb in (range(NCB) if KPHASES >= 6 else ()):
                    pr, j = cb // 2, cb % 2
                    t1 = p_te.tile([128, T], BF16, tag="t1")
                    nc.scalar.mul(t1, z2t[cb][:, 0:T], col(CFT1 + cb))
                    xfb = p_te.tile([128, T], BF16, tag="xfb")
                    nc.vector.scalar_tensor_tensor(
                        out=xfb, in0=z2t[cb][:, 1:T + 1],
                        scalar=col(CFT + cb), in1=t1, op0=AL.mult, op1=AL.add)
                    nc.scalar.copy(xfh[pr][:, j, :], xfb)
                    nc.gpsimd.tensor_sub(xfl[pr][:, j, :], xfb, xfh[pr][:, j, :])
            es_z2.close()

            es_srf = contextlib.ExitStack()
            p_srf = es_srf.enter_context(tc.tile_pool(name="p_srf", bufs=1))
            srf = [p_srf.tile([128, T], FP8, tag=f"srf{m}", name=f"srf{m}")
                   for m in range(NCB)]
            with tc.tile_pool(name="p_fwr", bufs=1) as p_fwr, \
                 tc.tile_pool(name="ps_fr", bufs=4, space="PSUM") as ps_fr:
                fwr = [p_fwr.tile([128, 4, 2, 1024], FP8, tag=f"fwr{i}",
                                  name=f"fwr{i}") for i in range(2)]
                if KPHASES >= 6:
                    nc.sync.dma_start(out=fwr[0], in_=fwr_d[0])
                    nc.sync.dma_start(out=fwr[1], in_=fwr_d[1])
                for m in (range(NCB) if KPHASES >= 6 else ()):
                    for n in range(NT):
                        pmm = ps_fr.tile([128, 512], F32, tag="pfr")
                        first = True
                        for (wi, xs) in ((0, xfh), (1, xfh), (0, xfl)):
                            for k2 in range(NPR):
                                nc.tensor.matmul(
                                    pmm, fwr[wi][:, k2, :, m * 128:(m + 1) * 128],
                                    xs[k2][:, :, n * 512:(n + 1) * 512],
                                    start=first,
                                    stop=(wi == 0 and xs is xfl and k2 == NPR - 1),
                                    perf_mode=DR)
                                first = False
                        nc.scalar.activation(srf[m][:, n * 512:(n + 1) * 512],
                                             pmm, AF.Sigmoid, bias=col(CBFR + m),
                                             scale=INV)

            # ============ PHASE F: FFN k/v matmuls + output ============
            # 3-pass residual fp8: W*x ~ Wh*xh + Wl*xh + Wh*xl
            with tc.tile_pool(name="p_fwkg", bufs=2) as p_fwkg, \
                 tc.tile_pool(name="p_kk", bufs=1) as p_kk, \
                 tc.tile_pool(name="p_rl", bufs=4) as p_rl, \
                 tc.tile_pool(name="p_x2c", bufs=3) as p_x2c, \
                 tc.tile_pool(name="p_fin", bufs=2) as p_fin, \
                 tc.tile_pool(name="p_ost", bufs=1) as p_ost, \
                 tc.tile_pool(name="ps_fk", bufs=3, space="PSUM") as ps_fk, \
                 tc.tile_pool(name="ps_fo", bufs=2, space="PSUM") as ps_fo, \
                 tc.tile_pool(name="ps_ot", bufs=3, space="PSUM") as ps_ot:
                for n in (range(NT) if KPHASES >= 7 else ()):
                    sl = slice(n * 512, (n + 1) * 512)
                    kkh = [p_kk.tile([128, 2, 512], FP8, tag=f"kkh{hp}",
                                     name=f"kkh{hp}_{n}") for hp in range(NHP)]
                    kkl = [p_kk.tile([128, 2, 512], FP8, tag=f"kkl{hp}",
                                     name=f"kkl{hp}_{n}") for hp in range(NHP)]
                    for g in range(8):
                        fg = [p_fwkg.tile([128, 4, 2, 512], FP8, tag=f"fwkg{i}",
                                          name=f"fwkg{i}_{n}_{g}")
                              for i in range(2)]
                        nc.sync.dma_start(out=fg[0], in_=fwk_d[0, g])
                        nc.sync.dma_start(out=fg[1], in_=fwk_d[1, g])
                        for i in range(4):
                            hb = g * 4 + i
                            hp, jh = hb // 2, hb % 2
                            pkk = ps_fk.tile([128, 512], F32, tag="pkk")
                            first = True
                            for (wi, xs) in ((0, xfh), (1, xfh), (0, xfl)):
                                for k2 in range(NPR):
                                    nc.tensor.matmul(
                                        pkk,
                                        fg[wi][:, k2, :, i * 128:(i + 1) * 128],
                                        xs[k2][:, :, sl],
                                        start=first,
                                        stop=(wi == 0 and xs is xfl
                                              and k2 == NPR - 1),
                                        perf_mode=DR)
                                    first = False
                            rl = p_rl.tile([128, 512], BF16, tag="rl")
                            if hb % 2 == 0:
                                nc.scalar.activation(rl, pkk, AF.Relu,
                                                     bias=col(CBFK + hb),
                                                     scale=INV)
                            else:
                                nc.vector.tensor_scalar(
                                    out=rl, in0=pkk, scalar1=INV,
                                    scalar2=0.0, op0=AL.mult, op1=AL.max)
                            t2 = p_rl.tile([128, 512], BF16, tag="t2")
                            nc.vector.tensor_mul(t2, rl, rl)
                            dh = kkh[hp][:, jh, :]
                            if hb % 2 == 0:
                                nc.scalar.copy(dh, t2)
                            else:
                                nc.vector.tensor_copy(out=dh, in_=t2)
                            nc.gpsimd.tensor_sub(kkl[hp][:, jh, :], t2, dh)
                    osts = [p_ost.tile([128, C], F32, tag=f"ost{j}",
                                       name=f"ost{n}_{j}") for j in range(4)]
                    for m in range(NCB):
                        po = ps_fo.tile([128, 512], F32, tag="po")
                        first = True
                        for (wi, ks) in ((0, kkh), (1, kkh), (0, kkl)):
                            for hp in range(NHP):
                                nc.tensor.matmul(
                                    po, fwv[wi][:, hp, :, m * 128:(m + 1) * 128],
                                    ks[hp],
                                    start=first,
                                    stop=(wi == 0 and ks is kkl
                                          and hp == NHP - 1),
                                    perf_mode=DR)
                                first = False
                        x2c = p_x2c.tile([128, 512], F32, tag="x2c")
                        nc.sync.dma_start(out=x2c, in_=x2_sp[m][:, sl])
                        rkv = p_fin.tile([128, 512], F32, tag="rkv")
                        nc.vector.scalar_tensor_tensor(
                            out=rkv, in0=po, scalar=INV, in1=srf[m][:, sl],
                            op0=AL.mult, op1=AL.mult)
                        fin = p_fin.tile([128, 512], F32, tag="fin")
                        if m % 2 == 0:
                            nc.gpsimd.tensor_add(fin, rkv, x2c)
                        else:
                            nc.vector.tensor_add(fin, rkv, x2c)
                        for j in range(4):
                            pt = ps_ot.tile([128, 128], F32, tag="ptr")
                            nc.tensor.transpose(pt, fin[:, j * 128:(j + 1) * 128],
                                                ident)
                            dst = osts[j][:, m * 128:(m + 1) * 128]
                            if (m + j) % 2 == 0:
                                nc.scalar.copy(dst, pt)
                            else:
                                nc.vector.tensor_copy(out=dst, in_=pt)
                    for j in range(4):
                        tb = n * 4 + j
                        nc.sync.dma_start(out=out_d[tb * 128:(tb + 1) * 128, :],
                                          in_=osts[j])
            es_srf.close()
            es_xf.close()
            es_fw.close()
    nc.finalize()
    return nc


_PROG = None


def _get_prog():
    global _PROG
    if _PROG is None:
        nc = bacc.Bacc()
        _PROG = _emit(nc)
    return _PROG


def _pair_w(WT, M_out):
    """WT: [K_in, M_out] fp8 (lhsT layout) -> [128, K_in//256, 2, M_out] flat."""
    K_in = WT.shape[0]
    npr = K_in // 256
    return np.ascontiguousarray(
        WT.reshape(npr, 2, 128, M_out).transpose(2, 0, 1, 3).reshape(128, -1))


def _q8_hl(WT):
    """WT f32 (pre-scaled by WS) -> (hi, lo) fp8 arrays."""
    f8 = ml_dtypes.float8_e4m3
    Ws = np.asarray(WT, np.float32) * np.float32(WS)
    assert np.abs(Ws).max() < 230.0
    hi = Ws.astype(f8)
    lo = (Ws - hi.astype(np.float32)).astype(f8)
    return hi, lo


def _fwk_hl(WT):
    """WT: [C, HID] -> fp8 [2(hl), 8(g), 128, 4(k2)*2(j)*512]; g = hid cols 512g."""
    hi, lo = _q8_hl(WT)
    out = []
    for W8 in (hi, lo):
        # pair layout per group: [128, k2, j, 512]
        Wp = W8.reshape(4, 2, 128, HID)  # [k2, j, c128, h]
        out.append(np.stack(
            [np.ascontiguousarray(
                Wp[:, :, :, g * 512:(g + 1) * 512].transpose(2, 0, 1, 3)
                .reshape(128, -1)) for g in range(8)]))
    return np.ascontiguousarray(np.stack(out))


def _fwv_hl(WT):
    """WT: [HID, C] -> fp8 [2(hl), 128, 16*2*1024] pair layout."""
    hi, lo = _q8_hl(WT)
    return np.ascontiguousarray(np.stack([_pair_w(W8, C) for W8 in (hi, lo)]))


def _q8s(W):
    """Scale by WS, quantize to fp8e4 (checks range)."""
    f8 = ml_dtypes.float8_e4m3
    Ws = np.asarray(W, np.float32) * np.float32(WS)
    assert np.abs(Ws).max() < 230.0, "weight scale overflow"
    return Ws.astype(f8)


def _prep_inputs(x, ln1_g, ln1_b, ln2_g, ln2_b, time_decay, time_first,
                 tmk, tmv, tmr, Wk, Wv, Wr, Wo, f_tmk, f_tmr, fWk, fWr, fWv):
    f32 = np.float32
    x = np.asarray(x, f32)
    g1 = np.asarray(ln1_g, f32); b1 = np.asarray(ln1_b, f32)
    g2 = np.asarray(ln2_g, f32); b2 = np.asarray(ln2_b, f32)
    td = np.asarray(time_decay, np.float64); tf = np.asarray(time_first, np.float64)
    tmk = np.asarray(tmk, f32).reshape(C); tmv = np.asarray(tmv, f32).reshape(C)
    tmr = np.asarray(tmr, f32).reshape(C)
    ftmk = np.asarray(f_tmk, f32).reshape(C); ftmr = np.asarray(f_tmr, f32).reshape(C)
    assert np.array_equal(ftmk, ftmr), "kernel assumes f_tmk == f_tmr"
    Wk = np.asarray(Wk, f32); Wv = np.asarray(Wv, f32); Wr = np.asarray(Wr, f32)
    Wo = np.asarray(Wo, f32); fWk = np.asarray(fWk, f32); fWr = np.asarray(fWr, f32)
    fWv = np.asarray(fWv, f32)

    Wk1 = Wk * g1[None, :]; Wv1 = Wv * g1[None, :]; Wr1 = Wr * g1[None, :]
    bk = Wk @ b1; bv = Wv @ b1; br = Wr @ b1
    fWk1 = fWk * g2[None, :]; fWr1 = fWr * g2[None, :]
    bfk = fWk @ b2; bfr = fWr @ b2
    assert np.allclose(bfk, 0.0), "kernel assumes zero ln2 beta for relu path"

    wbar = np.exp(-np.exp(td)).astype(f32)
    eu = np.exp(tf).astype(f32)

    def packc(v):
        return np.asarray(v, f32).reshape(-1, 128).T

    cst = np.zeros((128, NCOLS), f32)
    cst[:, CW:CW + 8] = packc(wbar)
    cst[:, CEU:CEU + 8] = packc(eu)
    cst[:, CBK:CBK + 8] = packc(bk)
    cst[:, CBV:CBV + 8] = packc(bv)
    cst[:, CBR:CBR + 8] = packc(br)
    cst[:, CFT:CFT + 8] = packc(ftmk)
    cst[:, CFT1:CFT1 + 8] = packc(1 - ftmk)
    cst[:, CBFR:CBFR + 8] = packc(bfr)
    cst[:, CEPS] = EPS
    cst[:, CBFK:CBFK + 32] = packc(bfk)

    def lerp_pair(W1, tm):
        # [128, 2(ab), 4(k2), 2(j), 1024] flat; a=0: W*tm, a=1: W*(1-tm)
        Wa = _pair_w(_q8s((W1 * tm[None, :]).T), C)
        Wb = _pair_w(_q8s((W1 * (1 - tm)[None, :]).T), C)
        return np.ascontiguousarray(
            np.stack([Wa.reshape(128, 4, 2, 1024),
                      Wb.reshape(128, 4, 2, 1024)], axis=1).reshape(128, -1))

    shared = {
        "wk": lerp_pair(Wk1, tmk),
        "wv": lerp_pair(Wv1, tmv),
        "wr": lerp_pair(Wr1, tmr),
        "wo": _fwv_hl(Wo.T),
        "fwr": _fwv_hl(fWr1.T),
        "fwk": _fwk_hl(fWk1.T),
        "fwv": _fwv_hl(fWv.T),
        "cst": cst,
        "ones1": np.ones((128, 1), f32),
        "onesb": np.ones((1, 128), ml_dtypes.bfloat16),
        "ident": np.eye(128, dtype=f32),
    }
    in_maps = [dict(shared, x=np.ascontiguousarray(x[b])) for b in range(B)]
    return in_maps


def _run(in_maps, trace=False, **kw):
    nc = _get_prog()
    res = run_bass_kernel_spmd(nc, in_maps, core_ids=list(range(B)), trace=trace,
                               **kw)
    out = np.stack([np.asarray(res.results[b]["out"]) for b in range(B)], axis=0)
    return out.astype(np.float32), res


def kernel(*a, **kw):
    out, _ = _run(_prep_inputs(*a, **kw))
    return out


if __name__ == "__main__":
    _get_prog()
    print("program built ok")


# revision 8
# speedup vs baseline: 1.0931x; 1.0109x over previous
"""RWKV-4 block (nn_Block_5669356833485) Trainium2 Bass kernel.

B=8, T=2048, C=1024, HID=4096. B-sharded across 8 NeuronCores (1 batch/core).
Feature-major layout [C-partitions, T-free].

fp8e4 DoubleRow matmuls (256-wide contraction, 0.5 cyc/row). Weights are
host-scaled by 128 before fp8 quantization (their natural ~0.02 magnitude
falls in e4m3's subnormal range) and unscaled in the matmul epilogues.
Time-mix lerps are folded into the matmuls by doubling the contraction
against z and shifted-z (z pair tiles [128, 2, 2064], data offset 16, pair
stride %16==0 per DoubleRow requirements). LN stats are pipelined into the
producing loops; WKV (bf16 scans, fp32 state) interleaves per channel block
with the projections. ek/v/sr/srf stay in SBUF; only xT and x2 round-trip
through DRAM for the residual adds.
Self-contained: hardcodes shapes; no sibling imports.
"""
import os
import sys
sys.path.insert(0, '/opt/trn_rl_repo')

KPHASES = int(os.environ.get("KPHASES", "99"))

import numpy as np
import ml_dtypes

import concourse.bass as bass
from concourse import bacc
import concourse.mybir as mybir
import concourse.tile as tile
from concourse.bass_utils import run_bass_kernel_spmd

F32 = mybir.dt.float32
F32R = mybir.dt.float32r
BF16 = mybir.dt.bfloat16
FP8 = mybir.dt.float8e4
AL = mybir.AluOpType
AF = mybir.ActivationFunctionType
DR = mybir.MatmulPerfMode.DoubleRow

B, T, C, HID = 8, 2048, 1024, 4096
NCB = C // 128          # 8 channel blocks
NPR = NCB // 2          # 4 channel pair-blocks
NHB = HID // 128        # 32 hidden blocks
NHP = NHB // 2          # 16 hidden pair-blocks
NT = T // 512           # 4 n-slices of 512
NTB = T // 128          # 16 token blocks
PAD = 16                # z pair tiles: [128, 2, PAD+T]; pair stride %16==0
TP = T + PAD
EPS = 1e-5
WS = 128.0              # weight pre-quantization scale
INV = 1.0 / WS

# cst columns (per 128-partition, indexed by block)
CW = 0        # wbar          [0:8)   by cb
CEU = 8       # exp(tf)       [8:16)  by cb
CBK = 16      # bk            [16:24) by m
CBV = 24      # bv            [24:32) by m
CBR = 32      # br            [32:40) by m
CFT = 40      # ftmk          [40:48) by cb
CFT1 = 48     # 1-ftmk        [48:56) by cb
CBFR = 56     # bfr           [56:64) by m
CEPS = 64     # eps           col 64
CBFK = 72     # bfk           [72:104) by hb
NCOLS = 104


def _emit(nc):
    # ---------------- DRAM I/O ----------------
    x_d = nc.declare_dram_parameter("x", [T, C], F32, isOutput=False)
    wk_d = nc.declare_dram_parameter("wk", [128, 2 * 4 * 2 * 1024], FP8, isOutput=False)
    wv_d = nc.declare_dram_parameter("wv", [128, 2 * 4 * 2 * 1024], FP8, isOutput=False)
    wr_d = nc.declare_dram_parameter("wr", [128, 2 * 4 * 2 * 1024], FP8, isOutput=False)
    wo_d = nc.declare_dram_parameter("wo", [2, 128, 4 * 2 * 1024], FP8, isOutput=False)
    fwr_d = nc.declare_dram_parameter("fwr", [2, 128, 4 * 2 * 1024], FP8, isOutput=False)
    fwk_d = nc.declare_dram_parameter("fwk", [2, 8, 128, 4 * 2 * 512], FP8, isOutput=False)
    fwv_d = nc.declare_dram_parameter("fwv", [2, 128, 16 * 2 * 1024], FP8, isOutput=False)
    cst_d = nc.declare_dram_parameter("cst", [128, NCOLS], F32, isOutput=False)
    ones1_d = nc.declare_dram_parameter("ones1", [128, 1], F32R, isOutput=False)
    onesb_d = nc.declare_dram_parameter("onesb", [1, 128], BF16, isOutput=False)
    ident_d = nc.declare_dram_parameter("ident", [128, 128], F32, isOutput=False)
    out_d = nc.declare_dram_parameter("out", [T, C], F32, isOutput=True)

    # DRAM scratch (per-cb granularity for fine deps)
    xT_sp = [nc.dram_tensor(f"xT_sp{i}", [128, T], F32) for i in range(NCB)]
    x2_sp = [nc.dram_tensor(f"x2_sp{i}", [128, T], F32) for i in range(NCB)]

    import contextlib

    with tile.TileContext(nc, pool_alloc_mode="queue") as tc:
        with tc.tile_pool(name="pc", bufs=1) as pc:
            cst = pc.tile([128, NCOLS], F32)
            nc.sync.dma_start(out=cst, in_=cst_d[:])
            ones1 = pc.tile([128, 1], F32R)
            nc.sync.dma_start(out=ones1, in_=ones1_d[:])
            onesb = pc.tile([1, 128], BF16)
            nc.sync.dma_start(out=onesb, in_=onesb_d[:])
            ident = pc.tile([128, 128], F32)
            nc.sync.dma_start(out=ident, in_=ident_d[:])
            ones_bf = pc.tile([128, T], BF16)
            nc.vector.memset(ones_bf, 1.0)

            def col(j):
                return cst[:, j:j + 1]

            # ---- incremental LN stats: two [1, T] psum tiles ----
            def ln_contrib(stat_ps, pool_tmp, src_f32r, cb, sl, tag):
                """Add channel-block cb's contribution for column slice sl."""
                mean_ps, msq_ps = stat_ps
                w = sl.stop - sl.start
                sq = pool_tmp.tile([128, w], F32R, tag=tag, bufs=3)
                nc.scalar.activation(sq, src_f32r.bitcast(F32)[:, sl], AF.Square)
                nc.tensor.matmul(mean_ps[:, sl], ones1, src_f32r[:, sl],
                                 start=(cb == 0), stop=(cb == NCB - 1))
                nc.tensor.matmul(msq_ps[:, sl], ones1, sq,
                                 start=(cb == 0), stop=(cb == NCB - 1))

            def ln_to_sbuf(stat_ps, pool_stat):
                mean_ps, msq_ps = stat_ps
                mean = pool_stat.tile([1, T], BF16, tag="mean_sb")
                msq = pool_stat.tile([1, T], BF16, tag="msq_sb")
                nc.scalar.mul(mean, mean_ps, 1.0 / C)
                nc.scalar.mul(msq, msq_ps, 1.0 / C)
                return mean, msq

            def ln_finish(mean, msq, pool_stat, uid):
                var = pool_stat.tile([1, T], BF16, tag="var_sb")
                nc.vector.tensor_mul(var, mean, mean)
                nc.vector.tensor_sub(var, msq, var)
                lnv = pool_stat.tile([1, T], BF16, tag="msq_sb", name=f"lnv{uid}")
                nc.scalar.activation(lnv, var, AF.Ln,
                                     bias=cst[0:1, CEPS:CEPS + 1], scale=1.0)
                rstd = pool_stat.tile([1, T], BF16, tag="var_sb", name=f"rstd{uid}")
                nc.scalar.activation(rstd, lnv, AF.Exp, bias=0.0, scale=-0.5)
                mrstd = pool_stat.tile([1, T], BF16, tag="mrstd_sb")
                nc.vector.tensor_mul(mrstd, mean, rstd)
                rstd_b = pool_stat.tile([128, T], BF16, tag="rstd_b")
                mrstd_b = pool_stat.tile([128, T], BF16, tag="mrstd_b")
                with tc.tile_pool(name=f"ps_bc{uid}", bufs=2, space="PSUM") as ps_bc:
                    for (src_s, dst) in ((rstd, rstd_b), (mrstd, mrstd_b)):
                        for n in range(NT):
                            sl = slice(n * 512, (n + 1) * 512)
                            bc = ps_bc.tile([128, 512], F32, tag="bc")
                            nc.tensor.matmul(bc, onesb, src_s[:, sl],
                                             start=True, stop=True)
                            if n % 2 == 0:
                                nc.scalar.copy(dst[:, sl], bc)
                            else:
                                nc.vector.tensor_copy(out=dst[:, sl], in_=bc)
                return rstd_b, mrstd_b

            # z pair tiles live through phase B (attention)
            es_z = contextlib.ExitStack()
            p_z = es_z.enter_context(tc.tile_pool(name="p_z", bufs=1, side="right"))
            z8 = [p_z.tile([128, 2, TP], FP8, tag=f"z{pr}", name=f"z{pr}")
                  for pr in range(NPR)]
            for pr in range(NPR):
                nc.vector.memset(z8[pr][:, :, 0:PAD], 0.0)

            # attention weights: prefetch during phase A
            es_wo = contextlib.ExitStack()
            p_wo = es_wo.enter_context(tc.tile_pool(name="p_wo", bufs=1))
            wo = [p_wo.tile([128, 4, 2, 1024], FP8, tag=f"wo{i}",
                            name=f"wo{i}") for i in range(2)]
            es_w = contextlib.ExitStack()
            p_w = es_w.enter_context(tc.tile_pool(name="p_w", bufs=1, side="right"))
            wk = p_w.tile([128, 2, 4, 2, 1024], FP8, tag="wk")
            wv = p_w.tile([128, 2, 4, 2, 1024], FP8, tag="wv")
            wr = p_w.tile([128, 2, 4, 2, 1024], FP8, tag="wr")

            # ================= PHASE A: load, transpose, LN1, z =================
            with tc.tile_pool(name="p_xT", bufs=1) as p_xT:
                xT = [p_xT.tile([128, T], F32R, tag=f"xT{cb}", name=f"xT{cb}")
                      for cb in range(NCB)]
                with tc.tile_pool(name="p_tmpA", bufs=1) as p_tmpA, \
                     tc.tile_pool(name="p_statA", bufs=1) as p_statA:
                    meanA = p_statA.tile([1, T], BF16, tag="mean_sb")
                    msqA = p_statA.tile([1, T], BF16, tag="msq_sb")
                    with tc.tile_pool(name="p_ld", bufs=3) as p_ld, \
                         tc.tile_pool(name="ps_stA", bufs=2,
                                      space="PSUM") as ps_stA, \
                         tc.tile_pool(name="ps_tr", bufs=4, space="PSUM") as ps_tr:
                        for tb in range(NTB):
                            xt = p_ld.tile([128, C], F32, tag="xtok")
                            nc.sync.dma_start(out=xt,
                                              in_=x_d[tb * 128:(tb + 1) * 128, :])
                            for cb in range(NCB):
                                pt = ps_tr.tile([128, 128], F32, tag="tr")
                                nc.tensor.transpose(
                                    pt, xt[:, cb * 128:(cb + 1) * 128], ident)
                                dst = xT[cb][:, tb * 128:(tb + 1) * 128]
                                if (tb + cb) % 2 == 0:
                                    nc.scalar.copy(dst, pt)
                                else:
                                    nc.vector.tensor_copy(out=dst, in_=pt)
                            if tb % 4 == 3:
                                n = tb // 4
                                sl = slice(n * 512, (n + 1) * 512)
                                mp = ps_stA.tile([1, 512], F32, tag="mA",
                                                 name=f"mA{n}")
                                qp = ps_stA.tile([1, 512], F32, tag="qA",
                                                 name=f"qA{n}")
                                for cb in range(NCB):
                                    sq = p_tmpA.tile([128, 512], F32R,
                                                     tag="sqA", bufs=3)
                                    nc.scalar.activation(
                                        sq, xT[cb].bitcast(F32)[:, sl],
                                        AF.Square)
                                    nc.tensor.matmul(
                                        mp, ones1, xT[cb][:, sl],
                                        start=(cb == 0), stop=(cb == NCB - 1))
                                    nc.tensor.matmul(
                                        qp, ones1, sq,
                                        start=(cb == 0), stop=(cb == NCB - 1))
                                nc.scalar.mul(meanA[:, sl], mp, 1.0 / C)
                                nc.scalar.mul(msqA[:, sl], qp, 1.0 / C)
                    # attention weight loads: small chunks so queue-jumps
                    # ahead of x-loads steal only ~0.7us DMA slots
                    for (wt_, wd_) in ((wk, wk_d), (wv, wv_d), (wr, wr_d)):
                        for a_ in range(2):
                            for k2_ in range(4):
                                nc.sync.dma_start(
                                    out=wt_[:, a_, k2_],
                                    in_=wd_[:, (a_ * 4 + k2_) * 2048:
                                            (a_ * 4 + k2_ + 1) * 2048])
                    for i_ in range(2):
                        for k2_ in range(4):
                            nc.sync.dma_start(
                                out=wo[i_][:, k2_],
                                in_=wo_d[i_, :, k2_ * 2048:(k2_ + 1) * 2048])
                    for cb in range(NCB):
                        nc.sync.dma_start(out=xT_sp[cb][:], in_=xT[cb].bitcast(F32))
                    rstd_b, mrstd_b = ln_finish(meanA, msqA, p_statA, "A")
                    # per-slice so phase B's first matmuls start after the
                    # first 512 tokens of z are ready, not the full rows
                    for n in range(NT):
                        sl = slice(n * 512, (n + 1) * 512)
                        for cb in range(NCB):
                            pr, j = cb // 2, cb % 2
                            zt = p_tmpA.tile([128, 512], F32, tag="zts", bufs=3)
                            nc.vector.tensor_mul(zt, xT[cb].bitcast(F32)[:, sl],
                                                 rstd_b[:, sl])
                            dst = z8[pr][:, j, PAD + n * 512:PAD + (n + 1) * 512]
                            if cb % 2 == 0:
                                nc.vector.tensor_sub(dst, zt, mrstd_b[:, sl])
                            else:
                                nc.gpsimd.tensor_sub(dst, zt, mrstd_b[:, sl])

            # ============ PHASE B: k/v/r projections + WKV per m ============
            es_sry = contextlib.ExitStack()
            p_sry = es_sry.enter_context(tc.tile_pool(name="p_sry", bufs=1))
            sryh = [p_sry.tile([128, 2, T], FP8, tag=f"sryh{pr}", name=f"sryh{pr}")
                    for pr in range(NPR)]
            sryl = [p_sry.tile([128, 2, T], FP8, tag=f"sryl{pr}", name=f"sryl{pr}")
                    for pr in range(NPR)]

            def zsl(k2, a, n):
                # a=0: current tokens; a=1: shifted by one
                lo = PAD - a + n * 512
                return z8[k2][:, :, lo:lo + 512]

            with tc.tile_pool(name="p_kvs", bufs=2) as p_kvs, \
                 tc.tile_pool(name="p_wt", bufs=2) as p_wt, \
                 tc.tile_pool(name="ps_mm", bufs=8, space="PSUM") as ps_mm:
                wkv_state = {}

                def wkv_front(m, ek, vv):
                    # scanB first: depends only on ek (k epilogues), so DVE can
                    # start while Act still runs v/r epilogues. ekv on DVE keeps
                    # the ekv->scanA handoff on-engine (no cross-engine sem).
                    wrow = p_wt.tile([128, T], BF16, tag="wrow", name=f"wr{m}")
                    nc.vector.tensor_scalar(out=wrow, in0=ones_bf,
                                            scalar1=col(CW + m),
                                            scalar2=None, op0=AL.mult)
                    A = p_wt.tile([128, T + 1], BF16, tag="A", name=f"A{m}")
                    Bt = p_wt.tile([128, T + 1], BF16, tag="B", name=f"B{m}")
                    nc.vector.memset(Bt[:, 0:1], 0.0)
                    nc.vector.tensor_tensor_scan(
                        out=Bt[:, 1:T + 1], data0=wrow, data1=ek,
                        initial=0.0, op0=AL.mult, op1=AL.add)
                    ekv = p_wt.tile([128, T], BF16, tag="ekv", name=f"ekv{m}")
                    nc.vector.tensor_mul(ekv, ek, vv)
                    nc.vector.memset(A[:, 0:1], 0.0)
                    nc.vector.tensor_tensor_scan(
                        out=A[:, 1:T + 1], data0=wrow, data1=ekv,
                        initial=0.0, op0=AL.mult, op1=AL.add)
                    nc.vector.scalar_tensor_tensor(
                        out=Bt[:, 0:T], in0=ek, scalar=col(CEU + m),
                        in1=Bt[:, 0:T], op0=AL.mult, op1=AL.add)
                    rec = p_wt.tile([128, T], BF16, tag="rec", name=f"rec{m}")
                    with nc.allow_low_precision(reason="wkv ratio bf16"):
                        nc.vector.reciprocal(rec, Bt[:, 0:T])
                    nc.vector.scalar_tensor_tensor(
                        out=A[:, 0:T], in0=ekv, scalar=col(CEU + m),
                        in1=A[:, 0:T], op0=AL.mult, op1=AL.add)
                    return A, rec

                def wkv_tail(m, A, rec, sr):
                    pr_m, j_m = m // 2, m % 2
                    y = p_wt.tile([128, T], BF16, tag="y", name=f"y{m}")
                    nc.gpsimd.tensor_mul(y, A[:, 0:T], rec)
                    sy = p_wt.tile([128, T], BF16, tag="sy", name=f"sy{m}")
                    nc.gpsimd.tensor_mul(sy, y, sr)
                    nc.scalar.copy(sryh[pr_m][:, j_m, :], sy)
                    nc.vector.tensor_sub(sryl[pr_m][:, j_m, :], sy,
                                         sryh[pr_m][:, j_m, :])

                def proj_one(wt, m, dst, act, bcol):
                    for n in range(NT):
                        pmm = ps_mm.tile([128, 512], F32, tag="pmm")
                        for a in range(2):
                            for k2 in range(NPR):
                                nc.tensor.matmul(
                                    pmm, wt[:, a, k2, :,
                                            m * 128:(m + 1) * 128],
                                    zsl(k2, a, n),
                                    start=(a == 0 and k2 == 0),
                                    stop=(a == 1 and k2 == NPR - 1),
                                    perf_mode=DR)
                        dsl = dst[:, n * 512:(n + 1) * 512]
                        nc.scalar.activation(dsl, pmm, act,
                                             bias=col(bcol + m), scale=INV)

                for m in (range(NCB) if KPHASES >= 2 else ()):
                    ek = p_kvs.tile([128, T], BF16, tag="ek", name=f"ek{m}")
                    vv = p_kvs.tile([128, T], BF16, tag="vv", name=f"vv{m}")
                    sr = p_kvs.tile([128, T], BF16, tag="sr", name=f"sr{m}")
                    proj_one(wk, m, ek, AF.Exp, CBK)
                    proj_one(wv, m, vv, AF.Identity, CBV)
                    if KPHASES >= 3:
                        # front chain starts as soon as k/v epilogues land;
                        # r-proj (only needed by tail, one block later) after
                        A, rec = wkv_front(m, ek, vv)
                    proj_one(wr, m, sr, AF.Sigmoid, CBR)
                    if KPHASES >= 3:
                        wkv_state[m] = (A, rec, sr)
                        if m >= 1:
                            wkv_tail(m - 1, *wkv_state.pop(m - 1))
                if KPHASES >= 3:
                    wkv_tail(NCB - 1, *wkv_state.pop(NCB - 1))

            es_w.close()
            es_z.close()

            # ===== PHASE C: out-proj + residual -> x2, fused LN2 stats =====
            es_x2 = contextlib.ExitStack()
            p_x2 = es_x2.enter_context(tc.tile_pool(name="p_x2", bufs=1))
            x2 = [p_x2.tile([128, T], F32R, tag=f"x2_{cb}", name=f"x2_{cb}")
                  for cb in range(NCB)]
            es_z2 = contextlib.ExitStack()
            p_z2 = es_z2.enter_context(tc.tile_pool(name="p_z2", bufs=1,
                                                    side="right"))
            z2t = [p_z2.tile([128, T + 1], BF16, tag=f"z2_{cb}", name=f"z2_{cb}")
                   for cb in range(NCB)]
            with tc.tile_pool(name="p_xr", bufs=2) as p_xr, \
                 tc.tile_pool(name="p_tmpD", bufs=1) as p_tmpD, \
                 tc.tile_pool(name="p_statD", bufs=1) as p_statD:
              with tc.tile_pool(name="ps_mo", bufs=4, space="PSUM") as ps_mo, \
                   tc.tile_pool(name="ps_st2", bufs=1, space="PSUM") as ps_st2:
                st_half = [(ps_st2.tile([1, 512], F32, tag=f"m{i}", name=f"mD{i}"),
                            ps_st2.tile([1, 512], F32, tag=f"q{i}", name=f"qD{i}"))
                           for i in range(2)]
                for m in (range(NCB) if KPHASES >= 4 else ()):
                    xr = p_xr.tile([128, T], F32, tag="xr")
                    nc.sync.dma_start(out=xr, in_=xT_sp[m][:])
                    for n in range(NT):
                        sl = slice(n * 512, (n + 1) * 512)
                        pmm = ps_mo.tile([128, 512], F32, tag="pmo")
                        first = True
                        for (wi, ss) in ((0, sryh), (1, sryh), (0, sryl)):
                            for k2 in range(NPR):
                                nc.tensor.matmul(
                                    pmm, wo[wi][:, k2, :, m * 128:(m + 1) * 128],
                                    ss[k2][:, :, sl],
                                    start=first,
                                    stop=(wi == 0 and ss is sryl
                                          and k2 == NPR - 1),
                                    perf_mode=DR)
                                first = False
                        nc.vector.scalar_tensor_tensor(
                            out=x2[m][:, sl], in0=pmm, scalar=INV,
                            in1=xr[:, sl], op0=AL.mult, op1=AL.add)
                    if KPHASES >= 5:
                        for i in range(2):
                            sl2 = slice(i * 512, (i + 1) * 512)
                            sq = p_tmpD.tile([128, 512], F32R, tag="sqD", bufs=3)
                            nc.scalar.activation(sq, x2[m].bitcast(F32)[:, sl2],
                                                 AF.Square)
                            nc.tensor.matmul(st_half[i][0], ones1, x2[m][:, sl2],
                                             start=(m == 0), stop=(m == NCB - 1))
                            nc.tensor.matmul(st_half[i][1], ones1, sq,
                                             start=(m == 0), stop=(m == NCB - 1))
                for m in (range(NCB) if KPHASES >= 4 else ()):
                    nc.sync.dma_start(out=x2_sp[m][:], in_=x2[m].bitcast(F32))
                if KPHASES >= 5:
                    meanD = p_statD.tile([1, T], BF16, tag="mean_sb")
                    msqD = p_statD.tile([1, T], BF16, tag="msq_sb")
                    for i in range(2):
                        sl2 = slice(i * 512, (i + 1) * 512)
                        nc.scalar.mul(meanD[:, sl2], st_half[i][0], 1.0 / C)
                        nc.scalar.mul(msqD[:, sl2], st_half[i][1], 1.0 / C)
              # ---- LN2 stats + finish -> z2 (plain bf16, col 0 zero) ----
              if True:
                if KPHASES >= 5:
                    with tc.tile_pool(name="ps_stD", bufs=1,
                                      space="PSUM") as ps_stD:
                        st2 = [(ps_stD.tile([1, 512], F32, tag=f"m2{i}",
                                            name=f"mD2{i}"),
                                ps_stD.tile([1, 512], F32, tag=f"q2{i}",
                                            name=f"qD2{i}")) for i in range(2)]
                        for i in range(2):
                            n = 2 + i
                            sl2 = slice(n * 512, (n + 1) * 512)
                            for cb in range(NCB):
                                sq = p_tmpD.tile([128, 512], F32R, tag="sqD",
                                                 bufs=3)
                                nc.scalar.activation(
                                    sq, x2[cb].bitcast(F32)[:, sl2], AF.Square)
                                nc.tensor.matmul(st2[i][0], ones1,
                                                 x2[cb][:, sl2],
                                                 start=(cb == 0),
                                                 stop=(cb == NCB - 1))
                                nc.tensor.matmul(st2[i][1], ones1, sq,
                                                 start=(cb == 0),
                                                 stop=(cb == NCB - 1))
                            nc.scalar.mul(meanD[:, sl2], st2[i][0], 1.0 / C)
                            nc.scalar.mul(msqD[:, sl2], st2[i][1], 1.0 / C)
                    rstd_b2, mrstd_b2 = ln_finish(meanD, msqD, p_statD, "D")
                    for cb in range(NCB):
                        nc.vector.memset(z2t[cb][:, 0:1], 0.0)
                    for n in range(NT):
                        sl = slice(n * 512, (n + 1) * 512)
                        for cb in range(NCB):
                            zt = p_tmpD.tile([128, 512], F32, tag="zt2s", bufs=3)
                            nc.vector.tensor_mul(zt, x2[cb].bitcast(F32)[:, sl],
                                                 rstd_b2[:, sl])
                            dst = z2t[cb][:, 1 + n * 512:1 + (n + 1) * 512]
                            if cb % 2 == 0:
                                nc.vector.tensor_sub(dst, zt, mrstd_b2[:, sl])
                            else:
                                nc.gpsimd.tensor_sub(dst, zt, mrstd_b2[:, sl])
            es_x2.close()
            es_sry.close()
            es_wo.close()

            # FFN weights: fwv hi/lo resident fp8; fwr till srf; fwk streamed
            es_fw = contextlib.ExitStack()
            p_fw = es_fw.enter_context(tc.tile_pool(name="p_fw", bufs=1))
            fwv = [p_fw.tile([128, 16, 2, 1024], FP8, tag=f"fwv{i}",
                             name=f"fwv{i}") for i in range(2)]
            if KPHASES >= 5:
                nc.sync.dma_start(out=fwv[0], in_=fwv_d[0])
                nc.sync.dma_start(out=fwv[1], in_=fwv_d[1])

            # ============ PHASE E: xf lerp (f_tmk == f_tmr), fWr -> srf ========
            es_xf = contextlib.ExitStack()
            p_xf = es_xf.enter_context(tc.tile_pool(name="p_xf", bufs=1))
            xfh = [p_xf.tile([128, 2, T], FP8, tag=f"xfh{pr}", name=f"xfh{pr}")
                   for pr in range(NPR)]
            xfl = [p_xf.tile([128, 2, T], FP8, tag=f"xfl{pr}", name=f"xfl{pr}")
                   for pr in range(NPR)]
            with tc.tile_pool(name="p_te", bufs=3) as p_te:
                for cb in (range(NCB) if KPHASES >= 6 else ()):
                    pr, j = cb // 2, cb % 2
                    t1 = p_te.tile([128, T], BF16, tag="t1")
                    nc.scalar.mul(t1, z2t[cb][:, 0:T], col(CFT1 + cb))
                    xfb = p_te.tile([128, T], BF16, tag="xfb")
                    nc.vector.scalar_tensor_tensor(
                        out=xfb, in0=z2t[cb][:, 1:T + 1],
                        scalar=col(CFT + cb), in1=t1, op0=AL.mult, op1=AL.add)
                    nc.scalar.copy(xfh[pr][:, j, :], xfb)
                    nc.gpsimd.tensor_sub(xfl[pr][:, j, :], xfb, xfh[pr][:, j, :])
            es_z2.close()

            es_srf = contextlib.ExitStack()
            p_srf = es_srf.enter_context(tc.tile_pool(name="p_srf", bufs=1))
            srf = [p_srf.tile([128, T], FP8, tag=f"srf{m}", name=f"srf{m}")
                   for m in range(NCB)]
            with tc.tile_pool(name="p_fwr", bufs=1) as p_fwr, \
                 tc.tile_pool(name="ps_fr", bufs=4, space="PSUM") as ps_fr:
                fwr = [p_fwr.tile([128, 4, 2, 1024], FP8, tag=f"fwr{i}",
                                  name=f"fwr{i}") for i in range(2)]
                if KPHASES >= 6:
                    nc.sync.dma_start(out=fwr[0], in_=fwr_d[0])
                    nc.sync.dma_start(out=fwr[1], in_=fwr_d[1])
                for m in (range(NCB) if KPHASES >= 6 else ()):
                    for n in range(NT):
                        pmm = ps_fr.tile([128, 512], F32, tag="pfr")
                        first = True
                        for (wi, xs) in ((0, xfh), (1, xfh), (0, xfl)):
                            for k2 in range(NPR):
                                nc.tensor.matmul(
                                    pmm, fwr[wi][:, k2, :, m * 128:(m + 1) * 128],
                                    xs[k2][:, :, n * 512:(n + 1) * 512],
                                    start=first,
                                    stop=(wi == 0 and xs is xfl and k2 == NPR - 1),
                                    perf_mode=DR)
                                first = False
                        nc.scalar.activation(srf[m][:, n * 512:(n + 1) * 512],
                                             pmm, AF.Sigmoid, bias=col(CBFR + m),
                                             scale=INV)

            # ============ PHASE F: FFN k/v matmuls + output ============
            # 3-pass residual fp8: W*x ~ Wh*xh + Wl*xh + Wh*xl
            with tc.tile_pool(name="p_fwkg", bufs=2) as p_fwkg, \
                 tc.tile_pool(name="p_kk", bufs=1) as p_kk, \
                 tc.tile_pool(name="p_rl", bufs=4) as p_rl, \
                 tc.tile_pool(name="p_x2c", bufs=3) as p_x2c, \
                 tc.tile_pool(name="p_fin", bufs=2) as p_fin, \
                 tc.tile_pool(name="p_ost", bufs=1) as p_ost, \
                 tc.tile_pool(name="ps_fk", bufs=3, space="PSUM") as ps_fk, \
                 tc.tile_pool(name="ps_fo", bufs=2, space="PSUM") as ps_fo, \
                 tc.tile_pool(name="ps_ot", bufs=3, space="PSUM") as ps_ot:
                for n in (range(NT) if KPHASES >= 7 else ()):
                    sl = slice(n * 512, (n + 1) * 512)
                    kkh = [p_kk.tile([128, 2, 512], FP8, tag=f"kkh{hp}",
                                     name=f"kkh{hp}_{n}") for hp in range(NHP)]
                    kkl = [p_kk.tile([128, 2, 512], FP8, tag=f"kkl{hp}",
                                     name=f"kkl{hp}_{n}") for hp in range(NHP)]
                    for g in range(8):
                        fg = [p_fwkg.tile([128, 4, 2, 512], FP8, tag=f"fwkg{i}",
                                          name=f"fwkg{i}_{n}_{g}")
                              for i in range(2)]
                        nc.sync.dma_start(out=fg[0], in_=fwk_d[0, g])
                        nc.sync.dma_start(out=fg[1], in_=fwk_d[1, g])
                        for i in range(4):
                            hb = g * 4 + i
                            hp, jh = hb // 2, hb % 2
                            pkk = ps_fk.tile([128, 512], F32, tag="pkk")
                            first = True
                            for (wi, xs) in ((0, xfh), (1, xfh), (0, xfl)):
                                for k2 in range(NPR):
                                    nc.tensor.matmul(
                                        pkk,
                                        fg[wi][:, k2, :, i * 128:(i + 1) * 128],
                                        xs[k2][:, :, sl],
                                        start=first,
                                        stop=(wi == 0 and xs is xfl
                                              and k2 == NPR - 1),
                                        perf_mode=DR)
                                    first = False
                            rl = p_rl.tile([128, 512], BF16, tag="rl")
                            if hb % 2 == 0:
                                nc.scalar.activation(rl, pkk, AF.Relu,
                                                     bias=col(CBFK + hb),
                                                     scale=INV)
                            else:
                                nc.vector.tensor_scalar(
                                    out=rl, in0=pkk, scalar1=INV,
                                    scalar2=0.0, op0=AL.mult, op1=AL.max)
                            t2 = p_rl.tile([128, 512], BF16, tag="t2")
                            nc.vector.tensor_mul(t2, rl, rl)
                            dh = kkh[hp][:, jh, :]
                            if hb % 2 == 0:
                                nc.scalar.copy(dh, t2)
                            else:
                                nc.vector.tensor_copy(out=dh, in_=t2)
                            nc.gpsimd.tensor_sub(kkl[hp][:, jh, :], t2, dh)
                    osts = [p_ost.tile([128, C], F32, tag=f"ost{j}",
                                       name=f"ost{n}_{j}") for j in range(4)]
                    for m in range(NCB):
                        po = ps_fo.tile([128, 512], F32, tag="po")
                        first = True
                        for (wi, ks) in ((0, kkh), (1, kkh), (0, kkl)):
                            for hp in range(NHP):
                                nc.tensor.matmul(
                                    po, fwv[wi][:, hp, :, m * 128:(m + 1) * 128],
                                    ks[hp],
                                    start=first,
                                    stop=(wi == 0 and ks is kkl
                                          and hp == NHP - 1),
                                    perf_mode=DR)
                                first = False
                        x2c = p_x2c.tile([128, 512], F32, tag="x2c")
                        nc.sync.dma_start(out=x2c, in_=x2_sp[m][:, sl])
                        rkv = p_fin.tile([128, 512], F32, tag="rkv")
                        nc.vector.scalar_tensor_tensor(
                            out=rkv, in0=po, scalar=INV, in1=srf[m][:, sl],
                            op0=AL.mult, op1=AL.mult)
                        fin = p_fin.tile([128, 512], F32, tag="fin")
                        if m % 2 == 0:
                            nc.gpsimd.tensor_add(fin, rkv, x2c)
                        else:
                            nc.vector.tensor_add(fin, rkv, x2c)
                        for j in range(4):
                            pt = ps_ot.tile([128, 128], F32, tag="ptr")
                            nc.tensor.transpose(pt, fin[:, j * 128:(j + 1) * 128],
                                                ident)
                            dst = osts[j][:, m * 128:(m + 1) * 128]
                            if (m + j) % 2 == 0:
                                nc.scalar.copy(dst, pt)
                            else:
                                nc.vector.tensor_copy(out=dst, in_=pt)
                    for j in range(4):
                        tb = n * 4 + j
                        nc.sync.dma_start(out=out_d[tb * 128:(tb + 1) * 128, :],
                                          in_=osts[j])
            es_srf.close()
            es_xf.close()
            es_fw.close()
    nc.finalize()
    return nc


_PROG = None


def _get_prog():
    global _PROG
    if _PROG is None:
        nc = bacc.Bacc()
        _PROG = _emit(nc)
    return _PROG


def _pair_w(WT, M_out):
    """WT: [K_in, M_out] fp8 (lhsT layout) -> [128, K_in//256, 2, M_out] flat."""
    K_in = WT.shape[0]
    npr = K_in // 256
    return np.ascontiguousarray(
        WT.reshape(npr, 2, 128, M_out).transpose(2, 0, 1, 3).reshape(128, -1))


def _q8_hl(WT):
    """WT f32 (pre-scaled by WS) -> (hi, lo) fp8 arrays."""
    f8 = ml_dtypes.float8_e4m3
    Ws = np.asarray(WT, np.float32) * np.float32(WS)
    assert np.abs(Ws).max() < 230.0
    hi = Ws.astype(f8)
    lo = (Ws - hi.astype(np.float32)).astype(f8)
    return hi, lo


def _fwk_hl(WT):
    """WT: [C, HID] -> fp8 [2(hl), 8(g), 128, 4(k2)*2(j)*512]; g = hid cols 512g."""
    hi, lo = _q8_hl(WT)
    out = []
    for W8 in (hi, lo):
        # pair layout per group: [128, k2, j, 512]
        Wp = W8.reshape(4, 2, 128, HID)  # [k2, j, c128, h]
        out.append(np.stack(
            [np.ascontiguousarray(
                Wp[:, :, :, g * 512:(g + 1) * 512].transpose(2, 0, 1, 3)
                .reshape(128, -1)) for g in range(8)]))
    return np.ascontiguousarray(np.stack(out))


def _fwv_hl(WT):
    """WT: [HID, C] -> fp8 [2(hl), 128, 16*2*1024] pair layout."""
    hi, lo = _q8_hl(WT)
    return np.ascontiguousarray(np.stack([_pair_w(W8, C) for W8 in (hi, lo)]))


def _q8s(W):
    """Scale by WS, quantize to fp8e4 (checks range)."""
    f8 = ml_dtypes.float8_e4m3
    Ws = np.asarray(W, np.float32) * np.float32(WS)
    assert np.abs(Ws).max() < 230.0, "weight scale overflow"
    return Ws.astype(f8)


def _prep_inputs(x, ln1_g, ln1_b, ln2_g, ln2_b, time_decay, time_first,
                 tmk, tmv, tmr, Wk, Wv, Wr, Wo, f_tmk, f_tmr, fWk, fWr, fWv):
    f32 = np.float32
    x = np.asarray(x, f32)
    g1 = np.asarray(ln1_g, f32); b1 = np.asarray(ln1_b, f32)
    g2 = np.asarray(ln2_g, f32); b2 = np.asarray(ln2_b, f32)
    td = np.asarray(time_decay, np.float64); tf = np.asarray(time_first, np.float64)
    tmk = np.asarray(tmk, f32).reshape(C); tmv = np.asarray(tmv, f32).reshape(C)
    tmr = np.asarray(tmr, f32).reshape(C)
    ftmk = np.asarray(f_tmk, f32).reshape(C); ftmr = np.asarray(f_tmr, f32).reshape(C)
    assert np.array_equal(ftmk, ftmr), "kernel assumes f_tmk == f_tmr"
    Wk = np.asarray(Wk, f32); Wv = np.asarray(Wv, f32); Wr = np.asarray(Wr, f32)
    Wo = np.asarray(Wo, f32); fWk = np.asarray(fWk, f32); fWr = np.asarray(fWr, f32)
    fWv = np.asarray(fWv, f32)

    Wk1 = Wk * g1[None, :]; Wv1 = Wv * g1[None, :]; Wr1 = Wr * g1[None, :]
    bk = Wk @ b1; bv = Wv @ b1; br = Wr @ b1
    fWk1 = fWk * g2[None, :]; fWr1 = fWr * g2[None, :]
    bfk = fWk @ b2; bfr = fWr @ b2
    assert np.allclose(bfk, 0.0), "kernel assumes zero ln2 beta for relu path"

    wbar = np.exp(-np.exp(td)).astype(f32)
    eu = np.exp(tf).astype(f32)

    def packc(v):
        return np.asarray(v, f32).reshape(-1, 128).T

    cst = np.zeros((128, NCOLS), f32)
    cst[:, CW:CW + 8] = packc(wbar)
    cst[:, CEU:CEU + 8] = packc(eu)
    cst[:, CBK:CBK + 8] = packc(bk)
    cst[:, CBV:CBV + 8] = packc(bv)
    cst[:, CBR:CBR + 8] = packc(br)
    cst[:, CFT:CFT + 8] = packc(ftmk)
    cst[:, CFT1:CFT1 + 8] = packc(1 - ftmk)
    cst[:, CBFR:CBFR + 8] = packc(bfr)
    cst[:, CEPS] = EPS
    cst[:, CBFK:CBFK + 32] = packc(bfk)

    def lerp_pair(W1, tm):
        # [128, 2(ab), 4(k2), 2(j), 1024] flat; a=0: W*tm, a=1: W*(1-tm)
        Wa = _pair_w(_q8s((W1 * tm[None, :]).T), C)
        Wb = _pair_w(_q8s((W1 * (1 - tm)[None, :]).T), C)
        return np.ascontiguousarray(
            np.stack([Wa.reshape(128, 4, 2, 1024),
                      Wb.reshape(128, 4, 2, 1024)], axis=1).reshape(128, -1))

    shared = {
        "wk": lerp_pair(Wk1, tmk),
        "wv": lerp_pair(Wv1, tmv),
        "wr": lerp_pair(Wr1, tmr),
        "wo": _fwv_hl(Wo.T),
        "fwr": _fwv_hl(fWr1.T),
        "fwk": _fwk_hl(fWk1.T),
        "fwv": _fwv_hl(fWv.T),
        "cst": cst,
        "ones1": np.ones((128, 1), f32),
        "onesb": np.ones((1, 128), ml_dtypes.bfloat16),
        "ident": np.eye(128, dtype=f32),
    }
    in_maps = [dict(shared, x=np.ascontiguousarray(x[b])) for b in range(B)]
    return in_maps


def _run(in_maps, trace=False, **kw):
    nc = _get_prog()
    res = run_bass_kernel_spmd(nc, in_maps, core_ids=list(range(B)), trace=trace,
                               **kw)
    out = np.stack([np.asarray(res.results[b]["out"]) for b in range(B)], axis=0)
    return out.astype(np.float32), res


def kernel(*a, **kw):
    out, _ = _run(_prep_inputs(*a, **kw))
    return out


if __name__ == "__main__":
    _get_prog()
    print("program built ok")


# revision 9
# speedup vs baseline: 1.0982x; 1.0046x over previous
"""RWKV-4 block (nn_Block_5669356833485) Trainium2 Bass kernel.

B=8, T=2048, C=1024, HID=4096. B-sharded across 8 NeuronCores (1 batch/core).
Feature-major layout [C-partitions, T-free].

fp8e4 DoubleRow matmuls (256-wide contraction, 0.5 cyc/row). Weights are
host-scaled by 128 before fp8 quantization (their natural ~0.02 magnitude
falls in e4m3's subnormal range) and unscaled in the matmul epilogues.
Time-mix lerps are folded into the matmuls by doubling the contraction
against z and shifted-z (z pair tiles [128, 2, 2064], data offset 16, pair
stride %16==0 per DoubleRow requirements). LN stats are pipelined into the
producing loops; WKV (bf16 scans, fp32 state) interleaves per channel block
with the projections. ek/v/sr/srf stay in SBUF; only xT and x2 round-trip
through DRAM for the residual adds.
Self-contained: hardcodes shapes; no sibling imports.
"""
import os
import sys
sys.path.insert(0, '/opt/trn_rl_repo')

KPHASES = int(os.environ.get("KPHASES", "99"))

import numpy as np
import ml_dtypes

import concourse.bass as bass
from concourse import bacc
import concourse.mybir as mybir
import concourse.tile as tile
from concourse.bass_utils import run_bass_kernel_spmd

F32 = mybir.dt.float32
F32R = mybir.dt.float32r
BF16 = mybir.dt.bfloat16
FP8 = mybir.dt.float8e4
AL = mybir.AluOpType
AF = mybir.ActivationFunctionType
DR = mybir.MatmulPerfMode.DoubleRow

B, T, C, HID = 8, 2048, 1024, 4096
NCB = C // 128          # 8 channel blocks
NPR = NCB // 2          # 4 channel pair-blocks
NHB = HID // 128        # 32 hidden blocks
NHP = NHB // 2          # 16 hidden pair-blocks
NT = T // 512           # 4 n-slices of 512
NTB = T // 128          # 16 token blocks
PAD = 16                # z pair tiles: [128, 2, PAD+T]; pair stride %16==0
TP = T + PAD
EPS = 1e-5
WS = 128.0              # weight pre-quantization scale
INV = 1.0 / WS

# cst columns (per 128-partition, indexed by block)
CW = 0        # wbar          [0:8)   by cb
CEU = 8       # exp(tf)       [8:16)  by cb
CBK = 16      # bk            [16:24) by m
CBV = 24      # bv            [24:32) by m
CBR = 32      # br            [32:40) by m
CFT = 40      # ftmk          [40:48) by cb
CFT1 = 48     # 1-ftmk        [48:56) by cb
CBFR = 56     # bfr           [56:64) by m
CEPS = 64     # eps           col 64
CBFK = 72     # bfk           [72:104) by hb
NCOLS = 104


def _emit(nc):
    # ---------------- DRAM I/O ----------------
    x_d = nc.declare_dram_parameter("x", [T, C], F32, isOutput=False)
    wk_d = nc.declare_dram_parameter("wk", [128, 2 * 4 * 2 * 1024], FP8, isOutput=False)
    wv_d = nc.declare_dram_parameter("wv", [128, 2 * 4 * 2 * 1024], FP8, isOutput=False)
    wr_d = nc.declare_dram_parameter("wr", [128, 2 * 4 * 2 * 1024], FP8, isOutput=False)
    wo_d = nc.declare_dram_parameter("wo", [2, 128, 4 * 2 * 1024], FP8, isOutput=False)
    fwr_d = nc.declare_dram_parameter("fwr", [2, 128, 4 * 2 * 1024], FP8, isOutput=False)
    fwk_d = nc.declare_dram_parameter("fwk", [2, 8, 128, 4 * 2 * 512], FP8, isOutput=False)
    fwv_d = nc.declare_dram_parameter("fwv", [2, 128, 16 * 2 * 1024], FP8, isOutput=False)
    cst_d = nc.declare_dram_parameter("cst", [128, NCOLS], F32, isOutput=False)
    ones1_d = nc.declare_dram_parameter("ones1", [128, 1], F32R, isOutput=False)
    onesb_d = nc.declare_dram_parameter("onesb", [1, 128], BF16, isOutput=False)
    ident_d = nc.declare_dram_parameter("ident", [128, 128], F32, isOutput=False)
    out_d = nc.declare_dram_parameter("out", [T, C], F32, isOutput=True)

    # DRAM scratch (per-cb granularity for fine deps)
    xT_sp = [nc.dram_tensor(f"xT_sp{i}", [128, T], F32) for i in range(NCB)]
    x2_sp = [nc.dram_tensor(f"x2_sp{i}", [128, T], F32) for i in range(NCB)]

    import contextlib

    with tile.TileContext(nc, pool_alloc_mode="queue") as tc:
        with tc.tile_pool(name="pc", bufs=1) as pc:
            cst = pc.tile([128, NCOLS], F32)
            nc.sync.dma_start(out=cst, in_=cst_d[:])
            ones1 = pc.tile([128, 1], F32R)
            nc.sync.dma_start(out=ones1, in_=ones1_d[:])
            onesb = pc.tile([1, 128], BF16)
            nc.sync.dma_start(out=onesb, in_=onesb_d[:])
            ident = pc.tile([128, 128], F32)
            nc.sync.dma_start(out=ident, in_=ident_d[:])
            ones_bf = pc.tile([128, T], BF16)
            nc.vector.memset(ones_bf, 1.0)

            def col(j):
                return cst[:, j:j + 1]

            # ---- incremental LN stats: two [1, T] psum tiles ----
            def ln_contrib(stat_ps, pool_tmp, src_f32r, cb, sl, tag):
                """Add channel-block cb's contribution for column slice sl."""
                mean_ps, msq_ps = stat_ps
                w = sl.stop - sl.start
                sq = pool_tmp.tile([128, w], F32R, tag=tag, bufs=3)
                nc.scalar.activation(sq, src_f32r.bitcast(F32)[:, sl], AF.Square)
                nc.tensor.matmul(mean_ps[:, sl], ones1, src_f32r[:, sl],
                                 start=(cb == 0), stop=(cb == NCB - 1))
                nc.tensor.matmul(msq_ps[:, sl], ones1, sq,
                                 start=(cb == 0), stop=(cb == NCB - 1))

            def ln_to_sbuf(stat_ps, pool_stat):
                mean_ps, msq_ps = stat_ps
                mean = pool_stat.tile([1, T], BF16, tag="mean_sb")
                msq = pool_stat.tile([1, T], BF16, tag="msq_sb")
                nc.scalar.mul(mean, mean_ps, 1.0 / C)
                nc.scalar.mul(msq, msq_ps, 1.0 / C)
                return mean, msq

            def ln_finish(mean, msq, pool_stat, uid):
                var = pool_stat.tile([1, T], BF16, tag="var_sb")
                nc.vector.tensor_mul(var, mean, mean)
                nc.vector.tensor_sub(var, msq, var)
                lnv = pool_stat.tile([1, T], BF16, tag="msq_sb", name=f"lnv{uid}")
                nc.scalar.activation(lnv, var, AF.Ln,
                                     bias=cst[0:1, CEPS:CEPS + 1], scale=1.0)
                rstd = pool_stat.tile([1, T], BF16, tag="var_sb", name=f"rstd{uid}")
                nc.scalar.activation(rstd, lnv, AF.Exp, bias=0.0, scale=-0.5)
                mrstd = pool_stat.tile([1, T], BF16, tag="mrstd_sb")
                nc.vector.tensor_mul(mrstd, mean, rstd)
                rstd_b = pool_stat.tile([128, T], BF16, tag="rstd_b")
                mrstd_b = pool_stat.tile([128, T], BF16, tag="mrstd_b")
                with tc.tile_pool(name=f"ps_bc{uid}", bufs=2, space="PSUM") as ps_bc:
                    for (src_s, dst) in ((rstd, rstd_b), (mrstd, mrstd_b)):
                        for n in range(NT):
                            sl = slice(n * 512, (n + 1) * 512)
                            bc = ps_bc.tile([128, 512], F32, tag="bc")
                            nc.tensor.matmul(bc, onesb, src_s[:, sl],
                                             start=True, stop=True)
                            if n % 2 == 0:
                                nc.scalar.copy(dst[:, sl], bc)
                            else:
                                nc.vector.tensor_copy(out=dst[:, sl], in_=bc)
                return rstd_b, mrstd_b

            # z pair tiles live through phase B (attention)
            es_z = contextlib.ExitStack()
            p_z = es_z.enter_context(tc.tile_pool(name="p_z", bufs=1, side="right"))
            z8 = [p_z.tile([128, 2, TP], FP8, tag=f"z{pr}", name=f"z{pr}")
                  for pr in range(NPR)]
            for pr in range(NPR):
                nc.vector.memset(z8[pr][:, :, 0:PAD], 0.0)

            # attention weights: prefetch during phase A
            es_wo = contextlib.ExitStack()
            p_wo = es_wo.enter_context(tc.tile_pool(name="p_wo", bufs=1))
            wo = [p_wo.tile([128, 4, 2, 1024], FP8, tag=f"wo{i}",
                            name=f"wo{i}") for i in range(2)]
            es_w = contextlib.ExitStack()
            p_w = es_w.enter_context(tc.tile_pool(name="p_w", bufs=1, side="right"))
            wk = p_w.tile([128, 2, 4, 2, 1024], FP8, tag="wk")
            wv = p_w.tile([128, 2, 4, 2, 1024], FP8, tag="wv")
            wr = p_w.tile([128, 2, 4, 2, 1024], FP8, tag="wr")

            # ================= PHASE A: load, transpose, LN1, z =================
            with tc.tile_pool(name="p_xT", bufs=1) as p_xT:
                xT = [p_xT.tile([128, T], F32R, tag=f"xT{cb}", name=f"xT{cb}")
                      for cb in range(NCB)]
                with tc.tile_pool(name="p_tmpA", bufs=1) as p_tmpA, \
                     tc.tile_pool(name="p_statA", bufs=1) as p_statA:
                    meanA = p_statA.tile([1, T], BF16, tag="mean_sb")
                    msqA = p_statA.tile([1, T], BF16, tag="msq_sb")
                    with tc.tile_pool(name="p_ld", bufs=3) as p_ld, \
                         tc.tile_pool(name="ps_stA", bufs=2,
                                      space="PSUM") as ps_stA, \
                         tc.tile_pool(name="ps_tr", bufs=4, space="PSUM") as ps_tr:
                        for tb in range(NTB):
                            xt = p_ld.tile([128, C], F32, tag="xtok")
                            nc.sync.dma_start(out=xt,
                                              in_=x_d[tb * 128:(tb + 1) * 128, :])
                            for cb in range(NCB):
                                pt = ps_tr.tile([128, 128], F32, tag="tr")
                                nc.tensor.transpose(
                                    pt, xt[:, cb * 128:(cb + 1) * 128], ident)
                                dst = xT[cb][:, tb * 128:(tb + 1) * 128]
                                if (tb + cb) % 2 == 0:
                                    nc.scalar.copy(dst, pt)
                                else:
                                    nc.vector.tensor_copy(out=dst, in_=pt)
                            if tb % 4 == 3:
                                n = tb // 4
                                sl = slice(n * 512, (n + 1) * 512)
                                mp = ps_stA.tile([1, 512], F32, tag="mA",
                                                 name=f"mA{n}")
                                qp = ps_stA.tile([1, 512], F32, tag="qA",
                                                 name=f"qA{n}")
                                for cb in range(NCB):
                                    sq = p_tmpA.tile([128, 512], F32R,
                                                     tag="sqA", bufs=3)
                                    nc.scalar.activation(
                                        sq, xT[cb].bitcast(F32)[:, sl],
                                        AF.Square)
                                    nc.tensor.matmul(
                                        mp, ones1, xT[cb][:, sl],
                                        start=(cb == 0), stop=(cb == NCB - 1))
                                    nc.tensor.matmul(
                                        qp, ones1, sq,
                                        start=(cb == 0), stop=(cb == NCB - 1))
                                nc.scalar.mul(meanA[:, sl], mp, 1.0 / C)
                                nc.scalar.mul(msqA[:, sl], qp, 1.0 / C)
                    # attention weight loads: small chunks so queue-jumps
                    # ahead of x-loads steal only ~0.7us DMA slots
                    for (wt_, wd_) in ((wk, wk_d), (wv, wv_d), (wr, wr_d)):
                        for a_ in range(2):
                            for k2_ in range(4):
                                nc.sync.dma_start(
                                    out=wt_[:, a_, k2_],
                                    in_=wd_[:, (a_ * 4 + k2_) * 2048:
                                            (a_ * 4 + k2_ + 1) * 2048])
                    for i_ in range(2):
                        for k2_ in range(4):
                            nc.sync.dma_start(
                                out=wo[i_][:, k2_],
                                in_=wo_d[i_, :, k2_ * 2048:(k2_ + 1) * 2048])
                    for cb in range(NCB):
                        nc.sync.dma_start(out=xT_sp[cb][:], in_=xT[cb].bitcast(F32))
                    rstd_b, mrstd_b = ln_finish(meanA, msqA, p_statA, "A")
                    # per-slice so phase B's first matmuls start after the
                    # first 512 tokens of z are ready, not the full rows
                    for n in range(NT):
                        sl = slice(n * 512, (n + 1) * 512)
                        for cb in range(NCB):
                            pr, j = cb // 2, cb % 2
                            zt = p_tmpA.tile([128, 512], F32, tag="zts", bufs=3)
                            nc.vector.tensor_mul(zt, xT[cb].bitcast(F32)[:, sl],
                                                 rstd_b[:, sl])
                            dst = z8[pr][:, j, PAD + n * 512:PAD + (n + 1) * 512]
                            if cb % 2 == 0:
                                nc.vector.tensor_sub(dst, zt, mrstd_b[:, sl])
                            else:
                                nc.gpsimd.tensor_sub(dst, zt, mrstd_b[:, sl])

            # ============ PHASE B: k/v/r projections + WKV per m ============
            es_sry = contextlib.ExitStack()
            p_sry = es_sry.enter_context(tc.tile_pool(name="p_sry", bufs=1))
            sryh = [p_sry.tile([128, 2, T], FP8, tag=f"sryh{pr}", name=f"sryh{pr}")
                    for pr in range(NPR)]
            sryl = [p_sry.tile([128, 2, T], FP8, tag=f"sryl{pr}", name=f"sryl{pr}")
                    for pr in range(NPR)]

            def zsl(k2, a, n):
                # a=0: current tokens; a=1: shifted by one
                lo = PAD - a + n * 512
                return z8[k2][:, :, lo:lo + 512]

            with tc.tile_pool(name="p_kvs", bufs=2) as p_kvs, \
                 tc.tile_pool(name="p_wt", bufs=2) as p_wt, \
                 tc.tile_pool(name="ps_mm", bufs=8, space="PSUM") as ps_mm:
                wkv_state = {}

                def wkv_front(m, ek, vv):
                    # scanB first: depends only on ek (k epilogues), so DVE can
                    # start while Act still runs v/r epilogues. ekv on DVE keeps
                    # the ekv->scanA handoff on-engine (no cross-engine sem).
                    wrow = p_wt.tile([128, T], BF16, tag="wrow", name=f"wr{m}")
                    nc.vector.tensor_scalar(out=wrow, in0=ones_bf,
                                            scalar1=col(CW + m),
                                            scalar2=None, op0=AL.mult)
                    A = p_wt.tile([128, T + 1], BF16, tag="A", name=f"A{m}")
                    Bt = p_wt.tile([128, T + 1], BF16, tag="B", name=f"B{m}")
                    nc.vector.memset(Bt[:, 0:1], 0.0)
                    nc.vector.tensor_tensor_scan(
                        out=Bt[:, 1:T + 1], data0=wrow, data1=ek,
                        initial=0.0, op0=AL.mult, op1=AL.add)
                    ekv = p_wt.tile([128, T], BF16, tag="ekv", name=f"ekv{m}")
                    nc.vector.tensor_mul(ekv, ek, vv)
                    nc.vector.memset(A[:, 0:1], 0.0)
                    nc.vector.tensor_tensor_scan(
                        out=A[:, 1:T + 1], data0=wrow, data1=ekv,
                        initial=0.0, op0=AL.mult, op1=AL.add)
                    nc.vector.scalar_tensor_tensor(
                        out=Bt[:, 0:T], in0=ek, scalar=col(CEU + m),
                        in1=Bt[:, 0:T], op0=AL.mult, op1=AL.add)
                    rec = p_wt.tile([128, T], BF16, tag="rec", name=f"rec{m}")
                    with nc.allow_low_precision(reason="wkv ratio bf16"):
                        nc.vector.reciprocal(rec, Bt[:, 0:T])
                    nc.vector.scalar_tensor_tensor(
                        out=A[:, 0:T], in0=ekv, scalar=col(CEU + m),
                        in1=A[:, 0:T], op0=AL.mult, op1=AL.add)
                    return A, rec

                def wkv_tail(m, A, rec, sr):
                    pr_m, j_m = m // 2, m % 2
                    y = p_wt.tile([128, T], BF16, tag="y", name=f"y{m}")
                    nc.gpsimd.tensor_mul(y, A[:, 0:T], rec)
                    sy = p_wt.tile([128, T], BF16, tag="sy", name=f"sy{m}")
                    nc.gpsimd.tensor_mul(sy, y, sr)
                    nc.scalar.copy(sryh[pr_m][:, j_m, :], sy)
                    nc.vector.tensor_sub(sryl[pr_m][:, j_m, :], sy,
                                         sryh[pr_m][:, j_m, :])

                def proj_one(wt, m, dst, act, bcol):
                    for n in range(NT):
                        pmm = ps_mm.tile([128, 512], F32, tag="pmm")
                        for a in range(2):
                            for k2 in range(NPR):
                                nc.tensor.matmul(
                                    pmm, wt[:, a, k2, :,
                                            m * 128:(m + 1) * 128],
                                    zsl(k2, a, n),
                                    start=(a == 0 and k2 == 0),
                                    stop=(a == 1 and k2 == NPR - 1),
                                    perf_mode=DR)
                        dsl = dst[:, n * 512:(n + 1) * 512]
                        nc.scalar.activation(dsl, pmm, act,
                                             bias=col(bcol + m), scale=INV)

                for m in (range(NCB) if KPHASES >= 2 else ()):
                    ek = p_kvs.tile([128, T], BF16, tag="ek", name=f"ek{m}")
                    vv = p_kvs.tile([128, T], BF16, tag="vv", name=f"vv{m}")
                    sr = p_kvs.tile([128, T], BF16, tag="sr", name=f"sr{m}")
                    proj_one(wk, m, ek, AF.Exp, CBK)
                    proj_one(wv, m, vv, AF.Identity, CBV)
                    if KPHASES >= 3:
                        # front chain starts as soon as k/v epilogues land;
                        # r-proj (only needed by tail, one block later) after
                        A, rec = wkv_front(m, ek, vv)
                    proj_one(wr, m, sr, AF.Sigmoid, CBR)
                    if KPHASES >= 3:
                        wkv_state[m] = (A, rec, sr)
                        if m >= 1:
                            wkv_tail(m - 1, *wkv_state.pop(m - 1))
                if KPHASES >= 3:
                    wkv_tail(NCB - 1, *wkv_state.pop(NCB - 1))

            es_w.close()
            es_z.close()

            # ===== PHASE C: out-proj + residual -> x2, fused LN2 stats =====
            es_x2 = contextlib.ExitStack()
            p_x2 = es_x2.enter_context(tc.tile_pool(name="p_x2", bufs=1))
            x2 = [p_x2.tile([128, T], F32R, tag=f"x2_{cb}", name=f"x2_{cb}")
                  for cb in range(NCB)]
            es_z2 = contextlib.ExitStack()
            p_z2 = es_z2.enter_context(tc.tile_pool(name="p_z2", bufs=1,
                                                    side="right"))
            z2t = [p_z2.tile([128, T + 1], BF16, tag=f"z2_{cb}", name=f"z2_{cb}")
                   for cb in range(NCB)]
            with tc.tile_pool(name="p_xr", bufs=2) as p_xr, \
                 tc.tile_pool(name="p_tmpD", bufs=1) as p_tmpD, \
                 tc.tile_pool(name="p_statD", bufs=1) as p_statD:
              with tc.tile_pool(name="ps_mo", bufs=4, space="PSUM") as ps_mo, \
                   tc.tile_pool(name="ps_st2", bufs=1, space="PSUM") as ps_st2:
                st_half = [(ps_st2.tile([1, 512], F32, tag=f"m{i}", name=f"mD{i}"),
                            ps_st2.tile([1, 512], F32, tag=f"q{i}", name=f"qD{i}"))
                           for i in range(2)]
                for m in (range(NCB) if KPHASES >= 4 else ()):
                    xr = p_xr.tile([128, T], F32, tag="xr")
                    nc.sync.dma_start(out=xr, in_=xT_sp[m][:])
                    for n in range(NT):
                        sl = slice(n * 512, (n + 1) * 512)
                        pmm = ps_mo.tile([128, 512], F32, tag="pmo")
                        first = True
                        for (wi, ss) in ((0, sryh), (1, sryh), (0, sryl)):
                            for k2 in range(NPR):
                                nc.tensor.matmul(
                                    pmm, wo[wi][:, k2, :, m * 128:(m + 1) * 128],
                                    ss[k2][:, :, sl],
                                    start=first,
                                    stop=(wi == 0 and ss is sryl
                                          and k2 == NPR - 1),
                                    perf_mode=DR)
                                first = False
                        nc.vector.scalar_tensor_tensor(
                            out=x2[m][:, sl], in0=pmm, scalar=INV,
                            in1=xr[:, sl], op0=AL.mult, op1=AL.add)
                    if KPHASES >= 5:
                        for i in range(2):
                            sl2 = slice(i * 512, (i + 1) * 512)
                            sq = p_tmpD.tile([128, 512], F32R, tag="sqD", bufs=3)
                            nc.scalar.activation(sq, x2[m].bitcast(F32)[:, sl2],
                                                 AF.Square)
                            nc.tensor.matmul(st_half[i][0], ones1, x2[m][:, sl2],
                                             start=(m == 0), stop=(m == NCB - 1))
                            nc.tensor.matmul(st_half[i][1], ones1, sq,
                                             start=(m == 0), stop=(m == NCB - 1))
                for m in (range(NCB) if KPHASES >= 4 else ()):
                    nc.sync.dma_start(out=x2_sp[m][:], in_=x2[m].bitcast(F32))
                if KPHASES >= 5:
                    meanD = p_statD.tile([1, T], BF16, tag="mean_sb")
                    msqD = p_statD.tile([1, T], BF16, tag="msq_sb")
                    for i in range(2):
                        sl2 = slice(i * 512, (i + 1) * 512)
                        nc.scalar.mul(meanD[:, sl2], st_half[i][0], 1.0 / C)
                        nc.scalar.mul(msqD[:, sl2], st_half[i][1], 1.0 / C)
              # ---- LN2 stats + finish -> z2 (plain bf16, col 0 zero) ----
              if True:
                if KPHASES >= 5:
                    with tc.tile_pool(name="ps_stD", bufs=1,
                                      space="PSUM") as ps_stD:
                        st2 = [(ps_stD.tile([1, 512], F32, tag=f"m2{i}",
                                            name=f"mD2{i}"),
                                ps_stD.tile([1, 512], F32, tag=f"q2{i}",
                                            name=f"qD2{i}")) for i in range(2)]
                        for i in range(2):
                            n = 2 + i
                            sl2 = slice(n * 512, (n + 1) * 512)
                            for cb in range(NCB):
                                sq = p_tmpD.tile([128, 512], F32R, tag="sqD",
                                                 bufs=3)
                                nc.scalar.activation(
                                    sq, x2[cb].bitcast(F32)[:, sl2], AF.Square)
                                nc.tensor.matmul(st2[i][0], ones1,
                                                 x2[cb][:, sl2],
                                                 start=(cb == 0),
                                                 stop=(cb == NCB - 1))
                                nc.tensor.matmul(st2[i][1], ones1, sq,
                                                 start=(cb == 0),
                                                 stop=(cb == NCB - 1))
                            nc.scalar.mul(meanD[:, sl2], st2[i][0], 1.0 / C)
                            nc.scalar.mul(msqD[:, sl2], st2[i][1], 1.0 / C)
                    rstd_b2, mrstd_b2 = ln_finish(meanD, msqD, p_statD, "D")
                    for cb in range(NCB):
                        nc.vector.memset(z2t[cb][:, 0:1], 0.0)
                    for n in range(NT):
                        sl = slice(n * 512, (n + 1) * 512)
                        for cb in range(NCB):
                            zt = p_tmpD.tile([128, 512], F32, tag="zt2s", bufs=3)
                            nc.vector.tensor_mul(zt, x2[cb].bitcast(F32)[:, sl],
                                                 rstd_b2[:, sl])
                            dst = z2t[cb][:, 1 + n * 512:1 + (n + 1) * 512]
                            if cb % 2 == 0:
                                nc.vector.tensor_sub(dst, zt, mrstd_b2[:, sl])
                            else:
                                nc.gpsimd.tensor_sub(dst, zt, mrstd_b2[:, sl])
            es_x2.close()
            es_sry.close()
            es_wo.close()

            # FFN weights: fwv hi/lo resident fp8; fwr till srf; fwk streamed
            es_fw = contextlib.ExitStack()
            p_fw = es_fw.enter_context(tc.tile_pool(name="p_fw", bufs=1))
            fwv = [p_fw.tile([128, 16, 2, 1024], FP8, tag=f"fwv{i}",
                             name=f"fwv{i}") for i in range(2)]
            if KPHASES >= 5:
                nc.sync.dma_start(out=fwv[0], in_=fwv_d[0])
                nc.sync.dma_start(out=fwv[1], in_=fwv_d[1])

            # ============ PHASE E: xf lerp (f_tmk == f_tmr), fWr -> srf ========
            es_xf = contextlib.ExitStack()
            p_xf = es_xf.enter_context(tc.tile_pool(name="p_xf", bufs=1))
            xfh = [p_xf.tile([128, 2, T], FP8, tag=f"xfh{pr}", name=f"xfh{pr}")
                   for pr in range(NPR)]
            xfl = [p_xf.tile([128, 2, T], FP8, tag=f"xfl{pr}", name=f"xfl{pr}")
                   for pr in range(NPR)]
            with tc.tile_pool(name="p_te", bufs=12) as p_te:
                # per-slice so fWr/fWk matmuls start on slice 0 early
                for n in (range(NT) if KPHASES >= 6 else ()):
                    lo = n * 512
                    for cb in range(NCB):
                        pr, j = cb // 2, cb % 2
                        t1 = p_te.tile([128, 512], BF16, tag="t1")
                        nc.scalar.mul(t1, z2t[cb][:, lo:lo + 512],
                                      col(CFT1 + cb))
                        xfb = p_te.tile([128, 512], BF16, tag="xfb")
                        nc.vector.scalar_tensor_tensor(
                            out=xfb, in0=z2t[cb][:, lo + 1:lo + 513],
                            scalar=col(CFT + cb), in1=t1,
                            op0=AL.mult, op1=AL.add)
                        dh = xfh[pr][:, j, lo:lo + 512]
                        nc.scalar.copy(dh, xfb)
                        nc.gpsimd.tensor_sub(xfl[pr][:, j, lo:lo + 512],
                                             xfb, dh)
            es_z2.close()

            es_srf = contextlib.ExitStack()
            p_srf = es_srf.enter_context(tc.tile_pool(name="p_srf", bufs=1))
            srf = [p_srf.tile([128, T], FP8, tag=f"srf{m}", name=f"srf{m}")
                   for m in range(NCB)]
            with tc.tile_pool(name="p_fwr", bufs=1) as p_fwr, \
                 tc.tile_pool(name="ps_fr", bufs=4, space="PSUM") as ps_fr:
                fwr = [p_fwr.tile([128, 4, 2, 1024], FP8, tag=f"fwr{i}",
                                  name=f"fwr{i}") for i in range(2)]
                if KPHASES >= 6:
                    nc.sync.dma_start(out=fwr[0], in_=fwr_d[0])
                    nc.sync.dma_start(out=fwr[1], in_=fwr_d[1])
                for m in (range(NCB) if KPHASES >= 6 else ()):
                    for n in range(NT):
                        pmm = ps_fr.tile([128, 512], F32, tag="pfr")
                        first = True
                        for (wi, xs) in ((0, xfh), (1, xfh), (0, xfl)):
                            for k2 in range(NPR):
                                nc.tensor.matmul(
                                    pmm, fwr[wi][:, k2, :, m * 128:(m + 1) * 128],
                                    xs[k2][:, :, n * 512:(n + 1) * 512],
                                    start=first,
                                    stop=(wi == 0 and xs is xfl and k2 == NPR - 1),
                                    perf_mode=DR)
                                first = False
                        nc.scalar.activation(srf[m][:, n * 512:(n + 1) * 512],
                                             pmm, AF.Sigmoid, bias=col(CBFR + m),
                                             scale=INV)

            # ============ PHASE F: FFN k/v matmuls + output ============
            # 3-pass residual fp8: W*x ~ Wh*xh + Wl*xh + Wh*xl
            with tc.tile_pool(name="p_fwkg", bufs=2) as p_fwkg, \
                 tc.tile_pool(name="p_kk", bufs=1) as p_kk, \
                 tc.tile_pool(name="p_rl", bufs=4) as p_rl, \
                 tc.tile_pool(name="p_x2c", bufs=3) as p_x2c, \
                 tc.tile_pool(name="p_fin", bufs=2) as p_fin, \
                 tc.tile_pool(name="p_ost", bufs=1) as p_ost, \
                 tc.tile_pool(name="ps_fk", bufs=3, space="PSUM") as ps_fk, \
                 tc.tile_pool(name="ps_fo", bufs=2, space="PSUM") as ps_fo, \
                 tc.tile_pool(name="ps_ot", bufs=3, space="PSUM") as ps_ot:
                for n in (range(NT) if KPHASES >= 7 else ()):
                    sl = slice(n * 512, (n + 1) * 512)
                    kkh = [p_kk.tile([128, 2, 512], FP8, tag=f"kkh{hp}",
                                     name=f"kkh{hp}_{n}") for hp in range(NHP)]
                    kkl = [p_kk.tile([128, 2, 512], FP8, tag=f"kkl{hp}",
                                     name=f"kkl{hp}_{n}") for hp in range(NHP)]
                    for g in range(8):
                        fg = [p_fwkg.tile([128, 4, 2, 512], FP8, tag=f"fwkg{i}",
                                          name=f"fwkg{i}_{n}_{g}")
                              for i in range(2)]
                        nc.sync.dma_start(out=fg[0], in_=fwk_d[0, g])
                        nc.sync.dma_start(out=fg[1], in_=fwk_d[1, g])
                        for i in range(4):
                            hb = g * 4 + i
                            hp, jh = hb // 2, hb % 2
                            pkk = ps_fk.tile([128, 512], F32, tag="pkk")
                            first = True
                            for (wi, xs) in ((0, xfh), (1, xfh), (0, xfl)):
                                for k2 in range(NPR):
                                    nc.tensor.matmul(
                                        pkk,
                                        fg[wi][:, k2, :, i * 128:(i + 1) * 128],
                                        xs[k2][:, :, sl],
                                        start=first,
                                        stop=(wi == 0 and xs is xfl
                                              and k2 == NPR - 1),
                                        perf_mode=DR)
                                    first = False
                            rl = p_rl.tile([128, 512], BF16, tag="rl")
                            if hb % 2 == 0:
                                nc.scalar.activation(rl, pkk, AF.Relu,
                                                     bias=col(CBFK + hb),
                                                     scale=INV)
                            else:
                                nc.vector.tensor_scalar(
                                    out=rl, in0=pkk, scalar1=INV,
                                    scalar2=0.0, op0=AL.mult, op1=AL.max)
                            t2 = p_rl.tile([128, 512], BF16, tag="t2")
                            nc.vector.tensor_mul(t2, rl, rl)
                            dh = kkh[hp][:, jh, :]
                            if hb % 2 == 0:
                                nc.scalar.copy(dh, t2)
                            else:
                                nc.vector.tensor_copy(out=dh, in_=t2)
                            nc.gpsimd.tensor_sub(kkl[hp][:, jh, :], t2, dh)
                    osts = [p_ost.tile([128, C], F32, tag=f"ost{j}",
                                       name=f"ost{n}_{j}") for j in range(4)]
                    for m in range(NCB):
                        po = ps_fo.tile([128, 512], F32, tag="po")
                        first = True
                        for (wi, ks) in ((0, kkh), (1, kkh), (0, kkl)):
                            for hp in range(NHP):
                                nc.tensor.matmul(
                                    po, fwv[wi][:, hp, :, m * 128:(m + 1) * 128],
                                    ks[hp],
                                    start=first,
                                    stop=(wi == 0 and ks is kkl
                                          and hp == NHP - 1),
                                    perf_mode=DR)
                                first = False
                        x2c = p_x2c.tile([128, 512], F32, tag="x2c")
                        nc.sync.dma_start(out=x2c, in_=x2_sp[m][:, sl])
                        rkv = p_fin.tile([128, 512], F32, tag="rkv")
                        nc.vector.scalar_tensor_tensor(
                            out=rkv, in0=po, scalar=INV, in1=srf[m][:, sl],
                            op0=AL.mult, op1=AL.mult)
                        fin = p_fin.tile([128, 512], F32, tag="fin")
                        if m % 2 == 0:
                            nc.gpsimd.tensor_add(fin, rkv, x2c)
                        else:
                            nc.vector.tensor_add(fin, rkv, x2c)
                        for j in range(4):
                            pt = ps_ot.tile([128, 128], F32, tag="ptr")
                            nc.tensor.transpose(pt, fin[:, j * 128:(j + 1) * 128],
                                                ident)
                            dst = osts[j][:, m * 128:(m + 1) * 128]
                            if (m + j) % 2 == 0:
                                nc.scalar.copy(dst, pt)
                            else:
                                nc.vector.tensor_copy(out=dst, in_=pt)
                    for j in range(4):
                        tb = n * 4 + j
                        nc.sync.dma_start(out=out_d[tb * 128:(tb + 1) * 128, :],
                                          in_=osts[j])
            es_srf.close()
            es_xf.close()
            es_fw.close()
    nc.finalize()
    return nc


_PROG = None


def _get_prog():
    global _PROG
    if _PROG is None:
        nc = bacc.Bacc()
        _PROG = _emit(nc)
    return _PROG


def _pair_w(WT, M_out):
    """WT: [K_in, M_out] fp8 (lhsT layout) -> [128, K_in//256, 2, M_out] flat."""
    K_in = WT.shape[0]
    npr = K_in // 256
    return np.ascontiguousarray(
        WT.reshape(npr, 2, 128, M_out).transpose(2, 0, 1, 3).reshape(128, -1))


def _q8_hl(WT):
    """WT f32 (pre-scaled by WS) -> (hi, lo) fp8 arrays."""
    f8 = ml_dtypes.float8_e4m3
    Ws = np.asarray(WT, np.float32) * np.float32(WS)
    assert np.abs(Ws).max() < 230.0
    hi = Ws.astype(f8)
    lo = (Ws - hi.astype(np.float32)).astype(f8)
    return hi, lo


def _fwk_hl(WT):
    """WT: [C, HID] -> fp8 [2(hl), 8(g), 128, 4(k2)*2(j)*512]; g = hid cols 512g."""
    hi, lo = _q8_hl(WT)
    out = []
    for W8 in (hi, lo):
        # pair layout per group: [128, k2, j, 512]
        Wp = W8.reshape(4, 2, 128, HID)  # [k2, j, c128, h]
        out.append(np.stack(
            [np.ascontiguousarray(
                Wp[:, :, :, g * 512:(g + 1) * 512].transpose(2, 0, 1, 3)
                .reshape(128, -1)) for g in range(8)]))
    return np.ascontiguousarray(np.stack(out))


def _fwv_hl(WT):
    """WT: [HID, C] -> fp8 [2(hl), 128, 16*2*1024] pair layout."""
    hi, lo = _q8_hl(WT)
    return np.ascontiguousarray(np.stack([_pair_w(W8, C) for W8 in (hi, lo)]))


def _q8s(W):
    """Scale by WS, quantize to fp8e4 (checks range)."""
    f8 = ml_dtypes.float8_e4m3
    Ws = np.asarray(W, np.float32) * np.float32(WS)
    assert np.abs(Ws).max() < 230.0, "weight scale overflow"
    return Ws.astype(f8)


def _prep_inputs(x, ln1_g, ln1_b, ln2_g, ln2_b, time_decay, time_first,
                 tmk, tmv, tmr, Wk, Wv, Wr, Wo, f_tmk, f_tmr, fWk, fWr, fWv):
    f32 = np.float32
    x = np.asarray(x, f32)
    g1 = np.asarray(ln1_g, f32); b1 = np.asarray(ln1_b, f32)
    g2 = np.asarray(ln2_g, f32); b2 = np.asarray(ln2_b, f32)
    td = np.asarray(time_decay, np.float64); tf = np.asarray(time_first, np.float64)
    tmk = np.asarray(tmk, f32).reshape(C); tmv = np.asarray(tmv, f32).reshape(C)
    tmr = np.asarray(tmr, f32).reshape(C)
    ftmk = np.asarray(f_tmk, f32).reshape(C); ftmr = np.asarray(f_tmr, f32).reshape(C)
    assert np.array_equal(ftmk, ftmr), "kernel assumes f_tmk == f_tmr"
    Wk = np.asarray(Wk, f32); Wv = np.asarray(Wv, f32); Wr = np.asarray(Wr, f32)
    Wo = np.asarray(Wo, f32); fWk = np.asarray(fWk, f32); fWr = np.asarray(fWr, f32)
    fWv = np.asarray(fWv, f32)

    Wk1 = Wk * g1[None, :]; Wv1 = Wv * g1[None, :]; Wr1 = Wr * g1[None, :]
    bk = Wk @ b1; bv = Wv @ b1; br = Wr @ b1
    fWk1 = fWk * g2[None, :]; fWr1 = fWr * g2[None, :]
    bfk = fWk @ b2; bfr = fWr @ b2
    assert np.allclose(bfk, 0.0), "kernel assumes zero ln2 beta for relu path"

    wbar = np.exp(-np.exp(td)).astype(f32)
    eu = np.exp(tf).astype(f32)

    def packc(v):
        return np.asarray(v, f32).reshape(-1, 128).T

    cst = np.zeros((128, NCOLS), f32)
    cst[:, CW:CW + 8] = packc(wbar)
    cst[:, CEU:CEU + 8] = packc(eu)
    cst[:, CBK:CBK + 8] = packc(bk)
    cst[:, CBV:CBV + 8] = packc(bv)
    cst[:, CBR:CBR + 8] = packc(br)
    cst[:, CFT:CFT + 8] = packc(ftmk)
    cst[:, CFT1:CFT1 + 8] = packc(1 - ftmk)
    cst[:, CBFR:CBFR + 8] = packc(bfr)
    cst[:, CEPS] = EPS
    cst[:, CBFK:CBFK + 32] = packc(bfk)

    def lerp_pair(W1, tm):
        # [128, 2(ab), 4(k2), 2(j), 1024] flat; a=0: W*tm, a=1: W*(1-tm)
        Wa = _pair_w(_q8s((W1 * tm[None, :]).T), C)
        Wb = _pair_w(_q8s((W1 * (1 - tm)[None, :]).T), C)
        return np.ascontiguousarray(
            np.stack([Wa.reshape(128, 4, 2, 1024),
                      Wb.reshape(128, 4, 2, 1024)], axis=1).reshape(128, -1))

    shared = {
        "wk": lerp_pair(Wk1, tmk),
        "wv": lerp_pair(Wv1, tmv),
        "wr": lerp_pair(Wr1, tmr),
        "wo": _fwv_hl(Wo.T),
        "fwr": _fwv_hl(fWr1.T),
        "fwk": _fwk_hl(fWk1.T),
        "fwv": _fwv_hl(fWv.T),
        "cst": cst,
        "ones1": np.ones((128, 1), f32),
        "onesb": np.ones((1, 128), ml_dtypes.bfloat16),
        "ident": np.eye(128, dtype=f32),
    }
    in_maps = [dict(shared, x=np.ascontiguousarray(x[b])) for b in range(B)]
    return in_maps


def _run(in_maps, trace=False, **kw):
    nc = _get_prog()
    res = run_bass_kernel_spmd(nc, in_maps, core_ids=list(range(B)), trace=trace,
                               **kw)
    out = np.stack([np.asarray(res.results[b]["out"]) for b in range(B)], axis=0)
    return out.astype(np.float32), res


def kernel(*a, **kw):
    out, _ = _run(_prep_inputs(*a, **kw))
    return out


if __name__ == "__main__":
    _get_prog()
    print("program built ok")
